# revision 13
# baseline (speedup 1.0000x reference)
"""GATv2 block (2 layers) on 8 Trainium2 NeuronCores via Bass/Tile — v3.

Structure vs v2 baseline:
- Edge source features gathered with bulk dma_gather (split-table for int16
  index range), table rows padded to 512B.
- Destination transform xr never round-trips DRAM: per-window xr tile stays in
  SBUF and is injected into the per-edge PSUM via the transpose ST of the
  aggregation one-hot S.
- v4: the leaky-relu runs directly on the Activation engine (AF.Lrelu with
  alpha=0.2), so gather-table rows carry plain x@W (128 fp16 = 256B): half the
  gather + AllGather traffic of the 0.6z+0.4|z| linear-rider scheme, and one
  ACT op per PSUM group instead of two (fewer activation-table switches).
- v4: edge channels are stored c-major (new col = c*H + h) so the exp*xs
  multiply and the softmax divide hit the DVE packed fast path; weights are
  permuted host-side and the output is unpermuted in kernel().
- layer-1 gather table (xl1 = x3 @ Wl1) is produced chunk-by-chunk inside
  layer-0's LayerNorm phase and AllGathered into a Shared-address DRAM tensor.
- host_prep fully vectorized (sort-by-(core,window,split) + scatter).

softmax num/den accumulate via one-hot segment matmul; BN stats via Gram
matrix AllReduce; graph-LN via one-hot segment matmuls.
"""
import sys
import math

sys.path.insert(0, '/opt/trn_rl_repo')

import numpy as np
import concourse.bass as bass
import concourse.tile as tile
from concourse import bacc, mybir
from concourse.bass_utils import run_bass_kernel_spmd

F32 = mybir.dt.float32
F16 = mybir.dt.float16
I16 = mybir.dt.int16
AF = mybir.ActivationFunctionType
ALU = mybir.AluOpType

P = 128
NCORE = 8
NEG = 0.2
EPS = 1e-5
ASHIFT = -4.0   # constant softmax shift: exp(alpha-4) keeps fp16 exp in range
EL = 128        # fp16 elements per gather-table row (256B)
SPLIT = 32768   # int16 index split point
REPS = 1
GMAX = 896      # max idxs per dma_gather op


# ----------------------------------------------------------------- host prep
def host_prep(x, node_batch, edge_index, edge_attr, Wl, bl, Wr, br, We, att,
              bias, Wres, W1, b1, bn_gamma, bn_beta, W2, b2, ln_gamma, ln_beta):
    N, D = x.shape
    E = edge_index.shape[1]
    ED = edge_attr.shape[1]
    L = Wl.shape[0]
    HID = W1.shape[2]
    G = int(node_batch.max()) + 1
    H = att.shape[1]
    C = att.shape[2]
    DA = D + H
    gpc = (G + NCORE - 1) // NCORE

    nb = np.asarray(node_batch).astype(np.int64)
    src = np.asarray(edge_index[0]).astype(np.int64)
    dst = np.asarray(edge_index[1]).astype(np.int64)
    ea = np.asarray(edge_attr, dtype=np.float32)
    xf = np.asarray(x, np.float32)

    gb = np.searchsorted(nb, np.arange(G + 1))
    n0s = np.array([gb[min(c * gpc, G)] for c in range(NCORE + 1)], dtype=np.int64)
    Nl = n0s[1:] - n0s[:-1]
    N_pad = int(math.ceil(max(Nl.max(), 1) / 512.0) * 512)
    W = N_pad // P
    NCH = N_pad // 512
    NPT = NCORE * N_pad

    core_of = np.searchsorted(n0s, np.arange(N), side='right') - 1
    glob_id = (core_of * N_pad + (np.arange(N) - n0s[core_of])).astype(np.int64)

    ecore = core_of[dst]
    gsrc = glob_id[src]
    dslot_all = dst - n0s[ecore]
    ewin_all = dslot_all // P
    eslot_all = dslot_all % P
    isB = (gsrc >= SPLIT).astype(np.int64)

    # Per (core, window, split): counts -> shared tile layout (max over cores).
    key = (ecore * W + ewin_all) * 2 + isB
    cnt2 = np.bincount(key, minlength=NCORE * W * 2).reshape(NCORE, W, 2)
    nA, nB = cnt2[..., 0], cnt2[..., 1]
    tA_w = np.maximum(np.ceil(nA.max(axis=0) / P).astype(np.int64), 1)
    tB_w = np.ceil(nB.max(axis=0) / P).astype(np.int64)
    T_w = tA_w + tB_w
    tstart = np.concatenate([[0], np.cumsum(T_w)])
    nT = int(tstart[-1])
    E_pad = nT * P
    tsA = np.concatenate([[0], np.cumsum(tA_w)])
    tsB = np.concatenate([[0], np.cumsum(tB_w)])
    baseA = tstart[:-1] * P
    baseB = baseA + tA_w * P
    colA = np.concatenate([[0], np.cumsum(tA_w * (P // 16))])
    colB = np.concatenate([[0], np.cumsum(tB_w * (P // 16))])
    LA = int(tsA[-1]) * P
    LB = int(tsB[-1]) * P

    # Stable sort by (core, window, split); rank within group gives each edge
    # a unique slot in its window's tile range.
    order = np.argsort(key, kind='stable')
    sk = key[order]
    starts = np.zeros(E, np.int64)
    gs = np.r_[0, np.flatnonzero(np.diff(sk)) + 1]
    starts[gs] = gs
    starts = np.maximum.accumulate(starts)
    rank = np.arange(E) - starts
    wo = ewin_all[order]
    bo = isB[order]
    co = ecore[order]
    pos = np.where(bo == 0, baseA[wo], baseB[wo]) + rank

    # c-major channel permutation: new col j = c*H + h holds old channel
    # h*C + c. Makes the exp*xs multiply and softmax divide DVE-packed.
    perm = np.array([h * C + c for c in range(C) for h in range(H)])
    inv_perm = np.argsort(perm)

    shared = {
        'iota_row': np.tile(np.arange(P, dtype=np.float16), (P, 1)),
        'giota_rep': np.tile(np.arange(gpc, dtype=np.float32), (P, 1)),
        'giota_col': np.arange(gpc, dtype=np.float32).reshape(gpc, 1),
        'ident': np.eye(P, dtype=np.float16),
        'ident32': np.eye(P, dtype=np.float32),
        'ones_col': np.ones((P, 1), np.float32),
        'ones_col16': np.ones((P, 1), np.float16),
        'ones_row': np.ones((1, 512), np.float32),
        'ones16': np.ones((1, P), np.float16),
        'ashift_col': np.full((P, 1), ASHIFT, np.float32),
    }
    WlA_f, blA_f, WrA_f = [], [], []
    for l in range(L):
        Wl_ = np.asarray(Wl[l], np.float32)
        Wr_ = np.asarray(Wr[l], np.float32)
        We_ = np.asarray(We[l], np.float32)
        bl_ = np.asarray(bl[l], np.float32)
        br_ = np.asarray(br[l], np.float32)
        # layer >= 1 inputs live in the permuted basis: permute weight ROWS.
        rp = perm if l >= 1 else np.arange(D)
        Wl_r = Wl_[rp]
        Wr_r = Wr_[rp]
        Wres_r = np.asarray(Wres[l], np.float32)[rp]
        WlA_f.append(Wl_r[:, perm])
        blA_f.append(bl_[perm])
        WrA_f.append(Wr_r[:, perm])
        shared[f'WlA{l}'] = WlA_f[l].astype(np.float16)
        shared[f'blA{l}'] = blA_f[l].reshape(1, D).astype(np.float16)
        shared[f'WrA{l}'] = WrA_f[l].astype(np.float16)
        wex = np.concatenate([We_, br_.reshape(1, D)], 0)
        shared[f'WeX{l}'] = wex[:, perm].astype(np.float16)
        shared[f'Wres{l}'] = Wres_r[:, perm].astype(np.float16)
        shared[f'combo{l}'] = np.asarray(bias[l], np.float32)[perm].astype(np.float16).reshape(1, D)
        # full att vector, c-major: [P, D]
        aC = np.asarray(att[l], np.float32).reshape(H * C)[perm].astype(np.float16)
        shared[f'attC{l}'] = np.tile(aC.reshape(1, D), (P, 1))
        shared[f'W1_{l}'] = np.asarray(W1[l], np.float32)[perm]
        w2 = np.asarray(W2[l], np.float32)[:, perm]
        shared[f'W2_{l}'] = np.concatenate(
            [w2[k * P:(k + 1) * P, :] for k in range(HID // P)], axis=1)
        shared[f'b2_{l}'] = np.asarray(b2[l], np.float32)[perm].reshape(1, D)
        shared[f'bng{l}'] = np.asarray(bn_gamma[l], np.float32).reshape(1, HID)
        shared[f'bnb{l}'] = np.asarray(bn_beta[l], np.float32).reshape(1, HID)
        shared[f'lng{l}'] = np.asarray(ln_gamma[l], np.float32)[perm].reshape(D, 1)
        shared[f'lnb{l}'] = np.asarray(ln_beta[l], np.float32)[perm].reshape(D, 1)

    # layer-0 host precomputes: gather table, xr0, resid0
    xl0 = np.zeros((NPT, EL), np.float16)
    xr0 = np.zeros((NCORE, P, W * D), np.float16)
    rs0 = np.zeros((NCORE, P, W * D), np.float16)
    for c in range(NCORE):
        xs = xf[n0s[c]:n0s[c + 1]]
        xl0[c * N_pad:c * N_pad + Nl[c], :D] = (xs @ WlA_f[0] + blA_f[0]).astype(np.float16)
        xrv = (xs @ WrA_f[0]).astype(np.float16)          # [Nl, D]
        rsv = (xs @ np.asarray(Wres[0], np.float32)[:, perm]
               + np.asarray(bias[0], np.float32)[perm]).astype(np.float16)
        pad_s = np.zeros((N_pad - Nl[c], D), np.float16)
        xr0[c] = np.concatenate([xrv, pad_s]).reshape(W, P, D).transpose(1, 0, 2).reshape(P, W * D)
        rs0[c] = np.concatenate([rsv, pad_s]).reshape(W, P, D).transpose(1, 0, 2).reshape(P, W * D)

    in_maps = []
    arangeP = np.arange(P)
    for c in range(NCORE):
        sel = co == c
        oc = order[sel]
        pc = pos[sel]
        wc = wo[sel]
        bc = bo[sel]
        rc = rank[sel]
        es = gsrc[oc]

        dflat = np.full(E_pad, -1.0, np.float32)
        dflat[pc] = eslot_all[oc]
        ST_h = (dflat[None, :] == arangeP[:, None]).astype(np.float16)
        dc = dflat.reshape(nT, P).T
        S_h = (dc[:, :, None] == arangeP[None, None, :]).astype(
            np.float16).reshape(P, nT * P)

        # per-layer edge transform, scattered to tile slots in the same
        # [e-slot, (t d)] layout the gather writes
        ea_oc = ea[oc]
        eaWs = {}
        for l in range(L):
            We_p = np.asarray(We[l], np.float32)[:, perm]
            br_p = np.asarray(br[l], np.float32)[perm]
            vals = (ea_oc @ We_p + br_p).astype(np.float16)
            full = np.zeros((E_pad, D), np.float16)
            full[pc] = vals
            eaWs[f'eaW{l}'] = np.ascontiguousarray(
                full.reshape(nT, P, D).transpose(1, 0, 2).reshape(P, nT * D))

        mA = bc == 0
        idxA_flat = np.zeros(LA, np.int64)
        idxA_flat[tsA[wc[mA]] * P + rc[mA]] = es[mA]
        idxA = np.concatenate(
            [idxA_flat[tsA[w] * P:tsA[w + 1] * P].reshape(-1, 16).T
             for w in range(W)], axis=1).astype(np.int16)
        idxA = np.tile(idxA, (8, 1))
        if LB:
            mB = ~mA
            idxB_flat = np.zeros(LB, np.int64)
            idxB_flat[tsB[wc[mB]] * P + rc[mB]] = es[mB] - SPLIT
            idxB = np.concatenate(
                [idxB_flat[tsB[w] * P:tsB[w + 1] * P].reshape(-1, 16).T
                 for w in range(W) if tB_w[w]], axis=1).astype(np.int16)
            idxB = np.tile(idxB, (8, 1))
        else:
            idxB = np.zeros((P, 16), np.int16)

        lg = nb[n0s[c]:n0s[c + 1]] - c * gpc
        batch = np.full(N_pad, -1.0, np.float32)
        batch[:Nl[c]] = lg.astype(np.float32)
        valid = np.zeros(N_pad, np.float32)
        valid[:Nl[c]] = 1.0
        cnt = np.maximum(gb[np.minimum(c * gpc + np.arange(1, gpc + 1), G)]
                         - gb[np.minimum(c * gpc + np.arange(gpc), G)], 1)
        im = dict(shared)
        im.update({
            'idxA': idxA,
            'idxB': idxB,
            'ST_h': ST_h,
            'S_h': S_h,
            **eaWs,
            'xl0': xl0,
            'xr0': xr0[c],
            'rs0': rs0[c],
            'batch_row': batch.reshape(1, N_pad).astype(np.float16),
            'batch_col': batch.reshape(W, P).T.copy(),
            'valid_col': valid.reshape(W, P).T.copy(),
            'invcntD': (1.0 / (cnt * D)).astype(np.float32).reshape(gpc, 1),
        })
        in_maps.append(im)

    dims = dict(N=N, D=D, E=E, ED=ED, L=L, HID=HID, G=G, H=H, C=C, gpc=gpc,
                N_pad=N_pad, W=W, NCH=NCH, NPT=NPT, nT=nT, E_pad=E_pad,
                T_w=[int(t) for t in T_w], tA_w=[int(t) for t in tA_w],
                tB_w=[int(t) for t in tB_w], tstart=[int(t) for t in tstart],
                colA=[int(t) for t in colA], colB=[int(t) for t in colB],
                nA_cols=int(colA[-1]), nB_cols=max(int(colB[-1]), 16),
                n0s=n0s, Nl=Nl, inv_perm=inv_perm)
    return in_maps, dims


# --------------------------------------------------------------- bass kernel
def build_nc(dims):
    D = dims['D']
    ED = dims['ED']
    L = dims['L']
    HID = dims['HID']
    H = dims['H']
    C = dims['C']
    DA = D + H
    gpc = dims['gpc']
    N_pad = dims['N_pad']
    W = dims['W']
    NCH = dims['NCH']
    NPT = dims['NPT']
    nT = dims['nT']
    E_pad = dims['E_pad']
    T_w = dims['T_w']
    tA_w = dims['tA_w']
    tB_w = dims['tB_w']
    tstart = dims['tstart']
    colA = dims['colA']
    colB = dims['colB']
    N = dims['N']
    HB = HID // P
    tpo = GMAX // P
    NB = min(REPS, 2)

    nc = bacc.Bacc("TRN2", target_bir_lowering=False, debug=False, num_devices=NCORE)

    def inp(name, shape, dt=F32):
        return nc.dram_tensor(name, list(shape), dt, kind="ExternalInput").ap()

    t_idxA = inp('idxA', (P, dims['nA_cols']), I16)
    t_idxB = inp('idxB', (P, dims['nB_cols']), I16)
    t_ST = inp('ST_h', (P, nT * P), F16)
    t_S = inp('S_h', (P, nT * P), F16)
    t_eaW = [inp(f'eaW{l}', (P, nT * D), F16) for l in range(L)]
    t_xl0 = inp('xl0', (NPT, EL), F16)
    t_xr0 = inp('xr0', (P, W * D), F16)
    t_rs0 = inp('rs0', (P, W * D), F16)
    t_batch_row = inp('batch_row', (1, N_pad), F16)
    t_batch_col = inp('batch_col', (P, W))
    t_valid_col = inp('valid_col', (P, W))
    t_invcntD = inp('invcntD', (gpc, 1))
    t_iota_row = inp('iota_row', (P, P), F16)
    t_giota_rep = inp('giota_rep', (P, gpc))
    t_giota_col = inp('giota_col', (gpc, 1))
    t_ident = inp('ident', (P, P), F16)
    t_ident32 = inp('ident32', (P, P), F32)
    t_ones_col = inp('ones_col', (P, 1))
    t_ones_col16 = inp('ones_col16', (P, 1), F16)
    t_ones_row = inp('ones_row', (1, 512))
    t_ones16 = inp('ones16', (1, P), F16)
    t_ashift = inp('ashift_col', (P, 1))
    tw = {}
    wspec = []
    for l in range(L):
        wspec += [(f'WlA{l}', (P, D), F16), (f'blA{l}', (1, D), F16),
                  (f'WrA{l}', (P, D), F16),
                  (f'Wres{l}', (P, D), F16), (f'combo{l}', (1, D), F16),
                  (f'attC{l}', (P, D), F16),
                  (f'W1_{l}', (P, HID), F32), (f'W2_{l}', (P, HID), F32),
                  (f'b2_{l}', (1, D), F32), (f'bng{l}', (1, HID), F32),
                  (f'bnb{l}', (1, HID), F32),
                  (f'lng{l}', (D, 1), F32), (f'lnb{l}', (D, 1), F32)]
    for key, shape, dt in wspec:
        tw[key] = inp(key, shape, dt)

    t_out = nc.dram_tensor('out_rows', [N_pad, D], F32, kind="ExternalOutput").ap()

    # layer-1 gather tables: AllGather output in Shared address space (fast
    # HBM-HBM collective path); input staged in Local scratch.
    t_xl1full = [nc.dram_tensor(f'xl1full{r}', [NPT, EL], F16,
                                kind="Internal", addr_space="Shared").ap()
                 for r in range(NB)]

    with tile.TileContext(nc) as tc:
        with tc.tile_pool(name="const", bufs=1) as cpool, \
             tc.tile_pool(name="dram", bufs=1, space="DRAM") as dpool, \
             tc.tile_pool(name="big", bufs=1) as bigpool:

            def ld(ap, shape, dt=F32, pool=cpool, name=None):
                if name is None:
                    name = 'c_' + ap.tensor.name
                t = pool.tile(list(shape), dt, name=name, tag=name)
                nc.sync.dma_start(t[:], ap[:])
                return t

            s_idxA = ld(t_idxA, (P, dims['nA_cols']), I16, bigpool)
            s_idxB = ld(t_idxB, (P, dims['nB_cols']), I16, bigpool)
            s_batch_col = ld(t_batch_col, (P, W))
            s_valid_col = ld(t_valid_col, (P, W))
            s_invcntD = ld(t_invcntD, (gpc, 1))
            s_iota_row = ld(t_iota_row, (P, P), F16)
            s_giota_rep = ld(t_giota_rep, (P, gpc))
            s_giota_col = ld(t_giota_col, (gpc, 1))
            s_ident = ld(t_ident, (P, P), F16)
            s_ident32 = ld(t_ident32, (P, P), F32)
            s_ones_col = ld(t_ones_col, (P, 1))
            s_ones_col16 = ld(t_ones_col16, (P, 1), F16)
            s_ones_row = ld(t_ones_row, (1, 512))
            s_ones16 = ld(t_ones16, (1, P), F16)
            s_ashift = ld(t_ashift, (P, 1))
            sw = {}
            for key, shape, dt in wspec:
                sw[key] = ld(tw[key], shape, dt)

            d_xl1loc_r = [dpool.tile([N_pad, EL], F16, tag=f'xl1loc{r}',
                                     name=f'd_xl1loc{r}') for r in range(NB)]
            d_arin = [dpool.tile([P, D + 1], F32, tag=f'arin{l}', name=f'd_arin{l}')
                      for l in range(L)]
            d_arout = [dpool.tile([P, D + 1], F32, tag=f'arout{l}', name=f'd_arout{l}')
                       for l in range(L)]

            x1Tb = [bigpool.tile([P, N_pad], F16, tag=f'x1T{i}', name=f'x1T{i}')
                    for i in range(NB)]
            x3Tb = [bigpool.tile([P, N_pad], F16, tag=f'x3T{i}', name=f'x3T{i}')
                    for i in range(NB)]

            from contextlib import ExitStack

            def emit_B(rep, l):
                x1T = x1Tb[rep % NB]
                x3T = x3Tb[rep % NB]
                tab = t_xl0 if l == 0 else t_xl1full[rep % NB]
                ctx = ExitStack()
                pC = ctx.enter_context(tc.tile_pool(name="pC", bufs=1))
                pCsp = ctx.enter_context(tc.tile_pool(name="pCs", bufs=1, space="PSUM"))
                pCs = pCsp.tile([P, D + 1], F32, tag='cs')
                with tc.tile_pool(name="pB", bufs=3) as pB, \
                     tc.tile_pool(name="pB1", bufs=3) as pB1, \
                     tc.tile_pool(name="pBz", bufs=3, space="PSUM") as pBz, \
                     tc.tile_pool(name="pBa", bufs=2, space="PSUM") as pBa, \
                     tc.tile_pool(name="pBr", bufs=1, space="PSUM") as pBr:
                    for w in range(W):
                        T = T_w[w]
                        tA = tA_w[w]
                        tB = tB_w[w]
                        tb = tstart[w]
                        EW = T * P
                        eaw = pB.tile([P, T * D], F16, tag='eaw')
                        nc.sync.dma_start(eaw[:], t_eaW[l][:, tb * D:(tb + T) * D])
                        xsv = pB.tile([P, T * EL], F16, tag='xsv')
                        xsr = xsv[:].rearrange("p (t q) -> p t q", q=EL)
                        for o in range(0, tA, tpo):
                            t0, t1 = o, min(o + tpo, tA)
                            ni = (t1 - t0) * P
                            nc.gpsimd.dma_gather(
                                xsr[:, t0:t1, :], tab,
                                s_idxA[:, (colA[w] + t0 * 8):(colA[w] + t1 * 8)],
                                ni, ni, EL)
                        for o in range(0, tB, tpo):
                            t0, t1 = o, min(o + tpo, tB)
                            ni = (t1 - t0) * P
                            nc.gpsimd.dma_gather(
                                xsr[:, tA + t0:tA + t1, :], tab[SPLIT:, :],
                                s_idxB[:, (colB[w] + t0 * 8):(colB[w] + t1 * 8)],
                                ni, ni, EL)
                        if l == 0:
                            xrw = pB.tile([P, D], F16, tag='xrw')
                            nc.sync.dma_start(xrw[:], t_xr0[:, w * D:(w + 1) * D])
                            rsw = pB.tile([P, D], F16, tag='rsw')
                            nc.sync.dma_start(rsw[:], t_rs0[:, w * D:(w + 1) * D])
                            xrw_ap = xrw[:]
                        else:
                            pxr = pBr.tile([P, D], F32, tag='pxr')
                            nc.tensor.matmul(pxr[:], lhsT=x3T[:, w * P:(w + 1) * P],
                                             rhs=sw['WrA1'][:], start=True, stop=True)
                            xrw = pB.tile([P, D], F16, tag='xrw')
                            nc.scalar.activation(out=xrw[:], in_=pxr[:], func=AF.Copy)
                            xrw_ap = xrw[:]
                        # S / ST one-hots: host-built, DMA-loaded
                        S = pB1.tile([P, EW], F16, tag='S')
                        nc.sync.dma_start(S[:], t_S[:, tb * P:tb * P + EW])
                        ST = pB1.tile([P, EW], F16, tag='ST')
                        nc.sync.dma_start(ST[:], t_ST[:, tb * P:tb * P + EW])
                        m16 = pB1.tile([P, T * D], F16, tag='m16')
                        KP = 4
                        for g0 in range(0, T, KP):
                            gn = min(KP, T - g0)
                            pz3 = pBz.tile([P, KP * D], F32, tag='pz')
                            for u in range(gn):
                                t = g0 + u
                                sl = slice(u * D, (u + 1) * D)
                                nc.tensor.matmul(pz3[:, sl], lhsT=ST[:, t * P:(t + 1) * P],
                                                 rhs=xrw_ap,
                                                 start=True, stop=True,
                                                 skip_group_check=True)
                            pzv = pz3[:].rearrange("p (u q) -> p u q", q=D)
                            mv = m16[:, g0 * D:(g0 + gn) * D].rearrange("p (u n) -> p u n", n=D)
                            nc.vector.tensor_tensor(out=mv, in0=xsr[:, g0:g0 + gn, 0:D],
                                                    in1=eaw[:].rearrange(
                                                        "p (t n) -> p t n", n=D)[:, g0:g0 + gn, :],
                                                    op=ALU.add)
                            nc.vector.tensor_tensor(out=mv, in0=mv,
                                                    in1=pzv[:, 0:gn, :],
                                                    op=ALU.add)
                            nc.scalar.activation(out=mv, in_=mv, func=AF.Prelu, alpha=NEG)
                        eng_tt = nc.vector if (w % 2 == 0) else nc.gpsimd
                        eng_tt.tensor_tensor(
                            out=m16[:].rearrange("p (t n) -> p t n", t=T),
                            in0=m16[:].rearrange("p (t n) -> p t n", t=T),
                            in1=sw[f'attC{l}'][:].rearrange("p (o n) -> p o n", o=1).to_broadcast([P, T, P]),
                            op=ALU.mult)
                        alpha2 = pB.tile([P, T * H], F32, tag='alpha2')
                        with nc.allow_low_precision(reason="16 fp16 terms, |alpha|<~30"):
                            nc.vector.tensor_reduce(
                                out=alpha2[:],
                                in_=m16[:].rearrange("p (t c h) -> p t h c", c=C, h=H),
                                axis=mybir.AxisListType.X, op=ALU.add)
                        ybuf = pB.tile([P, T * (D + 8)], F16, tag='ybuf')
                        yv = ybuf[:].rearrange("p (t q) -> p t q", q=D + 8)
                        nc.scalar.activation(
                            out=yv[:, :, D:D + 8],
                            in_=alpha2[:].rearrange("p (t h) -> p t h", t=T),
                            func=AF.Exp, bias=s_ashift[:])
                        nc.vector.tensor_tensor(
                            out=yv[:, :, 0:D].rearrange("p t (c h) -> p t c h", c=C),
                            in0=xsr[:, :, 0:D].rearrange("p t (c h) -> p t c h", c=C),
                            in1=yv[:, :, D:D + 8].rearrange("p t (o h) -> p t o h", o=1).to_broadcast([P, T, C, H]),
                            op=ALU.mult)
                        pagg = pBa.tile([P, D + 8], F32, tag='pagg')
                        for t in range(T):
                            nc.tensor.matmul(pagg[:], lhsT=S[:, t * P:(t + 1) * P],
                                             rhs=yv[:, t, :], start=(t == 0),
                                             stop=(t == T - 1))
                        den = pB.tile([P, H], F32, tag='den')
                        nc.vector.tensor_scalar(out=den[:], in0=pagg[:, D:D + 8],
                                                scalar1=1e-16, scalar2=None, op0=ALU.add)
                        rec = pB.tile([P, H], F32, tag='rec')
                        nc.vector.reciprocal(rec[:], den[:])
                        x1w = pB.tile([P, D + 1], F16, tag='x1w')
                        nc.vector.tensor_tensor(
                            out=x1w[:, 0:D].rearrange("p (c h) -> p c h", c=C),
                            in0=pagg[:, 0:D].rearrange("p (c h) -> p c h", c=C),
                            in1=rec[:].rearrange("p (o h) -> p o h", o=1).to_broadcast([P, C, H]),
                            op=ALU.mult)
                        if l == 0:
                            nc.vector.tensor_add(x1w[:, 0:D], x1w[:, 0:D],
                                                 rsw[:])
                        else:
                            pres = pBr.tile([P, P], F32, tag='pxr')
                            nc.tensor.matmul(pres[:], lhsT=x3T[:, w * P:(w + 1) * P],
                                             rhs=sw['Wres1'][:], start=True, stop=False)
                            nc.tensor.matmul(pres[:], lhsT=s_ones16[:, 0:1].to_broadcast([1, P]),
                                             rhs=sw['combo1'][:], start=False, stop=True)
                            nc.vector.tensor_add(x1w[:, 0:D], x1w[:, 0:D], pres[:])
                        nc.vector.tensor_scalar(out=x1w[:, 0:D], in0=x1w[:, 0:D],
                                                scalar1=s_valid_col[:, w:w + 1],
                                                scalar2=None, op0=ALU.mult)
                        nc.vector.tensor_copy(x1w[:, D:D + 1], s_valid_col[:, w:w + 1])
                        nc.tensor.matmul(pCs[:], lhsT=x1w[:, 0:D], rhs=x1w[:, 0:D + 1],
                                         start=(w == 0), stop=(w == W - 1),
                                         skip_group_check=True)
                        ptr = pBr.tile([P, P], F16, tag='ptr')
                        nc.tensor.transpose(out=ptr[:], in_=x1w[:, 0:D], identity=s_ident[:])
                        nc.scalar.activation(out=x1T[:, w * P:(w + 1) * P],
                                             in_=ptr[:], func=AF.Copy)
                return ctx, pC, pCs

            def emit_CDE(rep, l, ctx, pC, pCs):
                x1T = x1Tb[rep % NB]
                x3T = x3Tb[rep % NB]
                # ======== Phase C: BN stats (AllReduce) ==================
                pCp_cm = tc.tile_pool(name="pCp", bufs=1, space="PSUM")
                pCp = pCp_cm.__enter__()
                cs_sb = pC.tile([P, D + 1], F32, tag='cs')
                nc.vector.tensor_copy(cs_sb[:], pCs[:])
                nc.sync.dma_start(d_arin[l][:], cs_sb[:])
                nc.gpsimd.collective_compute(
                    "AllReduce", ALU.add,
                    replica_groups=[list(range(NCORE))],
                    ins=[d_arin[l][:].opt()], outs=[d_arout[l][:].opt()])
                csr = pC.tile([P, D + 1], F32, tag='csr')
                nc.sync.dma_start(csr[:], d_arout[l][:])
                mu = pC.tile([P, 1], F32, tag='mu')
                nc.vector.tensor_scalar(out=mu[:], in0=csr[:, D:D + 1],
                                        scalar1=1.0 / N, scalar2=None, op0=ALU.mult)
                pmu = pCp.tile([1, HID], F32, tag='pmu')
                nc.tensor.matmul(pmu[:], lhsT=mu[:], rhs=sw[f'W1_{l}'][:],
                                 start=True, stop=True)
                pP1 = pCp.tile([P, HID], F32, tag='pP1')
                nc.tensor.matmul(pP1[:], lhsT=csr[:, 0:D], rhs=sw[f'W1_{l}'][:],
                                 start=True, stop=True)
                w1p1 = pC.tile([P, HID], F32, tag='w1p1')
                nc.vector.tensor_tensor(out=w1p1[:], in0=sw[f'W1_{l}'][:],
                                        in1=pP1[:], op=ALU.mult)
                pt2 = pCp.tile([1, HID], F32, tag='pt2')
                nc.tensor.matmul(pt2[:], lhsT=s_ones_col[:], rhs=w1p1[:],
                                 start=True, stop=True)
                mh = pC.tile([1, HID], F32, tag='mh')
                nc.vector.tensor_copy(mh[:], pmu[:])
                var = pC.tile([1, HID], F32, tag='var')
                nc.vector.tensor_scalar(out=var[:], in0=pt2[:], scalar1=1.0 / N,
                                        scalar2=None, op0=ALU.mult)
                m2 = pC.tile([1, HID], F32, tag='m2')
                nc.vector.tensor_tensor(out=m2[:], in0=mh[:], in1=mh[:], op=ALU.mult)
                nc.vector.tensor_tensor(out=var[:], in0=var[:], in1=m2[:], op=ALU.subtract)
                sd = pC.tile([1, HID], F32, tag='sd')
                nc.vector.tensor_scalar(out=var[:], in0=var[:], scalar1=EPS,
                                        scalar2=None, op0=ALU.add)
                nc.scalar.activation(out=sd[:], in_=var[:], func=AF.Sqrt)
                rsd = pC.tile([1, HID], F32, tag='rsd')
                nc.vector.reciprocal(rsd[:], sd[:])
                geff = pC.tile([1, HID], F32, tag='geff')
                nc.vector.tensor_tensor(out=geff[:], in0=sw[f'bng{l}'][:],
                                        in1=rsd[:], op=ALU.mult)
                beff = pC.tile([1, HID], F32, tag='beff')
                nc.vector.tensor_tensor(out=beff[:], in0=mh[:], in1=geff[:], op=ALU.mult)
                nc.vector.tensor_tensor(out=beff[:], in0=sw[f'bnb{l}'][:],
                                        in1=beff[:], op=ALU.subtract)
                pgrep = pCp.tile([P, HID], F32, tag='pgrep')
                nc.tensor.matmul(pgrep[:], lhsT=s_ones_col[:1, :].rearrange("o p -> p o").to_broadcast([1, P]),
                                 rhs=geff[:], start=True, stop=True)
                w1eff = pC.tile([P, HID], F16, tag='w1eff')
                nc.vector.tensor_tensor(out=w1eff[:], in0=sw[f'W1_{l}'][:],
                                        in1=pgrep[:], op=ALU.mult)
                becol = pC.tile([P, HB], F32, tag='becol')
                for k in range(HB):
                    ptb = pCp.tile([P, 1], F32, tag='ptb')
                    nc.tensor.transpose(out=ptb[:], in_=beff[:, k * P:(k + 1) * P],
                                        identity=s_ident32[:1, :1])
                    nc.vector.tensor_copy(becol[:, k:k + 1], ptb[:])
                pCp_cm.__exit__(None, None, None)

                # ======== Phase D: MLP (x2 overwrites x1T in place) ======
                with tc.tile_pool(name="pD", bufs=2) as pD, \
                     tc.tile_pool(name="pDp", bufs=2, space="PSUM") as pDp, \
                     tc.tile_pool(name="pDx", bufs=2, space="PSUM") as pDx:
                    for i in range(NCH):
                        c0 = i * 512
                        px2 = pDx.tile([P, 512], F32, tag='px2')
                        for k in range(HB):
                            ph = pDp.tile([P, 512], F32, tag='ph')
                            nc.tensor.matmul(ph[:], lhsT=w1eff[:, k * P:(k + 1) * P],
                                             rhs=x1T[:, c0:c0 + 512],
                                             start=True, stop=True)
                            hs = pD.tile([P, 512], F32, tag='hs')
                            nc.scalar.activation(out=hs[:], in_=ph[:], func=AF.Relu,
                                                 bias=becol[:, k:k + 1], scale=1.0)
                            nc.tensor.matmul(px2[:], lhsT=sw[f'W2_{l}'][:, k * P:(k + 1) * P],
                                             rhs=hs[:], start=(k == 0), stop=False,
                                             skip_group_check=True)
                        nc.tensor.matmul(px2[:], lhsT=sw[f'b2_{l}'][:],
                                         rhs=s_ones_row[:], start=False, stop=True,
                                         skip_group_check=True)
                        nc.vector.tensor_add(x1T[:, c0:c0 + 512], px2[:],
                                             x1T[:, c0:c0 + 512])

                # ======== Phase E: graph LayerNorm (+ fused layer-1 table
                # production and AllGather when l == 0) ====================
                with tc.tile_pool(name="pE", bufs=2) as pE, \
                     tc.tile_pool(name="pEg", bufs=1, space="PSUM") as pEgp, \
                     tc.tile_pool(name="pEp", bufs=1, space="PSUM") as pEp:
                    pgs = pEgp.tile([gpc, 2], F32, tag='pgs')
                    for w in range(W):
                        sl = slice(w * P, (w + 1) * P)
                        sq = pE.tile([P, P], F16, tag='sq')
                        nc.scalar.activation(out=sq[:], in_=x1T[:, sl],
                                             func=AF.Square)
                        pcs = pEp.tile([1, 2 * P], F32, tag='pcs')
                        nc.tensor.matmul(pcs[:, 0:P], lhsT=s_ones_col16[:], rhs=x1T[:, sl],
                                         start=True, stop=True, skip_group_check=True)
                        nc.tensor.matmul(pcs[:, P:2 * P], lhsT=s_ones_col16[:], rhs=sq[:],
                                         start=True, stop=True, skip_group_check=True)
                        rows = pE.tile([1, 2 * P], F32, tag='rows')
                        nc.vector.tensor_copy(rows[:], pcs[:])
                        csc = pE.tile([P, 2], F32, tag='csc')
                        for q in range(2):
                            ptb = pEp.tile([P, 1], F32, tag='ptb2')
                            nc.tensor.transpose(out=ptb[:], in_=rows[:, q * P:(q + 1) * P],
                                                identity=s_ident32[:1, :1])
                            nc.vector.tensor_copy(csc[:, q:q + 1], ptb[:])
                        bg = pE.tile([P, gpc], F32, tag='bg')
                        nc.vector.tensor_scalar(out=bg[:], in0=s_giota_rep[:],
                                                scalar1=s_batch_col[:, w:w + 1],
                                                scalar2=None, op0=ALU.is_equal)
                        nc.tensor.matmul(pgs[:], lhsT=bg[:], rhs=csc[:],
                                         start=(w == 0), stop=(w == W - 1),
                                         skip_group_check=True)
                    gm = pE.tile([gpc, 1], F32, tag='gm')
                    nc.vector.tensor_tensor(out=gm[:], in0=pgs[:, 0:1],
                                            in1=s_invcntD[:], op=ALU.mult)
                    e2 = pE.tile([gpc, 1], F32, tag='e2')
                    nc.vector.tensor_tensor(out=e2[:], in0=pgs[:, 1:2],
                                            in1=s_invcntD[:], op=ALU.mult)
                    gv = pE.tile([gpc, 1], F32, tag='gv')
                    nc.vector.tensor_tensor(out=gv[:], in0=gm[:], in1=gm[:], op=ALU.mult)
                    nc.vector.tensor_tensor(out=gv[:], in0=e2[:], in1=gv[:], op=ALU.subtract)
                    sdg = pE.tile([gpc, 1], F32, tag='sdg')
                    nc.vector.tensor_scalar(out=gv[:], in0=gv[:], scalar1=EPS,
                                            scalar2=None, op0=ALU.add)
                    nc.scalar.activation(out=sdg[:], in_=gv[:], func=AF.Sqrt)
                    ivg = pE.tile([gpc, 1], F32, tag='ivg')
                    nc.vector.reciprocal(ivg[:], sdg[:])
                    gmr = pE.tile([gpc, P], F32, tag='gmr')
                    nc.vector.tensor_copy(gmr[:], gm[:].to_broadcast([gpc, P]))
                    ivr = pE.tile([gpc, P], F32, tag='ivr')
                    nc.vector.tensor_copy(ivr[:], ivg[:].to_broadcast([gpc, P]))
                    for i in range(NCH):
                        c0 = i * 512
                        brc = pE.tile([1, 512], F16, tag='brc')
                        nc.sync.dma_start(brc[:], t_batch_row[:, c0:c0 + 512])
                        pbr = pEp.tile([gpc, 512], F32, tag='pbr')
                        nc.tensor.matmul(pbr[:],
                                         lhsT=s_ones16[:, 0:1].to_broadcast([1, gpc]),
                                         rhs=brc[:],
                                         start=True, stop=True)
                        bgT = pE.tile([gpc, 512], F32, tag='bgT')
                        nc.vector.tensor_scalar(out=bgT[:], in0=pbr[:],
                                                scalar1=s_giota_col[:],
                                                scalar2=None, op0=ALU.is_equal)
                        pgm = pEp.tile([P, 512], F32, tag='pgm')
                        nc.tensor.matmul(pgm[:], lhsT=gmr[:], rhs=bgT[:],
                                         start=True, stop=True)
                        piv = pEp.tile([P, 512], F32, tag='piv')
                        nc.tensor.matmul(piv[:], lhsT=ivr[:], rhs=bgT[:],
                                         start=True, stop=True)
                        tmp = pE.tile([P, 512], F32, tag='tmp')
                        nc.vector.tensor_tensor(out=tmp[:], in0=x1T[:, c0:c0 + 512],
                                                in1=pgm[:], op=ALU.subtract)
                        nc.vector.tensor_tensor(out=tmp[:], in0=tmp[:],
                                                in1=piv[:], op=ALU.mult)
                        if l == 0:
                            nc.vector.tensor_scalar(out=x3T[:, c0:c0 + 512], in0=tmp[:],
                                                    scalar1=sw[f'lng{l}'][:],
                                                    scalar2=sw[f'lnb{l}'][:],
                                                    op0=ALU.mult, op1=ALU.add)
                            # fused layer-1 gather-table production
                            xa = pE.tile([P, 4 * D], F16, tag='xa')
                            for q in range(4):
                                pxa = pEp.tile([P, D], F32, tag='pxa')
                                nc.tensor.matmul(
                                    pxa[:], lhsT=x3T[:, c0 + q * P:c0 + (q + 1) * P],
                                    rhs=sw['WlA1'][:], start=True, stop=False)
                                nc.tensor.matmul(
                                    pxa[:], lhsT=s_ones16[:, 0:1].to_broadcast([1, P]),
                                    rhs=sw['blA1'][:], start=False, stop=True)
                                nc.vector.tensor_copy(xa[:, q * D:(q + 1) * D], pxa[:])
                            nc.sync.dma_start(
                                d_xl1loc_r[rep % NB][c0:c0 + 512, 0:D].rearrange(
                                    "(q p) d -> p q d", p=P),
                                xa[:].rearrange("p (q d) -> p q d", d=D))
                        else:
                            x3c = pE.tile([P, 512], F16, tag='x3c')
                            nc.vector.tensor_scalar(out=x3c[:], in0=tmp[:],
                                                    scalar1=sw[f'lng{l}'][:],
                                                    scalar2=sw[f'lnb{l}'][:],
                                                    op0=ALU.mult, op1=ALU.add)
                            for q in range(4):
                                ptb2 = pEp.tile([P, P], F16, tag='ptb2')
                                nc.tensor.transpose(out=ptb2[:],
                                                    in_=x3c[:, q * P:(q + 1) * P],
                                                    identity=s_ident[:])
                                orow = pE.tile([P, P], F32, tag='orow')
                                nc.vector.tensor_copy(orow[:], ptb2[:])
                                r0 = c0 + q * P
                                nc.sync.dma_start(t_out[r0:r0 + P, :], orow[:])
                if l == 0:
                    nc.gpsimd.collective_compute(
                        "AllGather", ALU.bypass,
                        replica_groups=[list(range(NCORE))],
                        ins=[d_xl1loc_r[rep % NB][:].opt()],
                        outs=[t_xl1full[rep % NB][:].opt()])
                ctx.close()

            for r in range(REPS):
                for l in range(L):
                    ctx, pC, pCs = emit_B(r, l)
                    emit_CDE(r, l, ctx, pC, pCs)

    nc.compile()
    return nc


# ---------------------------------------------------------------- entry point
_CACHE = {}


def kernel(**inputs):
    in_maps, dims = host_prep(**inputs)
    key = (REPS, dims['N'], dims['E'], dims['N_pad'], dims['nT'],
           tuple(dims['T_w']), tuple(dims['tA_w']))
    if key not in _CACHE:
        _CACHE[key] = build_nc(dims)
    nc = _CACHE[key]
    res = run_bass_kernel_spmd(nc, in_maps, core_ids=list(range(NCORE)), trace=False)
    global _last_res, _last_dims
    _last_res, _last_dims = res, dims
    N, D = dims['N'], dims['D']
    out = np.zeros((N, D), dtype=np.float32)
    n0s, Nl = dims['n0s'], dims['Nl']
    inv = dims['inv_perm']
    for c in range(NCORE):
        out[n0s[c]:n0s[c + 1]] = res.results[c]['out_rows'][:Nl[c]][:, inv]
    return out


# revision 14
# speedup vs baseline: 1.1918x; 1.1918x over previous
"""GATv2 block (2 layers) on 8 Trainium2 NeuronCores via Bass/Tile — v3.

Structure vs v2 baseline:
- Edge source features gathered with bulk dma_gather (split-table for int16
  index range), table rows padded to 512B.
- Destination transform xr never round-trips DRAM: per-window xr tile stays in
  SBUF and is injected into the per-edge PSUM via the transpose ST of the
  aggregation one-hot S.
- v4: the leaky-relu runs directly on the Activation engine (AF.Lrelu with
  alpha=0.2), so gather-table rows carry plain x@W (128 fp16 = 256B): half the
  gather + AllGather traffic of the 0.6z+0.4|z| linear-rider scheme, and one
  ACT op per PSUM group instead of two (fewer activation-table switches).
- v4: edge channels are stored c-major (new col = c*H + h) so the exp*xs
  multiply and the softmax divide hit the DVE packed fast path; weights are
  permuted host-side and the output is unpermuted in kernel().
- layer-1 gather table (xl1 = x3 @ Wl1) is produced chunk-by-chunk inside
  layer-0's LayerNorm phase and AllGathered into a Shared-address DRAM tensor.
- host_prep fully vectorized (sort-by-(core,window,split) + scatter).

softmax num/den accumulate via one-hot segment matmul; BN stats via Gram
matrix AllReduce; graph-LN via one-hot segment matmuls.
"""
import sys
import math

sys.path.insert(0, '/opt/trn_rl_repo')

import numpy as np
import concourse.bass as bass
import concourse.tile as tile
from concourse import bacc, mybir
from concourse.bass_utils import run_bass_kernel_spmd

F32 = mybir.dt.float32
F16 = mybir.dt.float16
I16 = mybir.dt.int16
AF = mybir.ActivationFunctionType
ALU = mybir.AluOpType

P = 128
NCORE = 8
NEG = 0.2
EPS = 1e-5
ASHIFT = -4.0   # constant softmax shift: exp(alpha-4) keeps fp16 exp in range
EL = 128        # fp16 elements per gather-table row (256B)
SPLIT = 32768   # int16 index split point
REPS = 1
GMAX = 896      # max idxs per dma_gather op


# ----------------------------------------------------------------- host prep
def host_prep(x, node_batch, edge_index, edge_attr, Wl, bl, Wr, br, We, att,
              bias, Wres, W1, b1, bn_gamma, bn_beta, W2, b2, ln_gamma, ln_beta):
    N, D = x.shape
    E = edge_index.shape[1]
    ED = edge_attr.shape[1]
    L = Wl.shape[0]
    HID = W1.shape[2]
    G = int(node_batch.max()) + 1
    H = att.shape[1]
    C = att.shape[2]
    DA = D + H
    gpc = (G + NCORE - 1) // NCORE

    nb = np.asarray(node_batch).astype(np.int64)
    src = np.asarray(edge_index[0]).astype(np.int64)
    dst = np.asarray(edge_index[1]).astype(np.int64)
    ea = np.asarray(edge_attr, dtype=np.float32)
    xf = np.asarray(x, np.float32)

    gb = np.searchsorted(nb, np.arange(G + 1))
    n0s = np.array([gb[min(c * gpc, G)] for c in range(NCORE + 1)], dtype=np.int64)
    Nl = n0s[1:] - n0s[:-1]
    N_pad = int(math.ceil(max(Nl.max(), 1) / 512.0) * 512)
    W = N_pad // P
    NCH = N_pad // 512
    NPT = NCORE * N_pad

    core_of = np.searchsorted(n0s, np.arange(N), side='right') - 1
    glob_id = (core_of * N_pad + (np.arange(N) - n0s[core_of])).astype(np.int64)

    ecore = core_of[dst]
    gsrc = glob_id[src]
    dslot_all = dst - n0s[ecore]
    ewin_all = dslot_all // P
    eslot_all = dslot_all % P
    isB = (gsrc >= SPLIT).astype(np.int64)

    # Per (core, window, split): counts -> shared tile layout (max over cores).
    key = (ecore * W + ewin_all) * 2 + isB
    cnt2 = np.bincount(key, minlength=NCORE * W * 2).reshape(NCORE, W, 2)
    nA, nB = cnt2[..., 0], cnt2[..., 1]
    tA_w = np.maximum(np.ceil(nA.max(axis=0) / P).astype(np.int64), 1)
    tB_w = np.ceil(nB.max(axis=0) / P).astype(np.int64)
    T_w = tA_w + tB_w
    tstart = np.concatenate([[0], np.cumsum(T_w)])
    nT = int(tstart[-1])
    E_pad = nT * P
    tsA = np.concatenate([[0], np.cumsum(tA_w)])
    tsB = np.concatenate([[0], np.cumsum(tB_w)])
    baseA = tstart[:-1] * P
    baseB = baseA + tA_w * P
    colA = np.concatenate([[0], np.cumsum(tA_w * (P // 16))])
    colB = np.concatenate([[0], np.cumsum(tB_w * (P // 16))])
    LA = int(tsA[-1]) * P
    LB = int(tsB[-1]) * P

    # Stable sort by (core, window, split); rank within group gives each edge
    # a unique slot in its window's tile range.
    order = np.argsort(key, kind='stable')
    sk = key[order]
    starts = np.zeros(E, np.int64)
    gs = np.r_[0, np.flatnonzero(np.diff(sk)) + 1]
    starts[gs] = gs
    starts = np.maximum.accumulate(starts)
    rank = np.arange(E) - starts
    wo = ewin_all[order]
    bo = isB[order]
    co = ecore[order]
    pos = np.where(bo == 0, baseA[wo], baseB[wo]) + rank

    # c-major channel permutation: new col j = c*H + h holds old channel
    # h*C + c. Makes the exp*xs multiply and softmax divide DVE-packed.
    perm = np.array([h * C + c for c in range(C) for h in range(H)])
    inv_perm = np.argsort(perm)

    shared = {
        'iota_row': np.tile(np.arange(P, dtype=np.float16), (P, 1)),
        'giota_rep': np.tile(np.arange(gpc, dtype=np.float32), (P, 1)),
        'giota_col': np.arange(gpc, dtype=np.float32).reshape(gpc, 1),
        'ident': np.eye(P, dtype=np.float16),
        'ident32': np.eye(P, dtype=np.float32),
        'ones_col': np.ones((P, 1), np.float32),
        'ones_col16': np.ones((P, 1), np.float16),
        'ones_row': np.ones((1, 512), np.float32),
        'ones16': np.ones((1, P), np.float16),
        'ashift_col': np.full((P, 1), ASHIFT, np.float32),
    }
    WlA_f, blA_f, WrA_f = [], [], []
    for l in range(L):
        Wl_ = np.asarray(Wl[l], np.float32)
        Wr_ = np.asarray(Wr[l], np.float32)
        We_ = np.asarray(We[l], np.float32)
        bl_ = np.asarray(bl[l], np.float32)
        br_ = np.asarray(br[l], np.float32)
        # layer >= 1 inputs live in the permuted basis: permute weight ROWS.
        rp = perm if l >= 1 else np.arange(D)
        Wl_r = Wl_[rp]
        Wr_r = Wr_[rp]
        Wres_r = np.asarray(Wres[l], np.float32)[rp]
        WlA_f.append(Wl_r[:, perm])
        blA_f.append(bl_[perm])
        WrA_f.append(Wr_r[:, perm])
        shared[f'WlA{l}'] = WlA_f[l].astype(np.float16)
        shared[f'blA{l}'] = blA_f[l].reshape(1, D).astype(np.float16)
        shared[f'WrA{l}'] = WrA_f[l].astype(np.float16)
        wex = np.concatenate([We_, br_.reshape(1, D)], 0)
        shared[f'WeX{l}'] = wex[:, perm].astype(np.float16)
        shared[f'Wres{l}'] = Wres_r[:, perm].astype(np.float16)
        shared[f'combo{l}'] = np.asarray(bias[l], np.float32)[perm].astype(np.float16).reshape(1, D)
        # full att vector, c-major: [P, D]
        aC = np.asarray(att[l], np.float32).reshape(H * C)[perm].astype(np.float16)
        shared[f'attC{l}'] = np.tile(aC.reshape(1, D), (P, 1))
        shared[f'W1_{l}'] = np.asarray(W1[l], np.float32)[perm]
        w2 = np.asarray(W2[l], np.float32)[:, perm]
        shared[f'W2_{l}'] = np.concatenate(
            [w2[k * P:(k + 1) * P, :] for k in range(HID // P)], axis=1)
        shared[f'b2_{l}'] = np.asarray(b2[l], np.float32)[perm].reshape(1, D)
        shared[f'bng{l}'] = np.asarray(bn_gamma[l], np.float32).reshape(1, HID)
        shared[f'bnb{l}'] = np.asarray(bn_beta[l], np.float32).reshape(1, HID)
        shared[f'lng{l}'] = np.asarray(ln_gamma[l], np.float32)[perm].reshape(D, 1)
        shared[f'lnb{l}'] = np.asarray(ln_beta[l], np.float32)[perm].reshape(D, 1)

    # layer-0 host precomputes: gather table, xr0, resid0
    xl0 = np.zeros((NPT, EL), np.float16)
    xr0 = np.zeros((NCORE, P, W * D), np.float16)
    rs0 = np.zeros((NCORE, P, W * D), np.float16)
    for c in range(NCORE):
        xs = xf[n0s[c]:n0s[c + 1]]
        xl0[c * N_pad:c * N_pad + Nl[c], :D] = (xs @ WlA_f[0] + blA_f[0]).astype(np.float16)
        xrv = (xs @ WrA_f[0]).astype(np.float16)          # [Nl, D]
        rsv = (xs @ np.asarray(Wres[0], np.float32)[:, perm]
               + np.asarray(bias[0], np.float32)[perm]).astype(np.float16)
        pad_s = np.zeros((N_pad - Nl[c], D), np.float16)
        xr0[c] = np.concatenate([xrv, pad_s]).reshape(W, P, D).transpose(1, 0, 2).reshape(P, W * D)
        rs0[c] = np.concatenate([rsv, pad_s]).reshape(W, P, D).transpose(1, 0, 2).reshape(P, W * D)

    in_maps = []
    arangeP = np.arange(P)
    for c in range(NCORE):
        sel = co == c
        oc = order[sel]
        pc = pos[sel]
        wc = wo[sel]
        bc = bo[sel]
        rc = rank[sel]
        es = gsrc[oc]

        dflat = np.full(E_pad, -1.0, np.float32)
        dflat[pc] = eslot_all[oc]
        ST_h = (dflat[None, :] == arangeP[:, None]).astype(np.float16)
        dc = dflat.reshape(nT, P).T
        S_h = (dc[:, :, None] == arangeP[None, None, :]).astype(
            np.float16).reshape(P, nT * P)

        eaf = np.zeros((ED + 1, E_pad), np.float16)
        eaf[:ED, pc] = ea[oc].T
        eaf[ED, pc] = 1.0

        mA = bc == 0
        idxA_flat = np.zeros(LA, np.int64)
        idxA_flat[tsA[wc[mA]] * P + rc[mA]] = es[mA]
        idxA = np.concatenate(
            [idxA_flat[tsA[w] * P:tsA[w + 1] * P].reshape(-1, 16).T
             for w in range(W)], axis=1).astype(np.int16)
        idxA = np.tile(idxA, (8, 1))
        if LB:
            mB = ~mA
            idxB_flat = np.zeros(LB, np.int64)
            idxB_flat[tsB[wc[mB]] * P + rc[mB]] = es[mB] - SPLIT
            idxB = np.concatenate(
                [idxB_flat[tsB[w] * P:tsB[w + 1] * P].reshape(-1, 16).T
                 for w in range(W) if tB_w[w]], axis=1).astype(np.int16)
            idxB = np.tile(idxB, (8, 1))
        else:
            idxB = np.zeros((P, 16), np.int16)

        lg = nb[n0s[c]:n0s[c + 1]] - c * gpc
        batch = np.full(N_pad, -1.0, np.float32)
        batch[:Nl[c]] = lg.astype(np.float32)
        valid = np.zeros(N_pad, np.float32)
        valid[:Nl[c]] = 1.0
        cnt = np.maximum(gb[np.minimum(c * gpc + np.arange(1, gpc + 1), G)]
                         - gb[np.minimum(c * gpc + np.arange(gpc), G)], 1)
        im = dict(shared)
        im.update({
            'idxA': idxA,
            'idxB': idxB,
            'ST_h': ST_h,
            'S_h': S_h,
            'eaT': eaf,
            'xl0': xl0,
            'xr0': xr0[c],
            'rs0': rs0[c],
            'batch_row': batch.reshape(1, N_pad).astype(np.float16),
            'batch_col': batch.reshape(W, P).T.copy(),
            'valid_col': valid.reshape(W, P).T.copy(),
            'invcntD': (1.0 / (cnt * D)).astype(np.float32).reshape(gpc, 1),
        })
        in_maps.append(im)

    dims = dict(N=N, D=D, E=E, ED=ED, L=L, HID=HID, G=G, H=H, C=C, gpc=gpc,
                N_pad=N_pad, W=W, NCH=NCH, NPT=NPT, nT=nT, E_pad=E_pad,
                T_w=[int(t) for t in T_w], tA_w=[int(t) for t in tA_w],
                tB_w=[int(t) for t in tB_w], tstart=[int(t) for t in tstart],
                colA=[int(t) for t in colA], colB=[int(t) for t in colB],
                nA_cols=int(colA[-1]), nB_cols=max(int(colB[-1]), 16),
                n0s=n0s, Nl=Nl, inv_perm=inv_perm)
    return in_maps, dims


# --------------------------------------------------------------- bass kernel
def build_nc(dims):
    D = dims['D']
    ED = dims['ED']
    L = dims['L']
    HID = dims['HID']
    H = dims['H']
    C = dims['C']
    DA = D + H
    gpc = dims['gpc']
    N_pad = dims['N_pad']
    W = dims['W']
    NCH = dims['NCH']
    NPT = dims['NPT']
    nT = dims['nT']
    E_pad = dims['E_pad']
    T_w = dims['T_w']
    tA_w = dims['tA_w']
    tB_w = dims['tB_w']
    tstart = dims['tstart']
    colA = dims['colA']
    colB = dims['colB']
    N = dims['N']
    HB = HID // P
    tpo = GMAX // P
    NB = min(REPS, 2)

    nc = bacc.Bacc("TRN2", target_bir_lowering=False, debug=False, num_devices=NCORE)

    def inp(name, shape, dt=F32):
        return nc.dram_tensor(name, list(shape), dt, kind="ExternalInput").ap()

    t_idxA = inp('idxA', (P, dims['nA_cols']), I16)
    t_idxB = inp('idxB', (P, dims['nB_cols']), I16)
    t_ST = inp('ST_h', (P, nT * P), F16)
    t_S = inp('S_h', (P, nT * P), F16)
    t_eaT = inp('eaT', (ED + 1, E_pad), F16)
    t_xl0 = inp('xl0', (NPT, EL), F16)
    t_xr0 = inp('xr0', (P, W * D), F16)
    t_rs0 = inp('rs0', (P, W * D), F16)
    t_batch_row = inp('batch_row', (1, N_pad), F16)
    t_batch_col = inp('batch_col', (P, W))
    t_valid_col = inp('valid_col', (P, W))
    t_invcntD = inp('invcntD', (gpc, 1))
    t_iota_row = inp('iota_row', (P, P), F16)
    t_giota_rep = inp('giota_rep', (P, gpc))
    t_giota_col = inp('giota_col', (gpc, 1))
    t_ident = inp('ident', (P, P), F16)
    t_ident32 = inp('ident32', (P, P), F32)
    t_ones_col = inp('ones_col', (P, 1))
    t_ones_col16 = inp('ones_col16', (P, 1), F16)
    t_ones_row = inp('ones_row', (1, 512))
    t_ones16 = inp('ones16', (1, P), F16)
    t_ashift = inp('ashift_col', (P, 1))
    tw = {}
    wspec = []
    for l in range(L):
        wspec += [(f'WlA{l}', (P, D), F16), (f'blA{l}', (1, D), F16),
                  (f'WrA{l}', (P, D), F16), (f'WeX{l}', (ED + 1, D), F16),
                  (f'Wres{l}', (P, D), F16), (f'combo{l}', (1, D), F16),
                  (f'attC{l}', (P, D), F16),
                  (f'W1_{l}', (P, HID), F32), (f'W2_{l}', (P, HID), F32),
                  (f'b2_{l}', (1, D), F32), (f'bng{l}', (1, HID), F32),
                  (f'bnb{l}', (1, HID), F32),
                  (f'lng{l}', (D, 1), F32), (f'lnb{l}', (D, 1), F32)]
    for key, shape, dt in wspec:
        tw[key] = inp(key, shape, dt)

    t_out = nc.dram_tensor('out_rows', [N_pad, D], F32, kind="ExternalOutput").ap()

    # layer-1 gather tables: AllGather output in Shared address space (fast
    # HBM-HBM collective path); input staged in Local scratch.
    t_xl1full = [nc.dram_tensor(f'xl1full{r}', [NPT, EL], F16,
                                kind="Internal", addr_space="Shared").ap()
                 for r in range(NB)]

    with tile.TileContext(nc) as tc:
        with tc.tile_pool(name="const", bufs=1) as cpool, \
             tc.tile_pool(name="dram", bufs=1, space="DRAM") as dpool, \
             tc.tile_pool(name="big", bufs=1) as bigpool:

            def ld(ap, shape, dt=F32, pool=cpool, name=None):
                if name is None:
                    name = 'c_' + ap.tensor.name
                t = pool.tile(list(shape), dt, name=name, tag=name)
                nc.sync.dma_start(t[:], ap[:])
                return t

            s_idxA = ld(t_idxA, (P, dims['nA_cols']), I16, bigpool)
            s_idxB = ld(t_idxB, (P, dims['nB_cols']), I16, bigpool)
            s_batch_col = ld(t_batch_col, (P, W))
            s_valid_col = ld(t_valid_col, (P, W))
            s_invcntD = ld(t_invcntD, (gpc, 1))
            s_iota_row = ld(t_iota_row, (P, P), F16)
            s_giota_rep = ld(t_giota_rep, (P, gpc))
            s_giota_col = ld(t_giota_col, (gpc, 1))
            s_ident = ld(t_ident, (P, P), F16)
            s_ident32 = ld(t_ident32, (P, P), F32)
            s_ones_col = ld(t_ones_col, (P, 1))
            s_ones_col16 = ld(t_ones_col16, (P, 1), F16)
            s_ones_row = ld(t_ones_row, (1, 512))
            s_ones16 = ld(t_ones16, (1, P), F16)
            s_ashift = ld(t_ashift, (P, 1))
            sw = {}
            for key, shape, dt in wspec:
                sw[key] = ld(tw[key], shape, dt)

            d_xl1loc_r = [dpool.tile([N_pad, EL], F16, tag=f'xl1loc{r}',
                                     name=f'd_xl1loc{r}') for r in range(NB)]
            d_arin = [dpool.tile([P, D + 1], F32, tag=f'arin{l}', name=f'd_arin{l}')
                      for l in range(L)]
            d_arout = [dpool.tile([P, D + 1], F32, tag=f'arout{l}', name=f'd_arout{l}')
                       for l in range(L)]

            x1Tb = [bigpool.tile([P, N_pad], F16, tag=f'x1T{i}', name=f'x1T{i}')
                    for i in range(NB)]
            x3Tb = [bigpool.tile([P, N_pad], F16, tag=f'x3T{i}', name=f'x3T{i}')
                    for i in range(NB)]

            from contextlib import ExitStack

            def emit_B(rep, l):
                x1T = x1Tb[rep % NB]
                x3T = x3Tb[rep % NB]
                tab = t_xl0 if l == 0 else t_xl1full[rep % NB]
                ctx = ExitStack()
                pC = ctx.enter_context(tc.tile_pool(name="pC", bufs=1))
                pCsp = ctx.enter_context(tc.tile_pool(name="pCs", bufs=1, space="PSUM"))
                pCs = pCsp.tile([P, D + 1], F32, tag='cs')
                with tc.tile_pool(name="pB", bufs=3) as pB, \
                     tc.tile_pool(name="pB1", bufs=3) as pB1, \
                     tc.tile_pool(name="pBz", bufs=3, space="PSUM") as pBz, \
                     tc.tile_pool(name="pBa", bufs=2, space="PSUM") as pBa, \
                     tc.tile_pool(name="pBr", bufs=1, space="PSUM") as pBr:
                    for w in range(W):
                        T = T_w[w]
                        tA = tA_w[w]
                        tB = tB_w[w]
                        tb = tstart[w]
                        EW = T * P
                        eat = pB.tile([ED + 1, EW], F16, tag='eat')
                        nc.sync.dma_start(eat[:], t_eaT[:, tb * P:tb * P + EW])
                        xsv = pB.tile([P, T * EL], F16, tag='xsv')
                        xsr = xsv[:].rearrange("p (t q) -> p t q", q=EL)
                        for o in range(0, tA, tpo):
                            t0, t1 = o, min(o + tpo, tA)
                            ni = (t1 - t0) * P
                            nc.gpsimd.dma_gather(
                                xsr[:, t0:t1, :], tab,
                                s_idxA[:, (colA[w] + t0 * 8):(colA[w] + t1 * 8)],
                                ni, ni, EL)
                        for o in range(0, tB, tpo):
                            t0, t1 = o, min(o + tpo, tB)
                            ni = (t1 - t0) * P
                            nc.gpsimd.dma_gather(
                                xsr[:, tA + t0:tA + t1, :], tab[SPLIT:, :],
                                s_idxB[:, (colB[w] + t0 * 8):(colB[w] + t1 * 8)],
                                ni, ni, EL)
                        if l == 0:
                            xrw = pB.tile([P, D], F16, tag='xrw')
                            nc.sync.dma_start(xrw[:], t_xr0[:, w * D:(w + 1) * D])
                            rsw = pB.tile([P, D], F16, tag='rsw')
                            nc.sync.dma_start(rsw[:], t_rs0[:, w * D:(w + 1) * D])
                            xrw_ap = xrw[:]
                        else:
                            pxr = pBr.tile([P, D], F32, tag='pxr')
                            nc.tensor.matmul(pxr[:], lhsT=x3T[:, w * P:(w + 1) * P],
                                             rhs=sw['WrA1'][:], start=True, stop=True)
                            xrw = pB.tile([P, D], F16, tag='xrw')
                            nc.scalar.activation(out=xrw[:], in_=pxr[:], func=AF.Copy)
                            xrw_ap = xrw[:]
                        # S / ST one-hots: host-built, DMA-loaded
                        S = pB1.tile([P, EW], F16, tag='S')
                        nc.sync.dma_start(S[:], t_S[:, tb * P:tb * P + EW])
                        ST = pB1.tile([P, EW], F16, tag='ST')
                        nc.sync.dma_start(ST[:], t_ST[:, tb * P:tb * P + EW])
                        m16 = pB1.tile([P, T * D], F16, tag='m16')
                        KP = 4
                        for g0 in range(0, T, KP):
                            gn = min(KP, T - g0)
                            pz3 = pBz.tile([P, KP * D], F32, tag='pz')
                            for u in range(gn):
                                t = g0 + u
                                sl = slice(u * D, (u + 1) * D)
                                nc.tensor.matmul(pz3[:, sl], lhsT=eat[:, t * P:(t + 1) * P],
                                                 rhs=sw[f'WeX{l}'][:], start=True, stop=False,
                                                 skip_group_check=True)
                                nc.tensor.matmul(pz3[:, sl], lhsT=ST[:, t * P:(t + 1) * P],
                                                 rhs=xrw_ap,
                                                 start=False, stop=True,
                                                 skip_group_check=True)
                            pzv = pz3[:].rearrange("p (u q) -> p u q", q=D)
                            mv = m16[:, g0 * D:(g0 + gn) * D].rearrange("p (u n) -> p u n", n=D)
                            nc.vector.tensor_tensor(out=mv, in0=pzv[:, 0:gn, :],
                                                    in1=xsr[:, g0:g0 + gn, 0:D],
                                                    op=ALU.add)
                            nc.scalar.activation(out=mv, in_=mv, func=AF.Prelu, alpha=NEG)
                        eng_tt = nc.vector if (w % 2 == 0) else nc.gpsimd
                        eng_tt.tensor_tensor(
                            out=m16[:].rearrange("p (t n) -> p t n", t=T),
                            in0=m16[:].rearrange("p (t n) -> p t n", t=T),
                            in1=sw[f'attC{l}'][:].rearrange("p (o n) -> p o n", o=1).to_broadcast([P, T, P]),
                            op=ALU.mult)
                        alpha2 = pB.tile([P, T * H], F32, tag='alpha2')
                        with nc.allow_low_precision(reason="16 fp16 terms, |alpha|<~30"):
                            nc.vector.tensor_reduce(
                                out=alpha2[:],
                                in_=m16[:].rearrange("p (t c h) -> p t h c", c=C, h=H),
                                axis=mybir.AxisListType.X, op=ALU.add)
                        ybuf = pB.tile([P, T * (D + 8)], F16, tag='ybuf')
                        yv = ybuf[:].rearrange("p (t q) -> p t q", q=D + 8)
                        nc.scalar.activation(
                            out=yv[:, :, D:D + 8],
                            in_=alpha2[:].rearrange("p (t h) -> p t h", t=T),
                            func=AF.Exp, bias=s_ashift[:])
                        nc.vector.tensor_tensor(
                            out=yv[:, :, 0:D].rearrange("p t (c h) -> p t c h", c=C),
                            in0=xsr[:, :, 0:D].rearrange("p t (c h) -> p t c h", c=C),
                            in1=yv[:, :, D:D + 8].rearrange("p t (o h) -> p t o h", o=1).to_broadcast([P, T, C, H]),
                            op=ALU.mult)
                        pagg = pBa.tile([P, D + 8], F32, tag='pagg')
                        for t in range(T):
                            nc.tensor.matmul(pagg[:], lhsT=S[:, t * P:(t + 1) * P],
                                             rhs=yv[:, t, :], start=(t == 0),
                                             stop=(t == T - 1))
                        den = pB.tile([P, H], F32, tag='den')
                        nc.vector.tensor_scalar(out=den[:], in0=pagg[:, D:D + 8],
                                                scalar1=1e-16, scalar2=None, op0=ALU.add)
                        rec = pB.tile([P, H], F32, tag='rec')
                        nc.vector.reciprocal(rec[:], den[:])
                        x1w = pB.tile([P, D + 1], F16, tag='x1w')
                        nc.vector.tensor_tensor(
                            out=x1w[:, 0:D].rearrange("p (c h) -> p c h", c=C),
                            in0=pagg[:, 0:D].rearrange("p (c h) -> p c h", c=C),
                            in1=rec[:].rearrange("p (o h) -> p o h", o=1).to_broadcast([P, C, H]),
                            op=ALU.mult)
                        if l == 0:
                            nc.vector.tensor_add(x1w[:, 0:D], x1w[:, 0:D],
                                                 rsw[:])
                        else:
                            pres = pBr.tile([P, P], F32, tag='pxr')
                            nc.tensor.matmul(pres[:], lhsT=x3T[:, w * P:(w + 1) * P],
                                             rhs=sw['Wres1'][:], start=True, stop=False)
                            nc.tensor.matmul(pres[:], lhsT=s_ones16[:, 0:1].to_broadcast([1, P]),
                                             rhs=sw['combo1'][:], start=False, stop=True)
                            nc.vector.tensor_add(x1w[:, 0:D], x1w[:, 0:D], pres[:])
                        nc.vector.tensor_scalar(out=x1w[:, 0:D], in0=x1w[:, 0:D],
                                                scalar1=s_valid_col[:, w:w + 1],
                                                scalar2=None, op0=ALU.mult)
                        nc.vector.tensor_copy(x1w[:, D:D + 1], s_valid_col[:, w:w + 1])
                        nc.tensor.matmul(pCs[:], lhsT=x1w[:, 0:D], rhs=x1w[:, 0:D + 1],
                                         start=(w == 0), stop=(w == W - 1),
                                         skip_group_check=True)
                        ptr = pBr.tile([P, P], F16, tag='ptr')
                        nc.tensor.transpose(out=ptr[:], in_=x1w[:, 0:D], identity=s_ident[:])
                        nc.scalar.activation(out=x1T[:, w * P:(w + 1) * P],
                                             in_=ptr[:], func=AF.Copy)
                return ctx, pC, pCs

            def emit_CDE(rep, l, ctx, pC, pCs):
                x1T = x1Tb[rep % NB]
                x3T = x3Tb[rep % NB]
                # ======== Phase C: BN stats (AllReduce) ==================
                pCp_cm = tc.tile_pool(name="pCp", bufs=1, space="PSUM")
                pCp = pCp_cm.__enter__()
                cs_sb = pC.tile([P, D + 1], F32, tag='cs')
                nc.vector.tensor_copy(cs_sb[:], pCs[:])
                nc.sync.dma_start(d_arin[l][:], cs_sb[:])
                nc.gpsimd.collective_compute(
                    "AllReduce", ALU.add,
                    replica_groups=[list(range(NCORE))],
                    ins=[d_arin[l][:].opt()], outs=[d_arout[l][:].opt()])
                csr = pC.tile([P, D + 1], F32, tag='csr')
                nc.sync.dma_start(csr[:], d_arout[l][:])
                mu = pC.tile([P, 1], F32, tag='mu')
                nc.vector.tensor_scalar(out=mu[:], in0=csr[:, D:D + 1],
                                        scalar1=1.0 / N, scalar2=None, op0=ALU.mult)
                pmu = pCp.tile([1, HID], F32, tag='pmu')
                nc.tensor.matmul(pmu[:], lhsT=mu[:], rhs=sw[f'W1_{l}'][:],
                                 start=True, stop=True)
                pP1 = pCp.tile([P, HID], F32, tag='pP1')
                nc.tensor.matmul(pP1[:], lhsT=csr[:, 0:D], rhs=sw[f'W1_{l}'][:],
                                 start=True, stop=True)
                w1p1 = pC.tile([P, HID], F32, tag='w1p1')
                nc.vector.tensor_tensor(out=w1p1[:], in0=sw[f'W1_{l}'][:],
                                        in1=pP1[:], op=ALU.mult)
                pt2 = pCp.tile([1, HID], F32, tag='pt2')
                nc.tensor.matmul(pt2[:], lhsT=s_ones_col[:], rhs=w1p1[:],
                                 start=True, stop=True)
                mh = pC.tile([1, HID], F32, tag='mh')
                nc.vector.tensor_copy(mh[:], pmu[:])
                var = pC.tile([1, HID], F32, tag='var')
                nc.vector.tensor_scalar(out=var[:], in0=pt2[:], scalar1=1.0 / N,
                                        scalar2=None, op0=ALU.mult)
                m2 = pC.tile([1, HID], F32, tag='m2')
                nc.vector.tensor_tensor(out=m2[:], in0=mh[:], in1=mh[:], op=ALU.mult)
                nc.vector.tensor_tensor(out=var[:], in0=var[:], in1=m2[:], op=ALU.subtract)
                sd = pC.tile([1, HID], F32, tag='sd')
                nc.vector.tensor_scalar(out=var[:], in0=var[:], scalar1=EPS,
                                        scalar2=None, op0=ALU.add)
                nc.scalar.activation(out=sd[:], in_=var[:], func=AF.Sqrt)
                rsd = pC.tile([1, HID], F32, tag='rsd')
                nc.vector.reciprocal(rsd[:], sd[:])
                geff = pC.tile([1, HID], F32, tag='geff')
                nc.vector.tensor_tensor(out=geff[:], in0=sw[f'bng{l}'][:],
                                        in1=rsd[:], op=ALU.mult)
                beff = pC.tile([1, HID], F32, tag='beff')
                nc.vector.tensor_tensor(out=beff[:], in0=mh[:], in1=geff[:], op=ALU.mult)
                nc.vector.tensor_tensor(out=beff[:], in0=sw[f'bnb{l}'][:],
                                        in1=beff[:], op=ALU.subtract)
                pgrep = pCp.tile([P, HID], F32, tag='pgrep')
                nc.tensor.matmul(pgrep[:], lhsT=s_ones_col[:1, :].rearrange("o p -> p o").to_broadcast([1, P]),
                                 rhs=geff[:], start=True, stop=True)
                w1eff = pC.tile([P, HID], F16, tag='w1eff')
                nc.vector.tensor_tensor(out=w1eff[:], in0=sw[f'W1_{l}'][:],
                                        in1=pgrep[:], op=ALU.mult)
                becol = pC.tile([P, HB], F32, tag='becol')
                for k in range(HB):
                    ptb = pCp.tile([P, 1], F32, tag='ptb')
                    nc.tensor.transpose(out=ptb[:], in_=beff[:, k * P:(k + 1) * P],
                                        identity=s_ident32[:1, :1])
                    nc.vector.tensor_copy(becol[:, k:k + 1], ptb[:])
                pCp_cm.__exit__(None, None, None)

                # ======== Phase D: MLP (x2 overwrites x1T in place) ======
                with tc.tile_pool(name="pD", bufs=2) as pD, \
                     tc.tile_pool(name="pDp", bufs=2, space="PSUM") as pDp, \
                     tc.tile_pool(name="pDx", bufs=2, space="PSUM") as pDx:
                    for i in range(NCH):
                        c0 = i * 512
                        px2 = pDx.tile([P, 512], F32, tag='px2')
                        for k in range(HB):
                            ph = pDp.tile([P, 512], F32, tag='ph')
                            nc.tensor.matmul(ph[:], lhsT=w1eff[:, k * P:(k + 1) * P],
                                             rhs=x1T[:, c0:c0 + 512],
                                             start=True, stop=True)
                            hs = pD.tile([P, 512], F32, tag='hs')
                            nc.scalar.activation(out=hs[:], in_=ph[:], func=AF.Relu,
                                                 bias=becol[:, k:k + 1], scale=1.0)
                            nc.tensor.matmul(px2[:], lhsT=sw[f'W2_{l}'][:, k * P:(k + 1) * P],
                                             rhs=hs[:], start=(k == 0), stop=False,
                                             skip_group_check=True)
                        nc.tensor.matmul(px2[:], lhsT=sw[f'b2_{l}'][:],
                                         rhs=s_ones_row[:], start=False, stop=True,
                                         skip_group_check=True)
                        nc.vector.tensor_add(x1T[:, c0:c0 + 512], px2[:],
                                             x1T[:, c0:c0 + 512])

                # ======== Phase E: graph LayerNorm (+ fused layer-1 table
                # production and AllGather when l == 0) ====================
                with tc.tile_pool(name="pE", bufs=2) as pE, \
                     tc.tile_pool(name="pEg", bufs=1, space="PSUM") as pEgp, \
                     tc.tile_pool(name="pEp", bufs=1, space="PSUM") as pEp:
                    pgs = pEgp.tile([gpc, 2], F32, tag='pgs')
                    for w in range(W):
                        sl = slice(w * P, (w + 1) * P)
                        sq = pE.tile([P, P], F16, tag='sq')
                        nc.scalar.activation(out=sq[:], in_=x1T[:, sl],
                                             func=AF.Square)
                        pcs = pEp.tile([1, 2 * P], F32, tag='pcs')
                        nc.tensor.matmul(pcs[:, 0:P], lhsT=s_ones_col16[:], rhs=x1T[:, sl],
                                         start=True, stop=True, skip_group_check=True)
                        nc.tensor.matmul(pcs[:, P:2 * P], lhsT=s_ones_col16[:], rhs=sq[:],
                                         start=True, stop=True, skip_group_check=True)
                        rows = pE.tile([1, 2 * P], F32, tag='rows')
                        nc.vector.tensor_copy(rows[:], pcs[:])
                        csc = pE.tile([P, 2], F32, tag='csc')
                        for q in range(2):
                            ptb = pEp.tile([P, 1], F32, tag='ptb2')
                            nc.tensor.transpose(out=ptb[:], in_=rows[:, q * P:(q + 1) * P],
                                                identity=s_ident32[:1, :1])
                            nc.vector.tensor_copy(csc[:, q:q + 1], ptb[:])
                        bg = pE.tile([P, gpc], F32, tag='bg')
                        nc.vector.tensor_scalar(out=bg[:], in0=s_giota_rep[:],
                                                scalar1=s_batch_col[:, w:w + 1],
                                                scalar2=None, op0=ALU.is_equal)
                        nc.tensor.matmul(pgs[:], lhsT=bg[:], rhs=csc[:],
                                         start=(w == 0), stop=(w == W - 1),
                                         skip_group_check=True)
                    gm = pE.tile([gpc, 1], F32, tag='gm')
                    nc.vector.tensor_tensor(out=gm[:], in0=pgs[:, 0:1],
                                            in1=s_invcntD[:], op=ALU.mult)
                    e2 = pE.tile([gpc, 1], F32, tag='e2')
                    nc.vector.tensor_tensor(out=e2[:], in0=pgs[:, 1:2],
                                            in1=s_invcntD[:], op=ALU.mult)
                    gv = pE.tile([gpc, 1], F32, tag='gv')
                    nc.vector.tensor_tensor(out=gv[:], in0=gm[:], in1=gm[:], op=ALU.mult)
                    nc.vector.tensor_tensor(out=gv[:], in0=e2[:], in1=gv[:], op=ALU.subtract)
                    sdg = pE.tile([gpc, 1], F32, tag='sdg')
                    nc.vector.tensor_scalar(out=gv[:], in0=gv[:], scalar1=EPS,
                                            scalar2=None, op0=ALU.add)
                    nc.scalar.activation(out=sdg[:], in_=gv[:], func=AF.Sqrt)
                    ivg = pE.tile([gpc, 1], F32, tag='ivg')
                    nc.vector.reciprocal(ivg[:], sdg[:])
                    gmr = pE.tile([gpc, P], F32, tag='gmr')
                    nc.vector.tensor_copy(gmr[:], gm[:].to_broadcast([gpc, P]))
                    ivr = pE.tile([gpc, P], F32, tag='ivr')
                    nc.vector.tensor_copy(ivr[:], ivg[:].to_broadcast([gpc, P]))
                    for i in range(NCH):
                        c0 = i * 512
                        brc = pE.tile([1, 512], F16, tag='brc')
                        nc.sync.dma_start(brc[:], t_batch_row[:, c0:c0 + 512])
                        pbr = pEp.tile([gpc, 512], F32, tag='pbr')
                        nc.tensor.matmul(pbr[:],
                                         lhsT=s_ones16[:, 0:1].to_broadcast([1, gpc]),
                                         rhs=brc[:],
                                         start=True, stop=True)
                        bgT = pE.tile([gpc, 512], F32, tag='bgT')
                        nc.vector.tensor_scalar(out=bgT[:], in0=pbr[:],
                                                scalar1=s_giota_col[:],
                                                scalar2=None, op0=ALU.is_equal)
                        pgm = pEp.tile([P, 512], F32, tag='pgm')
                        nc.tensor.matmul(pgm[:], lhsT=gmr[:], rhs=bgT[:],
                                         start=True, stop=True)
                        piv = pEp.tile([P, 512], F32, tag='piv')
                        nc.tensor.matmul(piv[:], lhsT=ivr[:], rhs=bgT[:],
                                         start=True, stop=True)
                        tmp = pE.tile([P, 512], F32, tag='tmp')
                        nc.vector.tensor_tensor(out=tmp[:], in0=x1T[:, c0:c0 + 512],
                                                in1=pgm[:], op=ALU.subtract)
                        nc.vector.tensor_tensor(out=tmp[:], in0=tmp[:],
                                                in1=piv[:], op=ALU.mult)
                        if l == 0:
                            nc.vector.tensor_scalar(out=x3T[:, c0:c0 + 512], in0=tmp[:],
                                                    scalar1=sw[f'lng{l}'][:],
                                                    scalar2=sw[f'lnb{l}'][:],
                                                    op0=ALU.mult, op1=ALU.add)
                            # fused layer-1 gather-table production
                            xa = pE.tile([P, 4 * D], F16, tag='xa')
                            for q in range(4):
                                pxa = pEp.tile([P, D], F32, tag='pxa')
                                nc.tensor.matmul(
                                    pxa[:], lhsT=x3T[:, c0 + q * P:c0 + (q + 1) * P],
                                    rhs=sw['WlA1'][:], start=True, stop=False)
                                nc.tensor.matmul(
                                    pxa[:], lhsT=s_ones16[:, 0:1].to_broadcast([1, P]),
                                    rhs=sw['blA1'][:], start=False, stop=True)
                                nc.vector.tensor_copy(xa[:, q * D:(q + 1) * D], pxa[:])
                            nc.sync.dma_start(
                                d_xl1loc_r[rep % NB][c0:c0 + 512, 0:D].rearrange(
                                    "(q p) d -> p q d", p=P),
                                xa[:].rearrange("p (q d) -> p q d", d=D))
                        else:
                            x3c = pE.tile([P, 512], F16, tag='x3c')
                            nc.vector.tensor_scalar(out=x3c[:], in0=tmp[:],
                                                    scalar1=sw[f'lng{l}'][:],
                                                    scalar2=sw[f'lnb{l}'][:],
                                                    op0=ALU.mult, op1=ALU.add)
                            for q in range(4):
                                ptb2 = pEp.tile([P, P], F16, tag='ptb2')
                                nc.tensor.transpose(out=ptb2[:],
                                                    in_=x3c[:, q * P:(q + 1) * P],
                                                    identity=s_ident[:])
                                orow = pE.tile([P, P], F32, tag='orow')
                                nc.vector.tensor_copy(orow[:], ptb2[:])
                                r0 = c0 + q * P
                                nc.sync.dma_start(t_out[r0:r0 + P, :], orow[:])
                if l == 0:
                    nc.gpsimd.collective_compute(
                        "AllGather", ALU.bypass,
                        replica_groups=[list(range(NCORE))],
                        ins=[d_xl1loc_r[rep % NB][:].opt()],
                        outs=[t_xl1full[rep % NB][:].opt()])
                ctx.close()

            for r in range(REPS):
                for l in range(L):
                    ctx, pC, pCs = emit_B(r, l)
                    emit_CDE(r, l, ctx, pC, pCs)

    nc.compile()
    return nc


# ---------------------------------------------------------------- entry point
_CACHE = {}


def kernel(**inputs):
    in_maps, dims = host_prep(**inputs)
    key = (REPS, dims['N'], dims['E'], dims['N_pad'], dims['nT'],
           tuple(dims['T_w']), tuple(dims['tA_w']))
    if key not in _CACHE:
        _CACHE[key] = build_nc(dims)
    nc = _CACHE[key]
    res = run_bass_kernel_spmd(nc, in_maps, core_ids=list(range(NCORE)), trace=False)
    global _last_res, _last_dims
    _last_res, _last_dims = res, dims
    N, D = dims['N'], dims['D']
    out = np.zeros((N, D), dtype=np.float32)
    n0s, Nl = dims['n0s'], dims['Nl']
    inv = dims['inv_perm']
    for c in range(NCORE):
        out[n0s[c]:n0s[c + 1]] = res.results[c]['out_rows'][:Nl[c]][:, inv]
    return out


# revision 15
# speedup vs baseline: 1.2535x; 1.0518x over previous
"""GATv2 block (2 layers) on 8 Trainium2 NeuronCores via Bass/Tile — v3.

Structure vs v2 baseline:
- Edge source features gathered with bulk dma_gather (split-table for int16
  index range), table rows padded to 512B.
- Destination transform xr never round-trips DRAM: per-window xr tile stays in
  SBUF and is injected into the per-edge PSUM via the transpose ST of the
  aggregation one-hot S.
- v4: the leaky-relu runs directly on the Activation engine (AF.Lrelu with
  alpha=0.2), so gather-table rows carry plain x@W (128 fp16 = 256B): half the
  gather + AllGather traffic of the 0.6z+0.4|z| linear-rider scheme, and one
  ACT op per PSUM group instead of two (fewer activation-table switches).
- v4: edge channels are stored c-major (new col = c*H + h) so the exp*xs
  multiply and the softmax divide hit the DVE packed fast path; weights are
  permuted host-side and the output is unpermuted in kernel().
- layer-1 gather table (xl1 = x3 @ Wl1) is produced chunk-by-chunk inside
  layer-0's LayerNorm phase and AllGathered into a Shared-address DRAM tensor.
- host_prep fully vectorized (sort-by-(core,window,split) + scatter).

softmax num/den accumulate via one-hot segment matmul; BN stats via Gram
matrix AllReduce; graph-LN via one-hot segment matmuls.
"""
import sys
import math

sys.path.insert(0, '/opt/trn_rl_repo')

import numpy as np
import concourse.bass as bass
import concourse.tile as tile
from concourse import bacc, mybir
from concourse.bass_utils import run_bass_kernel_spmd

F32 = mybir.dt.float32
F16 = mybir.dt.float16
I16 = mybir.dt.int16
AF = mybir.ActivationFunctionType
ALU = mybir.AluOpType

P = 128
NCORE = 8
NEG = 0.2
EPS = 1e-5
ASHIFT = -4.0   # constant softmax shift: exp(alpha-4) keeps fp16 exp in range
EL = 128        # fp16 elements per gather-table row (256B)
SPLIT = 32768   # int16 index split point
REPS = 1
GMAX = 896      # max idxs per dma_gather op


# ----------------------------------------------------------------- host prep
def host_prep(x, node_batch, edge_index, edge_attr, Wl, bl, Wr, br, We, att,
              bias, Wres, W1, b1, bn_gamma, bn_beta, W2, b2, ln_gamma, ln_beta):
    N, D = x.shape
    E = edge_index.shape[1]
    ED = edge_attr.shape[1]
    L = Wl.shape[0]
    HID = W1.shape[2]
    G = int(node_batch.max()) + 1
    H = att.shape[1]
    C = att.shape[2]
    DA = D + H
    gpc = (G + NCORE - 1) // NCORE

    nb = np.asarray(node_batch).astype(np.int64)
    src = np.asarray(edge_index[0]).astype(np.int64)
    dst = np.asarray(edge_index[1]).astype(np.int64)
    ea = np.asarray(edge_attr, dtype=np.float32)
    xf = np.asarray(x, np.float32)

    gb = np.searchsorted(nb, np.arange(G + 1))
    n0s = np.array([gb[min(c * gpc, G)] for c in range(NCORE + 1)], dtype=np.int64)
    Nl = n0s[1:] - n0s[:-1]
    N_pad = int(math.ceil(max(Nl.max(), 1) / 512.0) * 512)
    W = N_pad // P
    NCH = N_pad // 512
    NPT = NCORE * N_pad

    core_of = np.searchsorted(n0s, np.arange(N), side='right') - 1
    glob_id = (core_of * N_pad + (np.arange(N) - n0s[core_of])).astype(np.int64)

    ecore = core_of[dst]
    gsrc = glob_id[src]
    dslot_all = dst - n0s[ecore]
    ewin_all = dslot_all // P
    eslot_all = dslot_all % P
    isB = (gsrc >= SPLIT).astype(np.int64)

    # Per (core, window, split): counts -> shared tile layout (max over cores).
    key = (ecore * W + ewin_all) * 2 + isB
    cnt2 = np.bincount(key, minlength=NCORE * W * 2).reshape(NCORE, W, 2)
    nA, nB = cnt2[..., 0], cnt2[..., 1]
    tA_w = np.maximum(np.ceil(nA.max(axis=0) / P).astype(np.int64), 1)
    tB_w = np.ceil(nB.max(axis=0) / P).astype(np.int64)
    T_w = tA_w + tB_w
    tstart = np.concatenate([[0], np.cumsum(T_w)])
    nT = int(tstart[-1])
    E_pad = nT * P
    tsA = np.concatenate([[0], np.cumsum(tA_w)])
    tsB = np.concatenate([[0], np.cumsum(tB_w)])
    baseA = tstart[:-1] * P
    baseB = baseA + tA_w * P
    colA = np.concatenate([[0], np.cumsum(tA_w * (P // 16))])
    colB = np.concatenate([[0], np.cumsum(tB_w * (P // 16))])
    LA = int(tsA[-1]) * P
    LB = int(tsB[-1]) * P

    # Stable sort by (core, window, split); rank within group gives each edge
    # a unique slot in its window's tile range.
    order = np.argsort(key, kind='stable')
    sk = key[order]
    starts = np.zeros(E, np.int64)
    gs = np.r_[0, np.flatnonzero(np.diff(sk)) + 1]
    starts[gs] = gs
    starts = np.maximum.accumulate(starts)
    rank = np.arange(E) - starts
    wo = ewin_all[order]
    bo = isB[order]
    co = ecore[order]
    pos = np.where(bo == 0, baseA[wo], baseB[wo]) + rank

    # c-major channel permutation: new col j = c*H + h holds old channel
    # h*C + c. Makes the exp*xs multiply and softmax divide DVE-packed.
    perm = np.array([h * C + c for c in range(C) for h in range(H)])
    inv_perm = np.argsort(perm)

    shared = {
        'iota_row': np.tile(np.arange(P, dtype=np.float16), (P, 1)),
        'giota_rep': np.tile(np.arange(gpc, dtype=np.float32), (P, 1)),
        'giota_col': np.arange(gpc, dtype=np.float32).reshape(gpc, 1),
        'ident': np.eye(P, dtype=np.float16),
        'ident32': np.eye(P, dtype=np.float32),
        'ones_col': np.ones((P, 1), np.float32),
        'ones_col16': np.ones((P, 1), np.float16),
        'ones_row': np.ones((1, 512), np.float32),
        'ones16': np.ones((1, P), np.float16),
        'ashift_col': np.full((P, 1), ASHIFT, np.float32),
    }
    WlA_f, blA_f, WrA_f = [], [], []
    for l in range(L):
        Wl_ = np.asarray(Wl[l], np.float32)
        Wr_ = np.asarray(Wr[l], np.float32)
        We_ = np.asarray(We[l], np.float32)
        bl_ = np.asarray(bl[l], np.float32)
        br_ = np.asarray(br[l], np.float32)
        # layer >= 1 inputs live in the permuted basis: permute weight ROWS.
        rp = perm if l >= 1 else np.arange(D)
        Wl_r = Wl_[rp]
        Wr_r = Wr_[rp]
        Wres_r = np.asarray(Wres[l], np.float32)[rp]
        WlA_f.append(Wl_r[:, perm])
        blA_f.append(bl_[perm])
        WrA_f.append(Wr_r[:, perm])
        shared[f'WlA{l}'] = WlA_f[l].astype(np.float16)
        shared[f'blA{l}'] = blA_f[l].reshape(1, D).astype(np.float16)
        shared[f'WrA{l}'] = WrA_f[l].astype(np.float16)
        wex = np.concatenate([We_, br_.reshape(1, D)], 0)
        shared[f'WeX{l}'] = wex[:, perm].astype(np.float16)
        shared[f'Wres{l}'] = Wres_r[:, perm].astype(np.float16)
        shared[f'combo{l}'] = np.asarray(bias[l], np.float32)[perm].astype(np.float16).reshape(1, D)
        # full att vector, c-major: [P, D]
        aC = np.asarray(att[l], np.float32).reshape(H * C)[perm].astype(np.float16)
        shared[f'attC{l}'] = np.tile(aC.reshape(1, D), (P, 1))
        shared[f'W1_{l}'] = np.asarray(W1[l], np.float32)[perm]
        w2 = np.asarray(W2[l], np.float32)[:, perm]
        shared[f'W2_{l}'] = np.concatenate(
            [w2[k * P:(k + 1) * P, :] for k in range(HID // P)], axis=1)
        shared[f'b2_{l}'] = np.asarray(b2[l], np.float32)[perm].reshape(1, D)
        shared[f'bng{l}'] = np.asarray(bn_gamma[l], np.float32).reshape(1, HID)
        shared[f'bnb{l}'] = np.asarray(bn_beta[l], np.float32).reshape(1, HID)
        shared[f'lng{l}'] = np.asarray(ln_gamma[l], np.float32)[perm].reshape(D, 1)
        shared[f'lnb{l}'] = np.asarray(ln_beta[l], np.float32)[perm].reshape(D, 1)

    # layer-0 host precomputes: gather table, xr0, resid0
    xl0 = np.zeros((NPT, EL), np.float16)
    xr0 = np.zeros((NCORE, P, W * D), np.float16)
    rs0 = np.zeros((NCORE, P, W * D), np.float16)
    for c in range(NCORE):
        xs = xf[n0s[c]:n0s[c + 1]]
        xl0[c * N_pad:c * N_pad + Nl[c], :D] = (xs @ WlA_f[0] + blA_f[0]).astype(np.float16)
        xrv = (xs @ WrA_f[0]).astype(np.float16)          # [Nl, D]
        rsv = (xs @ np.asarray(Wres[0], np.float32)[:, perm]
               + np.asarray(bias[0], np.float32)[perm]).astype(np.float16)
        pad_s = np.zeros((N_pad - Nl[c], D), np.float16)
        xr0[c] = np.concatenate([xrv, pad_s]).reshape(W, P, D).transpose(1, 0, 2).reshape(P, W * D)
        rs0[c] = np.concatenate([rsv, pad_s]).reshape(W, P, D).transpose(1, 0, 2).reshape(P, W * D)

    in_maps = []
    arangeP = np.arange(P)
    for c in range(NCORE):
        sel = co == c
        oc = order[sel]
        pc = pos[sel]
        wc = wo[sel]
        bc = bo[sel]
        rc = rank[sel]
        es = gsrc[oc]

        dflat = np.full(E_pad, -1.0, np.float32)
        dflat[pc] = eslot_all[oc]
        ST_h = (dflat[None, :] == arangeP[:, None]).astype(np.float16)
        dc = dflat.reshape(nT, P).T
        S_h = (dc[:, :, None] == arangeP[None, None, :]).astype(
            np.float16).reshape(P, nT * P)

        eaf = np.zeros((ED + 1, E_pad), np.float16)
        eaf[:ED, pc] = ea[oc].T
        eaf[ED, pc] = 1.0

        mA = bc == 0
        idxA_flat = np.zeros(LA, np.int64)
        idxA_flat[tsA[wc[mA]] * P + rc[mA]] = es[mA]
        idxA = np.concatenate(
            [idxA_flat[tsA[w] * P:tsA[w + 1] * P].reshape(-1, 16).T
             for w in range(W)], axis=1).astype(np.int16)
        idxA = np.tile(idxA, (8, 1))
        if LB:
            mB = ~mA
            idxB_flat = np.zeros(LB, np.int64)
            idxB_flat[tsB[wc[mB]] * P + rc[mB]] = es[mB] - SPLIT
            idxB = np.concatenate(
                [idxB_flat[tsB[w] * P:tsB[w + 1] * P].reshape(-1, 16).T
                 for w in range(W) if tB_w[w]], axis=1).astype(np.int16)
            idxB = np.tile(idxB, (8, 1))
        else:
            idxB = np.zeros((P, 16), np.int16)

        lg = nb[n0s[c]:n0s[c + 1]] - c * gpc
        batch = np.full(N_pad, -1.0, np.float32)
        batch[:Nl[c]] = lg.astype(np.float32)
        valid = np.zeros(N_pad, np.float32)
        valid[:Nl[c]] = 1.0
        cnt = np.maximum(gb[np.minimum(c * gpc + np.arange(1, gpc + 1), G)]
                         - gb[np.minimum(c * gpc + np.arange(gpc), G)], 1)
        im = dict(shared)
        im.update({
            'idxA': idxA,
            'idxB': idxB,
            'ST_h': ST_h,
            'S_h': S_h,
            'eaT': eaf,
            'xl0': xl0,
            'xr0': xr0[c],
            'rs0': rs0[c],
            'batch_row': batch.reshape(1, N_pad).astype(np.float16),
            'batch_col': batch.reshape(W, P).T.copy(),
            'valid_col': valid.reshape(W, P).T.copy(),
            'invcntD': (1.0 / (cnt * D)).astype(np.float32).reshape(gpc, 1),
        })
        in_maps.append(im)

    dims = dict(N=N, D=D, E=E, ED=ED, L=L, HID=HID, G=G, H=H, C=C, gpc=gpc,
                N_pad=N_pad, W=W, NCH=NCH, NPT=NPT, nT=nT, E_pad=E_pad,
                T_w=[int(t) for t in T_w], tA_w=[int(t) for t in tA_w],
                tB_w=[int(t) for t in tB_w], tstart=[int(t) for t in tstart],
                colA=[int(t) for t in colA], colB=[int(t) for t in colB],
                nA_cols=int(colA[-1]), nB_cols=max(int(colB[-1]), 16),
                n0s=n0s, Nl=Nl, inv_perm=inv_perm)
    return in_maps, dims


# --------------------------------------------------------------- bass kernel
def build_nc(dims):
    D = dims['D']
    ED = dims['ED']
    L = dims['L']
    HID = dims['HID']
    H = dims['H']
    C = dims['C']
    DA = D + H
    gpc = dims['gpc']
    N_pad = dims['N_pad']
    W = dims['W']
    NCH = dims['NCH']
    NPT = dims['NPT']
    nT = dims['nT']
    E_pad = dims['E_pad']
    T_w = dims['T_w']
    tA_w = dims['tA_w']
    tB_w = dims['tB_w']
    tstart = dims['tstart']
    colA = dims['colA']
    colB = dims['colB']
    N = dims['N']
    HB = HID // P
    tpo = GMAX // P
    NB = min(REPS, 2)

    nc = bacc.Bacc("TRN2", target_bir_lowering=False, debug=False, num_devices=NCORE)

    def inp(name, shape, dt=F32):
        return nc.dram_tensor(name, list(shape), dt, kind="ExternalInput").ap()

    t_idxA = inp('idxA', (P, dims['nA_cols']), I16)
    t_idxB = inp('idxB', (P, dims['nB_cols']), I16)
    t_ST = inp('ST_h', (P, nT * P), F16)
    t_S = inp('S_h', (P, nT * P), F16)
    t_eaT = inp('eaT', (ED + 1, E_pad), F16)
    t_xl0 = inp('xl0', (NPT, EL), F16)
    t_xr0 = inp('xr0', (P, W * D), F16)
    t_rs0 = inp('rs0', (P, W * D), F16)
    t_batch_row = inp('batch_row', (1, N_pad), F16)
    t_batch_col = inp('batch_col', (P, W))
    t_valid_col = inp('valid_col', (P, W))
    t_invcntD = inp('invcntD', (gpc, 1))
    t_iota_row = inp('iota_row', (P, P), F16)
    t_giota_rep = inp('giota_rep', (P, gpc))
    t_giota_col = inp('giota_col', (gpc, 1))
    t_ident = inp('ident', (P, P), F16)
    t_ident32 = inp('ident32', (P, P), F32)
    t_ones_col = inp('ones_col', (P, 1))
    t_ones_col16 = inp('ones_col16', (P, 1), F16)
    t_ones_row = inp('ones_row', (1, 512))
    t_ones16 = inp('ones16', (1, P), F16)
    t_ashift = inp('ashift_col', (P, 1))
    tw = {}
    wspec = []
    for l in range(L):
        wspec += [(f'WlA{l}', (P, D), F16), (f'blA{l}', (1, D), F16),
                  (f'WrA{l}', (P, D), F16), (f'WeX{l}', (ED + 1, D), F16),
                  (f'Wres{l}', (P, D), F16), (f'combo{l}', (1, D), F16),
                  (f'attC{l}', (P, D), F16),
                  (f'W1_{l}', (P, HID), F32), (f'W2_{l}', (P, HID), F32),
                  (f'b2_{l}', (1, D), F32), (f'bng{l}', (1, HID), F32),
                  (f'bnb{l}', (1, HID), F32),
                  (f'lng{l}', (D, 1), F32), (f'lnb{l}', (D, 1), F32)]
    for key, shape, dt in wspec:
        tw[key] = inp(key, shape, dt)

    t_out = nc.dram_tensor('out_rows', [N_pad, D], F32, kind="ExternalOutput").ap()

    # layer-1 gather tables: AllGather output in Shared address space (fast
    # HBM-HBM collective path); input staged in Local scratch.
    t_xl1full = [nc.dram_tensor(f'xl1full{r}', [NPT, EL], F16,
                                kind="Internal", addr_space="Shared").ap()
                 for r in range(NB)]

    with tile.TileContext(nc) as tc:
        with tc.tile_pool(name="const", bufs=1) as cpool, \
             tc.tile_pool(name="dram", bufs=1, space="DRAM") as dpool, \
             tc.tile_pool(name="big", bufs=1) as bigpool:

            def ld(ap, shape, dt=F32, pool=cpool, name=None):
                if name is None:
                    name = 'c_' + ap.tensor.name
                t = pool.tile(list(shape), dt, name=name, tag=name)
                nc.sync.dma_start(t[:], ap[:])
                return t

            s_idxA = ld(t_idxA, (P, dims['nA_cols']), I16, bigpool)
            s_idxB = ld(t_idxB, (P, dims['nB_cols']), I16, bigpool)
            s_batch_col = ld(t_batch_col, (P, W))
            s_valid_col = ld(t_valid_col, (P, W))
            s_invcntD = ld(t_invcntD, (gpc, 1))
            s_iota_row = ld(t_iota_row, (P, P), F16)
            s_giota_rep = ld(t_giota_rep, (P, gpc))
            s_giota_col = ld(t_giota_col, (gpc, 1))
            s_ident = ld(t_ident, (P, P), F16)
            s_ident32 = ld(t_ident32, (P, P), F32)
            s_ones_col = ld(t_ones_col, (P, 1))
            s_ones_col16 = ld(t_ones_col16, (P, 1), F16)
            s_ones_row = ld(t_ones_row, (1, 512))
            s_ones16 = ld(t_ones16, (1, P), F16)
            s_ashift = ld(t_ashift, (P, 1))
            sw = {}
            for key, shape, dt in wspec:
                sw[key] = ld(tw[key], shape, dt)

            d_xl1loc_r = [dpool.tile([N_pad, EL], F16, tag=f'xl1loc{r}',
                                     name=f'd_xl1loc{r}') for r in range(NB)]
            d_arin = [dpool.tile([P, D + 1], F32, tag=f'arin{l}', name=f'd_arin{l}')
                      for l in range(L)]
            d_arout = [dpool.tile([P, D + 1], F32, tag=f'arout{l}', name=f'd_arout{l}')
                       for l in range(L)]

            x1Tb = [bigpool.tile([P, N_pad], F16, tag=f'x1T{i}', name=f'x1T{i}')
                    for i in range(NB)]
            x3Tb = [bigpool.tile([P, N_pad], F16, tag=f'x3T{i}', name=f'x3T{i}')
                    for i in range(NB)]

            from contextlib import ExitStack

            def emit_B(rep, l):
                x1T = x1Tb[rep % NB]
                x3T = x3Tb[rep % NB]
                tab = t_xl0 if l == 0 else t_xl1full[rep % NB]
                ctx = ExitStack()
                pC = ctx.enter_context(tc.tile_pool(name="pC", bufs=1))
                pCsp = ctx.enter_context(tc.tile_pool(name="pCs", bufs=1, space="PSUM"))
                pCs = pCsp.tile([P, D + 1], F32, tag='cs')
                with tc.tile_pool(name="pB", bufs=3) as pB, \
                     tc.tile_pool(name="pB1", bufs=3) as pB1, \
                     tc.tile_pool(name="pBz", bufs=3, space="PSUM") as pBz, \
                     tc.tile_pool(name="pBa", bufs=2, space="PSUM") as pBa, \
                     tc.tile_pool(name="pBr", bufs=1, space="PSUM") as pBr:
                    for w in range(W):
                        T = T_w[w]
                        tA = tA_w[w]
                        tB = tB_w[w]
                        tb = tstart[w]
                        EW = T * P
                        eat = pB.tile([ED + 1, EW], F16, tag='eat')
                        nc.sync.dma_start(eat[:], t_eaT[:, tb * P:tb * P + EW])
                        xsv = pB.tile([P, T * EL], F16, tag='xsv')
                        xsr = xsv[:].rearrange("p (t q) -> p t q", q=EL)
                        for o in range(0, tA, tpo):
                            t0, t1 = o, min(o + tpo, tA)
                            ni = (t1 - t0) * P
                            nc.gpsimd.dma_gather(
                                xsr[:, t0:t1, :], tab,
                                s_idxA[:, (colA[w] + t0 * 8):(colA[w] + t1 * 8)],
                                ni, ni, EL)
                        for o in range(0, tB, tpo):
                            t0, t1 = o, min(o + tpo, tB)
                            ni = (t1 - t0) * P
                            nc.gpsimd.dma_gather(
                                xsr[:, tA + t0:tA + t1, :], tab[SPLIT:, :],
                                s_idxB[:, (colB[w] + t0 * 8):(colB[w] + t1 * 8)],
                                ni, ni, EL)
                        if l == 0:
                            xrw = pB.tile([P, D], F16, tag='xrw')
                            nc.sync.dma_start(xrw[:], t_xr0[:, w * D:(w + 1) * D])
                            rsw = pB.tile([P, D], F16, tag='rsw')
                            nc.sync.dma_start(rsw[:], t_rs0[:, w * D:(w + 1) * D])
                            xrw_ap = xrw[:]
                        else:
                            pxr = pBr.tile([P, D], F32, tag='pxr')
                            nc.tensor.matmul(pxr[:], lhsT=x3T[:, w * P:(w + 1) * P],
                                             rhs=sw['WrA1'][:], start=True, stop=True)
                            xrw = pB.tile([P, D], F16, tag='xrw')
                            nc.scalar.activation(out=xrw[:], in_=pxr[:], func=AF.Copy)
                            xrw_ap = xrw[:]
                        # S / ST one-hots: host-built, DMA-loaded
                        S = pB1.tile([P, EW], F16, tag='S')
                        nc.sync.dma_start(S[:], t_S[:, tb * P:tb * P + EW])
                        ST = pB1.tile([P, EW], F16, tag='ST')
                        nc.sync.dma_start(ST[:], t_ST[:, tb * P:tb * P + EW])
                        m16 = pB1.tile([P, T * D], F16, tag='m16')
                        KP = 4
                        for g0 in range(0, T, KP):
                            gn = min(KP, T - g0)
                            pz3 = pBz.tile([P, KP * D], F32, tag='pz')
                            for u in range(gn):
                                t = g0 + u
                                sl = slice(u * D, (u + 1) * D)
                                nc.tensor.matmul(pz3[:, sl], lhsT=eat[:, t * P:(t + 1) * P],
                                                 rhs=sw[f'WeX{l}'][:], start=True, stop=False,
                                                 skip_group_check=True)
                                nc.tensor.matmul(pz3[:, sl], lhsT=ST[:, t * P:(t + 1) * P],
                                                 rhs=xrw_ap,
                                                 start=False, stop=True,
                                                 skip_group_check=True)
                            pzv = pz3[:].rearrange("p (u q) -> p u q", q=D)
                            mv = m16[:, g0 * D:(g0 + gn) * D].rearrange("p (u n) -> p u n", n=D)
                            nc.vector.tensor_tensor(out=mv, in0=pzv[:, 0:gn, :],
                                                    in1=xsr[:, g0:g0 + gn, 0:D],
                                                    op=ALU.add)
                            nc.scalar.activation(out=mv, in_=mv, func=AF.Prelu, alpha=NEG)
                        eng_tt = nc.vector if (w % 3 != 2) else nc.gpsimd
                        eng_tt.tensor_tensor(
                            out=m16[:].rearrange("p (t n) -> p t n", t=T),
                            in0=m16[:].rearrange("p (t n) -> p t n", t=T),
                            in1=sw[f'attC{l}'][:].rearrange("p (o n) -> p o n", o=1).to_broadcast([P, T, P]),
                            op=ALU.mult)
                        alpha2 = pB.tile([P, T * H], F32, tag='alpha2')
                        with nc.allow_low_precision(reason="16 fp16 terms, |alpha|<~30"):
                            nc.vector.tensor_reduce(
                                out=alpha2[:],
                                in_=m16[:].rearrange("p (t c h) -> p t h c", c=C, h=H),
                                axis=mybir.AxisListType.X, op=ALU.add)
                        ybuf = pB.tile([P, T * (D + 8)], F16, tag='ybuf')
                        yv = ybuf[:].rearrange("p (t q) -> p t q", q=D + 8)
                        nc.scalar.activation(
                            out=yv[:, :, D:D + 8],
                            in_=alpha2[:].rearrange("p (t h) -> p t h", t=T),
                            func=AF.Exp, bias=s_ashift[:])
                        nc.vector.tensor_tensor(
                            out=yv[:, :, 0:D].rearrange("p t (c h) -> p t c h", c=C),
                            in0=xsr[:, :, 0:D].rearrange("p t (c h) -> p t c h", c=C),
                            in1=yv[:, :, D:D + 8].rearrange("p t (o h) -> p t o h", o=1).to_broadcast([P, T, C, H]),
                            op=ALU.mult)
                        pagg = pBa.tile([P, D + 8], F32, tag='pagg')
                        for t in range(T):
                            nc.tensor.matmul(pagg[:], lhsT=S[:, t * P:(t + 1) * P],
                                             rhs=yv[:, t, :], start=(t == 0),
                                             stop=(t == T - 1))
                        den = pB.tile([P, H], F32, tag='den')
                        nc.vector.tensor_scalar(out=den[:], in0=pagg[:, D:D + 8],
                                                scalar1=1e-16, scalar2=None, op0=ALU.add)
                        rec = pB.tile([P, H], F32, tag='rec')
                        nc.vector.reciprocal(rec[:], den[:])
                        x1w = pB.tile([P, D + 1], F16, tag='x1w')
                        nc.vector.tensor_tensor(
                            out=x1w[:, 0:D].rearrange("p (c h) -> p c h", c=C),
                            in0=pagg[:, 0:D].rearrange("p (c h) -> p c h", c=C),
                            in1=rec[:].rearrange("p (o h) -> p o h", o=1).to_broadcast([P, C, H]),
                            op=ALU.mult)
                        if l == 0:
                            nc.vector.tensor_add(x1w[:, 0:D], x1w[:, 0:D],
                                                 rsw[:])
                        else:
                            pres = pBr.tile([P, P], F32, tag='pxr')
                            nc.tensor.matmul(pres[:], lhsT=x3T[:, w * P:(w + 1) * P],
                                             rhs=sw['Wres1'][:], start=True, stop=False)
                            nc.tensor.matmul(pres[:], lhsT=s_ones16[:, 0:1].to_broadcast([1, P]),
                                             rhs=sw['combo1'][:], start=False, stop=True)
                            nc.vector.tensor_add(x1w[:, 0:D], x1w[:, 0:D], pres[:])
                        nc.vector.tensor_scalar(out=x1w[:, 0:D], in0=x1w[:, 0:D],
                                                scalar1=s_valid_col[:, w:w + 1],
                                                scalar2=None, op0=ALU.mult)
                        nc.vector.tensor_copy(x1w[:, D:D + 1], s_valid_col[:, w:w + 1])
                        nc.tensor.matmul(pCs[:], lhsT=x1w[:, 0:D], rhs=x1w[:, 0:D + 1],
                                         start=(w == 0), stop=(w == W - 1),
                                         skip_group_check=True)
                        ptr = pBr.tile([P, P], F16, tag='ptr')
                        nc.tensor.transpose(out=ptr[:], in_=x1w[:, 0:D], identity=s_ident[:])
                        nc.scalar.activation(out=x1T[:, w * P:(w + 1) * P],
                                             in_=ptr[:], func=AF.Copy)
                return ctx, pC, pCs

            def emit_CDE(rep, l, ctx, pC, pCs):
                x1T = x1Tb[rep % NB]
                x3T = x3Tb[rep % NB]
                # ======== Phase C: BN stats (AllReduce) ==================
                pCp_cm = tc.tile_pool(name="pCp", bufs=1, space="PSUM")
                pCp = pCp_cm.__enter__()
                cs_sb = pC.tile([P, D + 1], F32, tag='cs')
                nc.vector.tensor_copy(cs_sb[:], pCs[:])
                nc.sync.dma_start(d_arin[l][:], cs_sb[:])
                nc.gpsimd.collective_compute(
                    "AllReduce", ALU.add,
                    replica_groups=[list(range(NCORE))],
                    ins=[d_arin[l][:].opt()], outs=[d_arout[l][:].opt()])
                csr = pC.tile([P, D + 1], F32, tag='csr')
                nc.sync.dma_start(csr[:], d_arout[l][:])
                mu = pC.tile([P, 1], F32, tag='mu')
                nc.vector.tensor_scalar(out=mu[:], in0=csr[:, D:D + 1],
                                        scalar1=1.0 / N, scalar2=None, op0=ALU.mult)
                pmu = pCp.tile([1, HID], F32, tag='pmu')
                nc.tensor.matmul(pmu[:], lhsT=mu[:], rhs=sw[f'W1_{l}'][:],
                                 start=True, stop=True)
                pP1 = pCp.tile([P, HID], F32, tag='pP1')
                nc.tensor.matmul(pP1[:], lhsT=csr[:, 0:D], rhs=sw[f'W1_{l}'][:],
                                 start=True, stop=True)
                w1p1 = pC.tile([P, HID], F32, tag='w1p1')
                nc.vector.tensor_tensor(out=w1p1[:], in0=sw[f'W1_{l}'][:],
                                        in1=pP1[:], op=ALU.mult)
                pt2 = pCp.tile([1, HID], F32, tag='pt2')
                nc.tensor.matmul(pt2[:], lhsT=s_ones_col[:], rhs=w1p1[:],
                                 start=True, stop=True)
                mh = pC.tile([1, HID], F32, tag='mh')
                nc.vector.tensor_copy(mh[:], pmu[:])
                var = pC.tile([1, HID], F32, tag='var')
                nc.vector.tensor_scalar(out=var[:], in0=pt2[:], scalar1=1.0 / N,
                                        scalar2=None, op0=ALU.mult)
                m2 = pC.tile([1, HID], F32, tag='m2')
                nc.vector.tensor_tensor(out=m2[:], in0=mh[:], in1=mh[:], op=ALU.mult)
                nc.vector.tensor_tensor(out=var[:], in0=var[:], in1=m2[:], op=ALU.subtract)
                sd = pC.tile([1, HID], F32, tag='sd')
                nc.vector.tensor_scalar(out=var[:], in0=var[:], scalar1=EPS,
                                        scalar2=None, op0=ALU.add)
                nc.scalar.activation(out=sd[:], in_=var[:], func=AF.Sqrt)
                rsd = pC.tile([1, HID], F32, tag='rsd')
                nc.vector.reciprocal(rsd[:], sd[:])
                geff = pC.tile([1, HID], F32, tag='geff')
                nc.vector.tensor_tensor(out=geff[:], in0=sw[f'bng{l}'][:],
                                        in1=rsd[:], op=ALU.mult)
                beff = pC.tile([1, HID], F32, tag='beff')
                nc.vector.tensor_tensor(out=beff[:], in0=mh[:], in1=geff[:], op=ALU.mult)
                nc.vector.tensor_tensor(out=beff[:], in0=sw[f'bnb{l}'][:],
                                        in1=beff[:], op=ALU.subtract)
                pgrep = pCp.tile([P, HID], F32, tag='pgrep')
                nc.tensor.matmul(pgrep[:], lhsT=s_ones_col[:1, :].rearrange("o p -> p o").to_broadcast([1, P]),
                                 rhs=geff[:], start=True, stop=True)
                w1eff = pC.tile([P, HID], F16, tag='w1eff')
                nc.vector.tensor_tensor(out=w1eff[:], in0=sw[f'W1_{l}'][:],
                                        in1=pgrep[:], op=ALU.mult)
                becol = pC.tile([P, HB], F32, tag='becol')
                for k in range(HB):
                    ptb = pCp.tile([P, 1], F32, tag='ptb')
                    nc.tensor.transpose(out=ptb[:], in_=beff[:, k * P:(k + 1) * P],
                                        identity=s_ident32[:1, :1])
                    nc.vector.tensor_copy(becol[:, k:k + 1], ptb[:])
                pCp_cm.__exit__(None, None, None)

                # ======== Phase D: MLP (x2 overwrites x1T in place) ======
                with tc.tile_pool(name="pD", bufs=2) as pD, \
                     tc.tile_pool(name="pDp", bufs=2, space="PSUM") as pDp, \
                     tc.tile_pool(name="pDx", bufs=2, space="PSUM") as pDx:
                    for i in range(NCH):
                        c0 = i * 512
                        px2 = pDx.tile([P, 512], F32, tag='px2')
                        for k in range(HB):
                            ph = pDp.tile([P, 512], F32, tag='ph')
                            nc.tensor.matmul(ph[:], lhsT=w1eff[:, k * P:(k + 1) * P],
                                             rhs=x1T[:, c0:c0 + 512],
                                             start=True, stop=True)
                            hs = pD.tile([P, 512], F32, tag='hs')
                            nc.scalar.activation(out=hs[:], in_=ph[:], func=AF.Relu,
                                                 bias=becol[:, k:k + 1], scale=1.0)
                            nc.tensor.matmul(px2[:], lhsT=sw[f'W2_{l}'][:, k * P:(k + 1) * P],
                                             rhs=hs[:], start=(k == 0), stop=False,
                                             skip_group_check=True)
                        nc.tensor.matmul(px2[:], lhsT=sw[f'b2_{l}'][:],
                                         rhs=s_ones_row[:], start=False, stop=True,
                                         skip_group_check=True)
                        nc.vector.tensor_add(x1T[:, c0:c0 + 512], px2[:],
                                             x1T[:, c0:c0 + 512])

                # ======== Phase E: graph LayerNorm (+ fused layer-1 table
                # production and AllGather when l == 0) ====================
                with tc.tile_pool(name="pE", bufs=2) as pE, \
                     tc.tile_pool(name="pEg", bufs=1, space="PSUM") as pEgp, \
                     tc.tile_pool(name="pEp", bufs=1, space="PSUM") as pEp:
                    pgs = pEgp.tile([gpc, 2], F32, tag='pgs')
                    for w in range(W):
                        sl = slice(w * P, (w + 1) * P)
                        sq = pE.tile([P, P], F16, tag='sq')
                        nc.scalar.activation(out=sq[:], in_=x1T[:, sl],
                                             func=AF.Square)
                        pcs = pEp.tile([1, 2 * P], F32, tag='pcs')
                        nc.tensor.matmul(pcs[:, 0:P], lhsT=s_ones_col16[:], rhs=x1T[:, sl],
                                         start=True, stop=True, skip_group_check=True)
                        nc.tensor.matmul(pcs[:, P:2 * P], lhsT=s_ones_col16[:], rhs=sq[:],
                                         start=True, stop=True, skip_group_check=True)
                        rows = pE.tile([1, 2 * P], F32, tag='rows')
                        nc.vector.tensor_copy(rows[:], pcs[:])
                        csc = pE.tile([P, 2], F32, tag='csc')
                        for q in range(2):
                            ptb = pEp.tile([P, 1], F32, tag='ptb2')
                            nc.tensor.transpose(out=ptb[:], in_=rows[:, q * P:(q + 1) * P],
                                                identity=s_ident32[:1, :1])
                            nc.vector.tensor_copy(csc[:, q:q + 1], ptb[:])
                        bg = pE.tile([P, gpc], F32, tag='bg')
                        nc.vector.tensor_scalar(out=bg[:], in0=s_giota_rep[:],
                                                scalar1=s_batch_col[:, w:w + 1],
                                                scalar2=None, op0=ALU.is_equal)
                        nc.tensor.matmul(pgs[:], lhsT=bg[:], rhs=csc[:],
                                         start=(w == 0), stop=(w == W - 1),
                                         skip_group_check=True)
                    gm = pE.tile([gpc, 1], F32, tag='gm')
                    nc.vector.tensor_tensor(out=gm[:], in0=pgs[:, 0:1],
                                            in1=s_invcntD[:], op=ALU.mult)
                    e2 = pE.tile([gpc, 1], F32, tag='e2')
                    nc.vector.tensor_tensor(out=e2[:], in0=pgs[:, 1:2],
                                            in1=s_invcntD[:], op=ALU.mult)
                    gv = pE.tile([gpc, 1], F32, tag='gv')
                    nc.vector.tensor_tensor(out=gv[:], in0=gm[:], in1=gm[:], op=ALU.mult)
                    nc.vector.tensor_tensor(out=gv[:], in0=e2[:], in1=gv[:], op=ALU.subtract)
                    sdg = pE.tile([gpc, 1], F32, tag='sdg')
                    nc.vector.tensor_scalar(out=gv[:], in0=gv[:], scalar1=EPS,
                                            scalar2=None, op0=ALU.add)
                    nc.scalar.activation(out=sdg[:], in_=gv[:], func=AF.Sqrt)
                    ivg = pE.tile([gpc, 1], F32, tag='ivg')
                    nc.vector.reciprocal(ivg[:], sdg[:])
                    gmr = pE.tile([gpc, P], F32, tag='gmr')
                    nc.vector.tensor_copy(gmr[:], gm[:].to_broadcast([gpc, P]))
                    ivr = pE.tile([gpc, P], F32, tag='ivr')
                    nc.vector.tensor_copy(ivr[:], ivg[:].to_broadcast([gpc, P]))
                    for i in range(NCH):
                        c0 = i * 512
                        brc = pE.tile([1, 512], F16, tag='brc')
                        nc.sync.dma_start(brc[:], t_batch_row[:, c0:c0 + 512])
                        pbr = pEp.tile([gpc, 512], F32, tag='pbr')
                        nc.tensor.matmul(pbr[:],
                                         lhsT=s_ones16[:, 0:1].to_broadcast([1, gpc]),
                                         rhs=brc[:],
                                         start=True, stop=True)
                        bgT = pE.tile([gpc, 512], F32, tag='bgT')
                        nc.vector.tensor_scalar(out=bgT[:], in0=pbr[:],
                                                scalar1=s_giota_col[:],
                                                scalar2=None, op0=ALU.is_equal)
                        pgm = pEp.tile([P, 512], F32, tag='pgm')
                        nc.tensor.matmul(pgm[:], lhsT=gmr[:], rhs=bgT[:],
                                         start=True, stop=True)
                        piv = pEp.tile([P, 512], F32, tag='piv')
                        nc.tensor.matmul(piv[:], lhsT=ivr[:], rhs=bgT[:],
                                         start=True, stop=True)
                        tmp = pE.tile([P, 512], F32, tag='tmp')
                        nc.vector.tensor_tensor(out=tmp[:], in0=x1T[:, c0:c0 + 512],
                                                in1=pgm[:], op=ALU.subtract)
                        nc.vector.tensor_tensor(out=tmp[:], in0=tmp[:],
                                                in1=piv[:], op=ALU.mult)
                        if l == 0:
                            nc.vector.tensor_scalar(out=x3T[:, c0:c0 + 512], in0=tmp[:],
                                                    scalar1=sw[f'lng{l}'][:],
                                                    scalar2=sw[f'lnb{l}'][:],
                                                    op0=ALU.mult, op1=ALU.add)
                            # fused layer-1 gather-table production
                            xa = pE.tile([P, 4 * D], F16, tag='xa')
                            for q in range(4):
                                pxa = pEp.tile([P, D], F32, tag='pxa')
                                nc.tensor.matmul(
                                    pxa[:], lhsT=x3T[:, c0 + q * P:c0 + (q + 1) * P],
                                    rhs=sw['WlA1'][:], start=True, stop=False)
                                nc.tensor.matmul(
                                    pxa[:], lhsT=s_ones16[:, 0:1].to_broadcast([1, P]),
                                    rhs=sw['blA1'][:], start=False, stop=True)
                                nc.vector.tensor_copy(xa[:, q * D:(q + 1) * D], pxa[:])
                            nc.sync.dma_start(
                                d_xl1loc_r[rep % NB][c0:c0 + 512, 0:D].rearrange(
                                    "(q p) d -> p q d", p=P),
                                xa[:].rearrange("p (q d) -> p q d", d=D))
                        else:
                            x3c = pE.tile([P, 512], F16, tag='x3c')
                            nc.vector.tensor_scalar(out=x3c[:], in0=tmp[:],
                                                    scalar1=sw[f'lng{l}'][:],
                                                    scalar2=sw[f'lnb{l}'][:],
                                                    op0=ALU.mult, op1=ALU.add)
                            for q in range(4):
                                ptb2 = pEp.tile([P, P], F16, tag='ptb2')
                                nc.tensor.transpose(out=ptb2[:],
                                                    in_=x3c[:, q * P:(q + 1) * P],
                                                    identity=s_ident[:])
                                orow = pE.tile([P, P], F32, tag='orow')
                                nc.vector.tensor_copy(orow[:], ptb2[:])
                                r0 = c0 + q * P
                                nc.sync.dma_start(t_out[r0:r0 + P, :], orow[:])
                if l == 0:
                    nc.gpsimd.collective_compute(
                        "AllGather", ALU.bypass,
                        replica_groups=[list(range(NCORE))],
                        ins=[d_xl1loc_r[rep % NB][:].opt()],
                        outs=[t_xl1full[rep % NB][:].opt()])
                ctx.close()

            for r in range(REPS):
                for l in range(L):
                    ctx, pC, pCs = emit_B(r, l)
                    emit_CDE(r, l, ctx, pC, pCs)

    nc.compile()
    return nc


# ---------------------------------------------------------------- entry point
_CACHE = {}


def kernel(**inputs):
    in_maps, dims = host_prep(**inputs)
    key = (REPS, dims['N'], dims['E'], dims['N_pad'], dims['nT'],
           tuple(dims['T_w']), tuple(dims['tA_w']))
    if key not in _CACHE:
        _CACHE[key] = build_nc(dims)
    nc = _CACHE[key]
    res = run_bass_kernel_spmd(nc, in_maps, core_ids=list(range(NCORE)), trace=False)
    global _last_res, _last_dims
    _last_res, _last_dims = res, dims
    N, D = dims['N'], dims['D']
    out = np.zeros((N, D), dtype=np.float32)
    n0s, Nl = dims['n0s'], dims['Nl']
    inv = dims['inv_perm']
    for c in range(NCORE):
        out[n0s[c]:n0s[c + 1]] = res.results[c]['out_rows'][:Nl[c]][:, inv]
    return out


# revision 16
# speedup vs baseline: 1.2844x; 1.0246x over previous
"""GATv2 block (2 layers) on 8 Trainium2 NeuronCores via Bass/Tile — v3.

Structure vs v2 baseline:
- Edge source features gathered with bulk dma_gather (split-table for int16
  index range), table rows padded to 512B.
- Destination transform xr never round-trips DRAM: per-window xr tile stays in
  SBUF and is injected into the per-edge PSUM via the transpose ST of the
  aggregation one-hot S.
- v4: the leaky-relu runs directly on the Activation engine (AF.Lrelu with
  alpha=0.2), so gather-table rows carry plain x@W (128 fp16 = 256B): half the
  gather + AllGather traffic of the 0.6z+0.4|z| linear-rider scheme, and one
  ACT op per PSUM group instead of two (fewer activation-table switches).
- v4: edge channels are stored c-major (new col = c*H + h) so the exp*xs
  multiply and the softmax divide hit the DVE packed fast path; weights are
  permuted host-side and the output is unpermuted in kernel().
- layer-1 gather table (xl1 = x3 @ Wl1) is produced chunk-by-chunk inside
  layer-0's LayerNorm phase and AllGathered into a Shared-address DRAM tensor.
- host_prep fully vectorized (sort-by-(core,window,split) + scatter).

softmax num/den accumulate via one-hot segment matmul; BN stats via Gram
matrix AllReduce; graph-LN via one-hot segment matmuls.
"""
import sys
import math

sys.path.insert(0, '/opt/trn_rl_repo')

import numpy as np
import concourse.bass as bass
import concourse.tile as tile
from concourse import bacc, mybir
from concourse.bass_utils import run_bass_kernel_spmd

F32 = mybir.dt.float32
F16 = mybir.dt.float16
I16 = mybir.dt.int16
AF = mybir.ActivationFunctionType
ALU = mybir.AluOpType

P = 128
NCORE = 8
NEG = 0.2
EPS = 1e-5
ASHIFT = -4.0   # constant softmax shift: exp(alpha-4) keeps fp16 exp in range
EL = 128        # fp16 elements per gather-table row (256B)
SPLIT = 32768   # int16 index split point
REPS = 1
GMAX = 896      # max idxs per dma_gather op


# ----------------------------------------------------------------- host prep
def host_prep(x, node_batch, edge_index, edge_attr, Wl, bl, Wr, br, We, att,
              bias, Wres, W1, b1, bn_gamma, bn_beta, W2, b2, ln_gamma, ln_beta):
    N, D = x.shape
    E = edge_index.shape[1]
    ED = edge_attr.shape[1]
    L = Wl.shape[0]
    HID = W1.shape[2]
    G = int(node_batch.max()) + 1
    H = att.shape[1]
    C = att.shape[2]
    DA = D + H
    gpc = (G + NCORE - 1) // NCORE

    nb = np.asarray(node_batch).astype(np.int64)
    src = np.asarray(edge_index[0]).astype(np.int64)
    dst = np.asarray(edge_index[1]).astype(np.int64)
    ea = np.asarray(edge_attr, dtype=np.float32)
    xf = np.asarray(x, np.float32)

    gb = np.searchsorted(nb, np.arange(G + 1))
    n0s = np.array([gb[min(c * gpc, G)] for c in range(NCORE + 1)], dtype=np.int64)
    Nl = n0s[1:] - n0s[:-1]
    N_pad = int(math.ceil(max(Nl.max(), 1) / 512.0) * 512)
    W = N_pad // P
    NCH = N_pad // 512
    NPT = NCORE * N_pad

    core_of = np.searchsorted(n0s, np.arange(N), side='right') - 1
    glob_id = (core_of * N_pad + (np.arange(N) - n0s[core_of])).astype(np.int64)

    ecore = core_of[dst]
    gsrc = glob_id[src]
    dslot_all = dst - n0s[ecore]
    ewin_all = dslot_all // P
    eslot_all = dslot_all % P
    isB = (gsrc >= SPLIT).astype(np.int64)

    # Per (core, window, split): counts -> shared tile layout (max over cores).
    key = (ecore * W + ewin_all) * 2 + isB
    cnt2 = np.bincount(key, minlength=NCORE * W * 2).reshape(NCORE, W, 2)
    nA, nB = cnt2[..., 0], cnt2[..., 1]
    tA_w = np.maximum(np.ceil(nA.max(axis=0) / P).astype(np.int64), 1)
    tB_w = np.ceil(nB.max(axis=0) / P).astype(np.int64)
    T_w = tA_w + tB_w
    tstart = np.concatenate([[0], np.cumsum(T_w)])
    nT = int(tstart[-1])
    E_pad = nT * P
    tsA = np.concatenate([[0], np.cumsum(tA_w)])
    tsB = np.concatenate([[0], np.cumsum(tB_w)])
    baseA = tstart[:-1] * P
    baseB = baseA + tA_w * P
    colA = np.concatenate([[0], np.cumsum(tA_w * (P // 16))])
    colB = np.concatenate([[0], np.cumsum(tB_w * (P // 16))])
    LA = int(tsA[-1]) * P
    LB = int(tsB[-1]) * P

    # Stable sort by (core, window, split); rank within group gives each edge
    # a unique slot in its window's tile range.
    order = np.argsort(key, kind='stable')
    sk = key[order]
    starts = np.zeros(E, np.int64)
    gs = np.r_[0, np.flatnonzero(np.diff(sk)) + 1]
    starts[gs] = gs
    starts = np.maximum.accumulate(starts)
    rank = np.arange(E) - starts
    wo = ewin_all[order]
    bo = isB[order]
    co = ecore[order]
    pos = np.where(bo == 0, baseA[wo], baseB[wo]) + rank

    # c-major channel permutation: new col j = c*H + h holds old channel
    # h*C + c. Makes the exp*xs multiply and softmax divide DVE-packed.
    perm = np.array([h * C + c for c in range(C) for h in range(H)])
    inv_perm = np.argsort(perm)

    shared = {
        'iota_row': np.tile(np.arange(P, dtype=np.float16), (P, 1)),
        'giota_rep': np.tile(np.arange(gpc, dtype=np.float32), (P, 1)),
        'giota_col': np.arange(gpc, dtype=np.float32).reshape(gpc, 1),
        'ident': np.eye(P, dtype=np.float16),
        'ident32': np.eye(P, dtype=np.float32),
        'ones_col': np.ones((P, 1), np.float32),
        'ones_col16': np.ones((P, 1), np.float16),
        'ones_row': np.ones((1, 512), np.float32),
        'ones16': np.ones((1, P), np.float16),
        'ashift_col': np.full((P, 1), ASHIFT, np.float32),
    }
    WlA_f, blA_f, WrA_f = [], [], []
    for l in range(L):
        Wl_ = np.asarray(Wl[l], np.float32)
        Wr_ = np.asarray(Wr[l], np.float32)
        We_ = np.asarray(We[l], np.float32)
        bl_ = np.asarray(bl[l], np.float32)
        br_ = np.asarray(br[l], np.float32)
        # layer >= 1 inputs live in the permuted basis: permute weight ROWS.
        rp = perm if l >= 1 else np.arange(D)
        Wl_r = Wl_[rp]
        Wr_r = Wr_[rp]
        Wres_r = np.asarray(Wres[l], np.float32)[rp]
        WlA_f.append(Wl_r[:, perm])
        blA_f.append(bl_[perm])
        WrA_f.append(Wr_r[:, perm])
        shared[f'WlA{l}'] = WlA_f[l].astype(np.float16)
        shared[f'blA{l}'] = blA_f[l].reshape(1, D).astype(np.float16)
        shared[f'WrA{l}'] = WrA_f[l].astype(np.float16)
        wex = np.concatenate([We_, br_.reshape(1, D)], 0)
        shared[f'WeX{l}'] = wex[:, perm].astype(np.float16)
        shared[f'Wres{l}'] = Wres_r[:, perm].astype(np.float16)
        shared[f'combo{l}'] = np.asarray(bias[l], np.float32)[perm].astype(np.float16).reshape(1, D)
        # full att vector, c-major: [P, D]
        aC = np.asarray(att[l], np.float32).reshape(H * C)[perm].astype(np.float16)
        shared[f'attC{l}'] = np.tile(aC.reshape(1, D), (P, 1))
        shared[f'W1_{l}'] = np.asarray(W1[l], np.float32)[perm]
        w2 = np.asarray(W2[l], np.float32)[:, perm]
        shared[f'W2_{l}'] = np.concatenate(
            [w2[k * P:(k + 1) * P, :] for k in range(HID // P)], axis=1)
        shared[f'b2_{l}'] = np.asarray(b2[l], np.float32)[perm].reshape(1, D)
        shared[f'bng{l}'] = np.asarray(bn_gamma[l], np.float32).reshape(1, HID)
        shared[f'bnb{l}'] = np.asarray(bn_beta[l], np.float32).reshape(1, HID)
        shared[f'lng{l}'] = np.asarray(ln_gamma[l], np.float32)[perm].reshape(D, 1)
        shared[f'lnb{l}'] = np.asarray(ln_beta[l], np.float32)[perm].reshape(D, 1)

    # layer-0 host precomputes: gather table, xr0, resid0
    xl0 = np.zeros((NPT, EL), np.float16)
    xr0 = np.zeros((NCORE, P, W * D), np.float16)
    rs0 = np.zeros((NCORE, P, W * D), np.float16)
    for c in range(NCORE):
        xs = xf[n0s[c]:n0s[c + 1]]
        xl0[c * N_pad:c * N_pad + Nl[c], :D] = (xs @ WlA_f[0] + blA_f[0]).astype(np.float16)
        xrv = (xs @ WrA_f[0]).astype(np.float16)          # [Nl, D]
        rsv = (xs @ np.asarray(Wres[0], np.float32)[:, perm]
               + np.asarray(bias[0], np.float32)[perm]).astype(np.float16)
        pad_s = np.zeros((N_pad - Nl[c], D), np.float16)
        xr0[c] = np.concatenate([xrv, pad_s]).reshape(W, P, D).transpose(1, 0, 2).reshape(P, W * D)
        rs0[c] = np.concatenate([rsv, pad_s]).reshape(W, P, D).transpose(1, 0, 2).reshape(P, W * D)

    in_maps = []
    arangeP = np.arange(P)
    for c in range(NCORE):
        sel = co == c
        oc = order[sel]
        pc = pos[sel]
        wc = wo[sel]
        bc = bo[sel]
        rc = rank[sel]
        es = gsrc[oc]

        dflat = np.full(E_pad, -1.0, np.float32)
        dflat[pc] = eslot_all[oc]
        ST_h = (dflat[None, :] == arangeP[:, None]).astype(np.float16)
        dc = dflat.reshape(nT, P).T
        S_h = (dc[:, :, None] == arangeP[None, None, :]).astype(
            np.float16).reshape(P, nT * P)

        eaf = np.zeros((ED + 1, E_pad), np.float16)
        eaf[:ED, pc] = ea[oc].T
        eaf[ED, pc] = 1.0

        mA = bc == 0
        idxA_flat = np.zeros(LA, np.int64)
        idxA_flat[tsA[wc[mA]] * P + rc[mA]] = es[mA]
        idxA = np.concatenate(
            [idxA_flat[tsA[w] * P:tsA[w + 1] * P].reshape(-1, 16).T
             for w in range(W)], axis=1).astype(np.int16)
        idxA = np.tile(idxA, (8, 1))
        if LB:
            mB = ~mA
            idxB_flat = np.zeros(LB, np.int64)
            idxB_flat[tsB[wc[mB]] * P + rc[mB]] = es[mB] - SPLIT
            idxB = np.concatenate(
                [idxB_flat[tsB[w] * P:tsB[w + 1] * P].reshape(-1, 16).T
                 for w in range(W) if tB_w[w]], axis=1).astype(np.int16)
            idxB = np.tile(idxB, (8, 1))
        else:
            idxB = np.zeros((P, 16), np.int16)

        lg = nb[n0s[c]:n0s[c + 1]] - c * gpc
        batch = np.full(N_pad, -1.0, np.float32)
        batch[:Nl[c]] = lg.astype(np.float32)
        valid = np.zeros(N_pad, np.float32)
        valid[:Nl[c]] = 1.0
        cnt = np.maximum(gb[np.minimum(c * gpc + np.arange(1, gpc + 1), G)]
                         - gb[np.minimum(c * gpc + np.arange(gpc), G)], 1)
        im = dict(shared)
        im.update({
            'idxA': idxA,
            'idxB': idxB,
            'ST_h': ST_h,
            'S_h': S_h,
            'eaT': eaf,
            'xl0': xl0,
            'xr0': xr0[c],
            'rs0': rs0[c],
            'batch_row': batch.reshape(1, N_pad).astype(np.float16),
            'batch_col': batch.reshape(W, P).T.copy(),
            'valid_col': valid.reshape(W, P).T.copy(),
            'invcntD': (1.0 / (cnt * D)).astype(np.float32).reshape(gpc, 1),
        })
        in_maps.append(im)

    dims = dict(N=N, D=D, E=E, ED=ED, L=L, HID=HID, G=G, H=H, C=C, gpc=gpc,
                N_pad=N_pad, W=W, NCH=NCH, NPT=NPT, nT=nT, E_pad=E_pad,
                T_w=[int(t) for t in T_w], tA_w=[int(t) for t in tA_w],
                tB_w=[int(t) for t in tB_w], tstart=[int(t) for t in tstart],
                colA=[int(t) for t in colA], colB=[int(t) for t in colB],
                nA_cols=int(colA[-1]), nB_cols=max(int(colB[-1]), 16),
                n0s=n0s, Nl=Nl, inv_perm=inv_perm)
    return in_maps, dims


# --------------------------------------------------------------- bass kernel
def build_nc(dims):
    D = dims['D']
    ED = dims['ED']
    L = dims['L']
    HID = dims['HID']
    H = dims['H']
    C = dims['C']
    DA = D + H
    gpc = dims['gpc']
    N_pad = dims['N_pad']
    W = dims['W']
    NCH = dims['NCH']
    NPT = dims['NPT']
    nT = dims['nT']
    E_pad = dims['E_pad']
    T_w = dims['T_w']
    tA_w = dims['tA_w']
    tB_w = dims['tB_w']
    tstart = dims['tstart']
    colA = dims['colA']
    colB = dims['colB']
    N = dims['N']
    HB = HID // P
    tpo = GMAX // P
    NB = min(REPS, 2)

    nc = bacc.Bacc("TRN2", target_bir_lowering=False, debug=False, num_devices=NCORE)

    def inp(name, shape, dt=F32):
        return nc.dram_tensor(name, list(shape), dt, kind="ExternalInput").ap()

    t_idxA = inp('idxA', (P, dims['nA_cols']), I16)
    t_idxB = inp('idxB', (P, dims['nB_cols']), I16)
    t_ST = inp('ST_h', (P, nT * P), F16)
    t_S = inp('S_h', (P, nT * P), F16)
    t_eaT = inp('eaT', (ED + 1, E_pad), F16)
    t_xl0 = inp('xl0', (NPT, EL), F16)
    t_xr0 = inp('xr0', (P, W * D), F16)
    t_rs0 = inp('rs0', (P, W * D), F16)
    t_batch_row = inp('batch_row', (1, N_pad), F16)
    t_batch_col = inp('batch_col', (P, W))
    t_valid_col = inp('valid_col', (P, W))
    t_invcntD = inp('invcntD', (gpc, 1))
    t_iota_row = inp('iota_row', (P, P), F16)
    t_giota_rep = inp('giota_rep', (P, gpc))
    t_giota_col = inp('giota_col', (gpc, 1))
    t_ident = inp('ident', (P, P), F16)
    t_ident32 = inp('ident32', (P, P), F32)
    t_ones_col = inp('ones_col', (P, 1))
    t_ones_col16 = inp('ones_col16', (P, 1), F16)
    t_ones_row = inp('ones_row', (1, 512))
    t_ones16 = inp('ones16', (1, P), F16)
    t_ashift = inp('ashift_col', (P, 1))
    tw = {}
    wspec = []
    for l in range(L):
        wspec += [(f'WlA{l}', (P, D), F16), (f'blA{l}', (1, D), F16),
                  (f'WrA{l}', (P, D), F16), (f'WeX{l}', (ED + 1, D), F16),
                  (f'Wres{l}', (P, D), F16), (f'combo{l}', (1, D), F16),
                  (f'attC{l}', (P, D), F16),
                  (f'W1_{l}', (P, HID), F32), (f'W2_{l}', (P, HID), F32),
                  (f'b2_{l}', (1, D), F32), (f'bng{l}', (1, HID), F32),
                  (f'bnb{l}', (1, HID), F32),
                  (f'lng{l}', (D, 1), F32), (f'lnb{l}', (D, 1), F32)]
    for key, shape, dt in wspec:
        tw[key] = inp(key, shape, dt)

    t_out = nc.dram_tensor('out_rows', [N_pad, D], F32, kind="ExternalOutput").ap()

    # layer-1 gather tables: AllGather output in Shared address space (fast
    # HBM-HBM collective path); input staged in Local scratch.
    t_xl1full = [nc.dram_tensor(f'xl1full{r}', [NPT, EL], F16,
                                kind="Internal", addr_space="Shared").ap()
                 for r in range(NB)]

    with tile.TileContext(nc) as tc:
        with tc.tile_pool(name="const", bufs=1) as cpool, \
             tc.tile_pool(name="dram", bufs=1, space="DRAM") as dpool, \
             tc.tile_pool(name="big", bufs=1) as bigpool:

            def ld(ap, shape, dt=F32, pool=cpool, name=None):
                if name is None:
                    name = 'c_' + ap.tensor.name
                t = pool.tile(list(shape), dt, name=name, tag=name)
                nc.sync.dma_start(t[:], ap[:])
                return t

            s_idxA = ld(t_idxA, (P, dims['nA_cols']), I16, bigpool)
            s_idxB = ld(t_idxB, (P, dims['nB_cols']), I16, bigpool)
            s_batch_col = ld(t_batch_col, (P, W))
            s_valid_col = ld(t_valid_col, (P, W))
            s_invcntD = ld(t_invcntD, (gpc, 1))
            s_iota_row = ld(t_iota_row, (P, P), F16)
            s_giota_rep = ld(t_giota_rep, (P, gpc))
            s_giota_col = ld(t_giota_col, (gpc, 1))
            s_ident = ld(t_ident, (P, P), F16)
            s_ident32 = ld(t_ident32, (P, P), F32)
            s_ones_col = ld(t_ones_col, (P, 1))
            s_ones_col16 = ld(t_ones_col16, (P, 1), F16)
            s_ones_row = ld(t_ones_row, (1, 512))
            s_ones16 = ld(t_ones16, (1, P), F16)
            s_ashift = ld(t_ashift, (P, 1))
            sw = {}
            for key, shape, dt in wspec:
                sw[key] = ld(tw[key], shape, dt)

            d_xl1loc_r = [dpool.tile([N_pad, EL], F16, tag=f'xl1loc{r}',
                                     name=f'd_xl1loc{r}') for r in range(NB)]
            d_arin = [dpool.tile([P, D + 1], F32, tag=f'arin{l}', name=f'd_arin{l}')
                      for l in range(L)]
            d_arout = [dpool.tile([P, D + 1], F32, tag=f'arout{l}', name=f'd_arout{l}')
                       for l in range(L)]

            x1Tb = [bigpool.tile([P, N_pad], F16, tag=f'x1T{i}', name=f'x1T{i}')
                    for i in range(NB)]
            x3Tb = [bigpool.tile([P, N_pad], F16, tag=f'x3T{i}', name=f'x3T{i}')
                    for i in range(NB)]

            from contextlib import ExitStack

            def emit_B(rep, l):
                x1T = x1Tb[rep % NB]
                x3T = x3Tb[rep % NB]
                tab = t_xl0 if l == 0 else t_xl1full[rep % NB]
                ctx = ExitStack()
                pC = ctx.enter_context(tc.tile_pool(name="pC", bufs=1))
                pCsp = ctx.enter_context(tc.tile_pool(name="pCs", bufs=1, space="PSUM"))
                pCs = pCsp.tile([P, D + 1], F32, tag='cs')
                with tc.tile_pool(name="pB", bufs=3) as pB, \
                     tc.tile_pool(name="pB1", bufs=2) as pB1, \
                     tc.tile_pool(name="pBz", bufs=3, space="PSUM") as pBz, \
                     tc.tile_pool(name="pBa", bufs=2, space="PSUM") as pBa, \
                     tc.tile_pool(name="pBr", bufs=1, space="PSUM") as pBr:
                    if l == 1:
                        # all windows' dst transforms up front: independent of
                        # the gather table, so PE computes them during the
                        # AllGather instead of stalling behind window 0.
                        xr1T = pC.tile([P, W * D], F16, tag='xr1T')
                        for w2 in range(W):
                            pxr = pBr.tile([P, D], F32, tag='pxr')
                            nc.tensor.matmul(pxr[:], lhsT=x3T[:, w2 * P:(w2 + 1) * P],
                                             rhs=sw['WrA1'][:], start=True, stop=True)
                            nc.scalar.activation(out=xr1T[:, w2 * D:(w2 + 1) * D],
                                                 in_=pxr[:], func=AF.Copy)
                    for w in range(W):
                        T = T_w[w]
                        tA = tA_w[w]
                        tB = tB_w[w]
                        tb = tstart[w]
                        EW = T * P
                        eat = pB.tile([ED + 1, EW], F16, tag='eat')
                        nc.sync.dma_start(eat[:], t_eaT[:, tb * P:tb * P + EW])
                        xsv = pB.tile([P, T * EL], F16, tag='xsv')
                        xsr = xsv[:].rearrange("p (t q) -> p t q", q=EL)
                        for o in range(0, tA, tpo):
                            t0, t1 = o, min(o + tpo, tA)
                            ni = (t1 - t0) * P
                            nc.gpsimd.dma_gather(
                                xsr[:, t0:t1, :], tab,
                                s_idxA[:, (colA[w] + t0 * 8):(colA[w] + t1 * 8)],
                                ni, ni, EL)
                        for o in range(0, tB, tpo):
                            t0, t1 = o, min(o + tpo, tB)
                            ni = (t1 - t0) * P
                            nc.gpsimd.dma_gather(
                                xsr[:, tA + t0:tA + t1, :], tab[SPLIT:, :],
                                s_idxB[:, (colB[w] + t0 * 8):(colB[w] + t1 * 8)],
                                ni, ni, EL)
                        if l == 0:
                            xrw = pB.tile([P, D], F16, tag='xrw')
                            nc.sync.dma_start(xrw[:], t_xr0[:, w * D:(w + 1) * D])
                            rsw = pB.tile([P, D], F16, tag='rsw')
                            nc.sync.dma_start(rsw[:], t_rs0[:, w * D:(w + 1) * D])
                            xrw_ap = xrw[:]
                        else:
                            xrw_ap = xr1T[:, w * D:(w + 1) * D]
                        # S / ST one-hots: host-built, DMA-loaded
                        S = pB1.tile([P, EW], F16, tag='S')
                        nc.sync.dma_start(S[:], t_S[:, tb * P:tb * P + EW])
                        ST = pB1.tile([P, EW], F16, tag='ST')
                        nc.sync.dma_start(ST[:], t_ST[:, tb * P:tb * P + EW])
                        m16 = pB1.tile([P, T * D], F16, tag='m16')
                        KP = 4
                        for g0 in range(0, T, KP):
                            gn = min(KP, T - g0)
                            pz3 = pBz.tile([P, KP * D], F32, tag='pz')
                            for u in range(gn):
                                t = g0 + u
                                sl = slice(u * D, (u + 1) * D)
                                nc.tensor.matmul(pz3[:, sl], lhsT=eat[:, t * P:(t + 1) * P],
                                                 rhs=sw[f'WeX{l}'][:], start=True, stop=False,
                                                 skip_group_check=True)
                                nc.tensor.matmul(pz3[:, sl], lhsT=ST[:, t * P:(t + 1) * P],
                                                 rhs=xrw_ap,
                                                 start=False, stop=True,
                                                 skip_group_check=True)
                            pzv = pz3[:].rearrange("p (u q) -> p u q", q=D)
                            mv = m16[:, g0 * D:(g0 + gn) * D].rearrange("p (u n) -> p u n", n=D)
                            nc.vector.tensor_tensor(out=mv, in0=pzv[:, 0:gn, :],
                                                    in1=xsr[:, g0:g0 + gn, 0:D],
                                                    op=ALU.add)
                            nc.scalar.activation(out=mv, in_=mv, func=AF.Prelu, alpha=NEG)
                        eng_tt = nc.vector if (w % 3 != 2) else nc.gpsimd
                        eng_tt.tensor_tensor(
                            out=m16[:].rearrange("p (t n) -> p t n", t=T),
                            in0=m16[:].rearrange("p (t n) -> p t n", t=T),
                            in1=sw[f'attC{l}'][:].rearrange("p (o n) -> p o n", o=1).to_broadcast([P, T, P]),
                            op=ALU.mult)
                        alpha2 = pB.tile([P, T * H], F32, tag='alpha2')
                        with nc.allow_low_precision(reason="16 fp16 terms, |alpha|<~30"):
                            nc.vector.tensor_reduce(
                                out=alpha2[:],
                                in_=m16[:].rearrange("p (t c h) -> p t h c", c=C, h=H),
                                axis=mybir.AxisListType.X, op=ALU.add)
                        ybuf = pB.tile([P, T * (D + 8)], F16, tag='ybuf')
                        yv = ybuf[:].rearrange("p (t q) -> p t q", q=D + 8)
                        nc.scalar.activation(
                            out=yv[:, :, D:D + 8],
                            in_=alpha2[:].rearrange("p (t h) -> p t h", t=T),
                            func=AF.Exp, bias=s_ashift[:])
                        nc.vector.tensor_tensor(
                            out=yv[:, :, 0:D].rearrange("p t (c h) -> p t c h", c=C),
                            in0=xsr[:, :, 0:D].rearrange("p t (c h) -> p t c h", c=C),
                            in1=yv[:, :, D:D + 8].rearrange("p t (o h) -> p t o h", o=1).to_broadcast([P, T, C, H]),
                            op=ALU.mult)
                        pagg = pBa.tile([P, D + 8], F32, tag='pagg')
                        for t in range(T):
                            nc.tensor.matmul(pagg[:], lhsT=S[:, t * P:(t + 1) * P],
                                             rhs=yv[:, t, :], start=(t == 0),
                                             stop=(t == T - 1))
                        den = pB.tile([P, H], F32, tag='den')
                        nc.vector.tensor_scalar(out=den[:], in0=pagg[:, D:D + 8],
                                                scalar1=1e-16, scalar2=None, op0=ALU.add)
                        rec = pB.tile([P, H], F32, tag='rec')
                        nc.vector.reciprocal(rec[:], den[:])
                        x1w = pB.tile([P, D + 1], F16, tag='x1w')
                        nc.vector.tensor_tensor(
                            out=x1w[:, 0:D].rearrange("p (c h) -> p c h", c=C),
                            in0=pagg[:, 0:D].rearrange("p (c h) -> p c h", c=C),
                            in1=rec[:].rearrange("p (o h) -> p o h", o=1).to_broadcast([P, C, H]),
                            op=ALU.mult)
                        if l == 0:
                            nc.vector.tensor_add(x1w[:, 0:D], x1w[:, 0:D],
                                                 rsw[:])
                        else:
                            pres = pBr.tile([P, P], F32, tag='pxr')
                            nc.tensor.matmul(pres[:], lhsT=x3T[:, w * P:(w + 1) * P],
                                             rhs=sw['Wres1'][:], start=True, stop=False)
                            nc.tensor.matmul(pres[:], lhsT=s_ones16[:, 0:1].to_broadcast([1, P]),
                                             rhs=sw['combo1'][:], start=False, stop=True)
                            nc.vector.tensor_add(x1w[:, 0:D], x1w[:, 0:D], pres[:])
                        nc.vector.tensor_scalar(out=x1w[:, 0:D], in0=x1w[:, 0:D],
                                                scalar1=s_valid_col[:, w:w + 1],
                                                scalar2=None, op0=ALU.mult)
                        nc.vector.tensor_copy(x1w[:, D:D + 1], s_valid_col[:, w:w + 1])
                        nc.tensor.matmul(pCs[:], lhsT=x1w[:, 0:D], rhs=x1w[:, 0:D + 1],
                                         start=(w == 0), stop=(w == W - 1),
                                         skip_group_check=True)
                        ptr = pBr.tile([P, P], F16, tag='ptr')
                        nc.tensor.transpose(out=ptr[:], in_=x1w[:, 0:D], identity=s_ident[:])
                        nc.scalar.activation(out=x1T[:, w * P:(w + 1) * P],
                                             in_=ptr[:], func=AF.Copy)
                return ctx, pC, pCs

            def emit_CDE(rep, l, ctx, pC, pCs):
                x1T = x1Tb[rep % NB]
                x3T = x3Tb[rep % NB]
                # ======== Phase C: BN stats (AllReduce) ==================
                pCp_cm = tc.tile_pool(name="pCp", bufs=1, space="PSUM")
                pCp = pCp_cm.__enter__()
                cs_sb = pC.tile([P, D + 1], F32, tag='cs')
                nc.vector.tensor_copy(cs_sb[:], pCs[:])
                nc.sync.dma_start(d_arin[l][:], cs_sb[:])
                nc.gpsimd.collective_compute(
                    "AllReduce", ALU.add,
                    replica_groups=[list(range(NCORE))],
                    ins=[d_arin[l][:].opt()], outs=[d_arout[l][:].opt()])
                csr = pC.tile([P, D + 1], F32, tag='csr')
                nc.sync.dma_start(csr[:], d_arout[l][:])
                mu = pC.tile([P, 1], F32, tag='mu')
                nc.vector.tensor_scalar(out=mu[:], in0=csr[:, D:D + 1],
                                        scalar1=1.0 / N, scalar2=None, op0=ALU.mult)
                pmu = pCp.tile([1, HID], F32, tag='pmu')
                nc.tensor.matmul(pmu[:], lhsT=mu[:], rhs=sw[f'W1_{l}'][:],
                                 start=True, stop=True)
                pP1 = pCp.tile([P, HID], F32, tag='pP1')
                nc.tensor.matmul(pP1[:], lhsT=csr[:, 0:D], rhs=sw[f'W1_{l}'][:],
                                 start=True, stop=True)
                w1p1 = pC.tile([P, HID], F32, tag='w1p1')
                nc.vector.tensor_tensor(out=w1p1[:], in0=sw[f'W1_{l}'][:],
                                        in1=pP1[:], op=ALU.mult)
                pt2 = pCp.tile([1, HID], F32, tag='pt2')
                nc.tensor.matmul(pt2[:], lhsT=s_ones_col[:], rhs=w1p1[:],
                                 start=True, stop=True)
                mh = pC.tile([1, HID], F32, tag='mh')
                nc.vector.tensor_copy(mh[:], pmu[:])
                var = pC.tile([1, HID], F32, tag='var')
                nc.vector.tensor_scalar(out=var[:], in0=pt2[:], scalar1=1.0 / N,
                                        scalar2=None, op0=ALU.mult)
                m2 = pC.tile([1, HID], F32, tag='m2')
                nc.vector.tensor_tensor(out=m2[:], in0=mh[:], in1=mh[:], op=ALU.mult)
                nc.vector.tensor_tensor(out=var[:], in0=var[:], in1=m2[:], op=ALU.subtract)
                sd = pC.tile([1, HID], F32, tag='sd')
                nc.vector.tensor_scalar(out=var[:], in0=var[:], scalar1=EPS,
                                        scalar2=None, op0=ALU.add)
                nc.scalar.activation(out=sd[:], in_=var[:], func=AF.Sqrt)
                rsd = pC.tile([1, HID], F32, tag='rsd')
                nc.vector.reciprocal(rsd[:], sd[:])
                geff = pC.tile([1, HID], F32, tag='geff')
                nc.vector.tensor_tensor(out=geff[:], in0=sw[f'bng{l}'][:],
                                        in1=rsd[:], op=ALU.mult)
                beff = pC.tile([1, HID], F32, tag='beff')
                nc.vector.tensor_tensor(out=beff[:], in0=mh[:], in1=geff[:], op=ALU.mult)
                nc.vector.tensor_tensor(out=beff[:], in0=sw[f'bnb{l}'][:],
                                        in1=beff[:], op=ALU.subtract)
                pgrep = pCp.tile([P, HID], F32, tag='pgrep')
                nc.tensor.matmul(pgrep[:], lhsT=s_ones_col[:1, :].rearrange("o p -> p o").to_broadcast([1, P]),
                                 rhs=geff[:], start=True, stop=True)
                w1eff = pC.tile([P, HID], F16, tag='w1eff')
                nc.vector.tensor_tensor(out=w1eff[:], in0=sw[f'W1_{l}'][:],
                                        in1=pgrep[:], op=ALU.mult)
                becol = pC.tile([P, HB], F32, tag='becol')
                for k in range(HB):
                    ptb = pCp.tile([P, 1], F32, tag='ptb')
                    nc.tensor.transpose(out=ptb[:], in_=beff[:, k * P:(k + 1) * P],
                                        identity=s_ident32[:1, :1])
                    nc.vector.tensor_copy(becol[:, k:k + 1], ptb[:])
                pCp_cm.__exit__(None, None, None)

                # ======== Phase D: MLP (x2 overwrites x1T in place) ======
                with tc.tile_pool(name="pD", bufs=2) as pD, \
                     tc.tile_pool(name="pDp", bufs=2, space="PSUM") as pDp, \
                     tc.tile_pool(name="pDx", bufs=2, space="PSUM") as pDx:
                    for i in range(NCH):
                        c0 = i * 512
                        px2 = pDx.tile([P, 512], F32, tag='px2')
                        for k in range(HB):
                            ph = pDp.tile([P, 512], F32, tag='ph')
                            nc.tensor.matmul(ph[:], lhsT=w1eff[:, k * P:(k + 1) * P],
                                             rhs=x1T[:, c0:c0 + 512],
                                             start=True, stop=True)
                            hs = pD.tile([P, 512], F32, tag='hs')
                            nc.scalar.activation(out=hs[:], in_=ph[:], func=AF.Relu,
                                                 bias=becol[:, k:k + 1], scale=1.0)
                            nc.tensor.matmul(px2[:], lhsT=sw[f'W2_{l}'][:, k * P:(k + 1) * P],
                                             rhs=hs[:], start=(k == 0), stop=False,
                                             skip_group_check=True)
                        nc.tensor.matmul(px2[:], lhsT=sw[f'b2_{l}'][:],
                                         rhs=s_ones_row[:], start=False, stop=True,
                                         skip_group_check=True)
                        nc.vector.tensor_add(x1T[:, c0:c0 + 512], px2[:],
                                             x1T[:, c0:c0 + 512])

                # ======== Phase E: graph LayerNorm (+ fused layer-1 table
                # production and AllGather when l == 0) ====================
                with tc.tile_pool(name="pE", bufs=2) as pE, \
                     tc.tile_pool(name="pEg", bufs=1, space="PSUM") as pEgp, \
                     tc.tile_pool(name="pEp", bufs=1, space="PSUM") as pEp:
                    pgs = pEgp.tile([gpc, 2], F32, tag='pgs')
                    for w in range(W):
                        sl = slice(w * P, (w + 1) * P)
                        sq = pE.tile([P, P], F16, tag='sq')
                        nc.scalar.activation(out=sq[:], in_=x1T[:, sl],
                                             func=AF.Square)
                        pcs = pEp.tile([1, 2 * P], F32, tag='pcs')
                        nc.tensor.matmul(pcs[:, 0:P], lhsT=s_ones_col16[:], rhs=x1T[:, sl],
                                         start=True, stop=True, skip_group_check=True)
                        nc.tensor.matmul(pcs[:, P:2 * P], lhsT=s_ones_col16[:], rhs=sq[:],
                                         start=True, stop=True, skip_group_check=True)
                        rows = pE.tile([1, 2 * P], F32, tag='rows')
                        nc.vector.tensor_copy(rows[:], pcs[:])
                        csc = pE.tile([P, 2], F32, tag='csc')
                        for q in range(2):
                            ptb = pEp.tile([P, 1], F32, tag='ptb2')
                            nc.tensor.transpose(out=ptb[:], in_=rows[:, q * P:(q + 1) * P],
                                                identity=s_ident32[:1, :1])
                            nc.vector.tensor_copy(csc[:, q:q + 1], ptb[:])
                        bg = pE.tile([P, gpc], F32, tag='bg')
                        nc.vector.tensor_scalar(out=bg[:], in0=s_giota_rep[:],
                                                scalar1=s_batch_col[:, w:w + 1],
                                                scalar2=None, op0=ALU.is_equal)
                        nc.tensor.matmul(pgs[:], lhsT=bg[:], rhs=csc[:],
                                         start=(w == 0), stop=(w == W - 1),
                                         skip_group_check=True)
                    gm = pE.tile([gpc, 1], F32, tag='gm')
                    nc.vector.tensor_tensor(out=gm[:], in0=pgs[:, 0:1],
                                            in1=s_invcntD[:], op=ALU.mult)
                    e2 = pE.tile([gpc, 1], F32, tag='e2')
                    nc.vector.tensor_tensor(out=e2[:], in0=pgs[:, 1:2],
                                            in1=s_invcntD[:], op=ALU.mult)
                    gv = pE.tile([gpc, 1], F32, tag='gv')
                    nc.vector.tensor_tensor(out=gv[:], in0=gm[:], in1=gm[:], op=ALU.mult)
                    nc.vector.tensor_tensor(out=gv[:], in0=e2[:], in1=gv[:], op=ALU.subtract)
                    sdg = pE.tile([gpc, 1], F32, tag='sdg')
                    nc.vector.tensor_scalar(out=gv[:], in0=gv[:], scalar1=EPS,
                                            scalar2=None, op0=ALU.add)
                    nc.scalar.activation(out=sdg[:], in_=gv[:], func=AF.Sqrt)
                    ivg = pE.tile([gpc, 1], F32, tag='ivg')
                    nc.vector.reciprocal(ivg[:], sdg[:])
                    gmr = pE.tile([gpc, P], F32, tag='gmr')
                    nc.vector.tensor_copy(gmr[:], gm[:].to_broadcast([gpc, P]))
                    ivr = pE.tile([gpc, P], F32, tag='ivr')
                    nc.vector.tensor_copy(ivr[:], ivg[:].to_broadcast([gpc, P]))
                    for i in range(NCH):
                        c0 = i * 512
                        brc = pE.tile([1, 512], F16, tag='brc')
                        nc.sync.dma_start(brc[:], t_batch_row[:, c0:c0 + 512])
                        pbr = pEp.tile([gpc, 512], F32, tag='pbr')
                        nc.tensor.matmul(pbr[:],
                                         lhsT=s_ones16[:, 0:1].to_broadcast([1, gpc]),
                                         rhs=brc[:],
                                         start=True, stop=True)
                        bgT = pE.tile([gpc, 512], F32, tag='bgT')
                        nc.vector.tensor_scalar(out=bgT[:], in0=pbr[:],
                                                scalar1=s_giota_col[:],
                                                scalar2=None, op0=ALU.is_equal)
                        pgm = pEp.tile([P, 512], F32, tag='pgm')
                        nc.tensor.matmul(pgm[:], lhsT=gmr[:], rhs=bgT[:],
                                         start=True, stop=True)
                        piv = pEp.tile([P, 512], F32, tag='piv')
                        nc.tensor.matmul(piv[:], lhsT=ivr[:], rhs=bgT[:],
                                         start=True, stop=True)
                        tmp = pE.tile([P, 512], F32, tag='tmp')
                        nc.vector.tensor_tensor(out=tmp[:], in0=x1T[:, c0:c0 + 512],
                                                in1=pgm[:], op=ALU.subtract)
                        nc.vector.tensor_tensor(out=tmp[:], in0=tmp[:],
                                                in1=piv[:], op=ALU.mult)
                        if l == 0:
                            nc.vector.tensor_scalar(out=x3T[:, c0:c0 + 512], in0=tmp[:],
                                                    scalar1=sw[f'lng{l}'][:],
                                                    scalar2=sw[f'lnb{l}'][:],
                                                    op0=ALU.mult, op1=ALU.add)
                            # fused layer-1 gather-table production
                            xa = pE.tile([P, 4 * D], F16, tag='xa')
                            for q in range(4):
                                pxa = pEp.tile([P, D], F32, tag='pxa')
                                nc.tensor.matmul(
                                    pxa[:], lhsT=x3T[:, c0 + q * P:c0 + (q + 1) * P],
                                    rhs=sw['WlA1'][:], start=True, stop=False)
                                nc.tensor.matmul(
                                    pxa[:], lhsT=s_ones16[:, 0:1].to_broadcast([1, P]),
                                    rhs=sw['blA1'][:], start=False, stop=True)
                                nc.vector.tensor_copy(xa[:, q * D:(q + 1) * D], pxa[:])
                            nc.sync.dma_start(
                                d_xl1loc_r[rep % NB][c0:c0 + 512, 0:D].rearrange(
                                    "(q p) d -> p q d", p=P),
                                xa[:].rearrange("p (q d) -> p q d", d=D))
                        else:
                            x3c = pE.tile([P, 512], F16, tag='x3c')
                            nc.vector.tensor_scalar(out=x3c[:], in0=tmp[:],
                                                    scalar1=sw[f'lng{l}'][:],
                                                    scalar2=sw[f'lnb{l}'][:],
                                                    op0=ALU.mult, op1=ALU.add)
                            for q in range(4):
                                ptb2 = pEp.tile([P, P], F16, tag='ptb2')
                                nc.tensor.transpose(out=ptb2[:],
                                                    in_=x3c[:, q * P:(q + 1) * P],
                                                    identity=s_ident[:])
                                orow = pE.tile([P, P], F32, tag='orow')
                                nc.vector.tensor_copy(orow[:], ptb2[:])
                                r0 = c0 + q * P
                                nc.sync.dma_start(t_out[r0:r0 + P, :], orow[:])
                if l == 0:
                    nc.gpsimd.collective_compute(
                        "AllGather", ALU.bypass,
                        replica_groups=[list(range(NCORE))],
                        ins=[d_xl1loc_r[rep % NB][:].opt()],
                        outs=[t_xl1full[rep % NB][:].opt()])
                ctx.close()

            for r in range(REPS):
                for l in range(L):
                    ctx, pC, pCs = emit_B(r, l)
                    emit_CDE(r, l, ctx, pC, pCs)

    nc.compile()
    return nc


# ---------------------------------------------------------------- entry point
_CACHE = {}


def kernel(**inputs):
    in_maps, dims = host_prep(**inputs)
    key = (REPS, dims['N'], dims['E'], dims['N_pad'], dims['nT'],
           tuple(dims['T_w']), tuple(dims['tA_w']))
    if key not in _CACHE:
        _CACHE[key] = build_nc(dims)
    nc = _CACHE[key]
    res = run_bass_kernel_spmd(nc, in_maps, core_ids=list(range(NCORE)), trace=False)
    global _last_res, _last_dims
    _last_res, _last_dims = res, dims
    N, D = dims['N'], dims['D']
    out = np.zeros((N, D), dtype=np.float32)
    n0s, Nl = dims['n0s'], dims['Nl']
    inv = dims['inv_perm']
    for c in range(NCORE):
        out[n0s[c]:n0s[c + 1]] = res.results[c]['out_rows'][:Nl[c]][:, inv]
    return out


# revision 17
# speedup vs baseline: 1.3054x; 1.0164x over previous
"""GATv2 block (2 layers) on 8 Trainium2 NeuronCores via Bass/Tile — v3.

Structure vs v2 baseline:
- Edge source features gathered with bulk dma_gather (split-table for int16
  index range), table rows padded to 512B.
- Destination transform xr never round-trips DRAM: per-window xr tile stays in
  SBUF and is injected into the per-edge PSUM via the transpose ST of the
  aggregation one-hot S.
- v4: the leaky-relu runs directly on the Activation engine (AF.Lrelu with
  alpha=0.2), so gather-table rows carry plain x@W (128 fp16 = 256B): half the
  gather + AllGather traffic of the 0.6z+0.4|z| linear-rider scheme, and one
  ACT op per PSUM group instead of two (fewer activation-table switches).
- v4: edge channels are stored c-major (new col = c*H + h) so the exp*xs
  multiply and the softmax divide hit the DVE packed fast path; weights are
  permuted host-side and the output is unpermuted in kernel().
- layer-1 gather table (xl1 = x3 @ Wl1) is produced chunk-by-chunk inside
  layer-0's LayerNorm phase and AllGathered into a Shared-address DRAM tensor.
- host_prep fully vectorized (sort-by-(core,window,split) + scatter).

softmax num/den accumulate via one-hot segment matmul; BN stats via Gram
matrix AllReduce; graph-LN via one-hot segment matmuls.
"""
import sys
import math

sys.path.insert(0, '/opt/trn_rl_repo')

import numpy as np
import concourse.bass as bass
import concourse.tile as tile
from concourse import bacc, mybir
from concourse.bass_utils import run_bass_kernel_spmd

F32 = mybir.dt.float32
F16 = mybir.dt.float16
I16 = mybir.dt.int16
AF = mybir.ActivationFunctionType
ALU = mybir.AluOpType

P = 128
NCORE = 8
NEG = 0.2
EPS = 1e-5
ASHIFT = -4.0   # constant softmax shift: exp(alpha-4) keeps fp16 exp in range
EL = 128        # fp16 elements per gather-table row (256B)
SPLIT = 32768   # int16 index split point
REPS = 1
GMAX = 896      # max idxs per dma_gather op


# ----------------------------------------------------------------- host prep
def host_prep(x, node_batch, edge_index, edge_attr, Wl, bl, Wr, br, We, att,
              bias, Wres, W1, b1, bn_gamma, bn_beta, W2, b2, ln_gamma, ln_beta):
    N, D = x.shape
    E = edge_index.shape[1]
    ED = edge_attr.shape[1]
    L = Wl.shape[0]
    HID = W1.shape[2]
    G = int(node_batch.max()) + 1
    H = att.shape[1]
    C = att.shape[2]
    DA = D + H
    gpc = (G + NCORE - 1) // NCORE

    nb = np.asarray(node_batch).astype(np.int64)
    src = np.asarray(edge_index[0]).astype(np.int64)
    dst = np.asarray(edge_index[1]).astype(np.int64)
    ea = np.asarray(edge_attr, dtype=np.float32)
    xf = np.asarray(x, np.float32)

    gb = np.searchsorted(nb, np.arange(G + 1))
    n0s = np.array([gb[min(c * gpc, G)] for c in range(NCORE + 1)], dtype=np.int64)
    Nl = n0s[1:] - n0s[:-1]
    N_pad = int(math.ceil(max(Nl.max(), 1) / 512.0) * 512)
    W = N_pad // P
    NCH = N_pad // 512
    NPT = NCORE * N_pad

    core_of = np.searchsorted(n0s, np.arange(N), side='right') - 1
    glob_id = (core_of * N_pad + (np.arange(N) - n0s[core_of])).astype(np.int64)

    ecore = core_of[dst]
    gsrc = glob_id[src]
    dslot_all = dst - n0s[ecore]
    ewin_all = dslot_all // P
    eslot_all = dslot_all % P
    isB = (gsrc >= SPLIT).astype(np.int64)

    # Per (core, window, split): counts -> shared tile layout (max over cores).
    key = (ecore * W + ewin_all) * 2 + isB
    cnt2 = np.bincount(key, minlength=NCORE * W * 2).reshape(NCORE, W, 2)
    nA, nB = cnt2[..., 0], cnt2[..., 1]
    tA_w = np.maximum(np.ceil(nA.max(axis=0) / P).astype(np.int64), 1)
    tB_w = np.ceil(nB.max(axis=0) / P).astype(np.int64)
    T_w = tA_w + tB_w
    tstart = np.concatenate([[0], np.cumsum(T_w)])
    nT = int(tstart[-1])
    E_pad = nT * P
    tsA = np.concatenate([[0], np.cumsum(tA_w)])
    tsB = np.concatenate([[0], np.cumsum(tB_w)])
    baseA = tstart[:-1] * P
    baseB = baseA + tA_w * P
    colA = np.concatenate([[0], np.cumsum(tA_w * (P // 16))])
    colB = np.concatenate([[0], np.cumsum(tB_w * (P // 16))])
    LA = int(tsA[-1]) * P
    LB = int(tsB[-1]) * P

    # Stable sort by (core, window, split); rank within group gives each edge
    # a unique slot in its window's tile range.
    order = np.argsort(key, kind='stable')
    sk = key[order]
    starts = np.zeros(E, np.int64)
    gs = np.r_[0, np.flatnonzero(np.diff(sk)) + 1]
    starts[gs] = gs
    starts = np.maximum.accumulate(starts)
    rank = np.arange(E) - starts
    wo = ewin_all[order]
    bo = isB[order]
    co = ecore[order]
    pos = np.where(bo == 0, baseA[wo], baseB[wo]) + rank

    # c-major channel permutation: new col j = c*H + h holds old channel
    # h*C + c. Makes the exp*xs multiply and softmax divide DVE-packed.
    perm = np.array([h * C + c for c in range(C) for h in range(H)])
    inv_perm = np.argsort(perm)

    shared = {
        'iota_row': np.tile(np.arange(P, dtype=np.float16), (P, 1)),
        'giota_rep': np.tile(np.arange(gpc, dtype=np.float32), (P, 1)),
        'giota_col': np.arange(gpc, dtype=np.float32).reshape(gpc, 1),
        'ident': np.eye(P, dtype=np.float16),
        'ident32': np.eye(P, dtype=np.float32),
        'ones_col': np.ones((P, 1), np.float32),
        'ones_col16': np.ones((P, 1), np.float16),
        'ones_row': np.ones((1, 512), np.float32),
        'ones16': np.ones((1, P), np.float16),
        'ashift_col': np.full((P, 1), ASHIFT, np.float32),
    }
    WlA_f, blA_f, WrA_f = [], [], []
    for l in range(L):
        Wl_ = np.asarray(Wl[l], np.float32)
        Wr_ = np.asarray(Wr[l], np.float32)
        We_ = np.asarray(We[l], np.float32)
        bl_ = np.asarray(bl[l], np.float32)
        br_ = np.asarray(br[l], np.float32)
        # layer >= 1 inputs live in the permuted basis: permute weight ROWS.
        rp = perm if l >= 1 else np.arange(D)
        Wl_r = Wl_[rp]
        Wr_r = Wr_[rp]
        Wres_r = np.asarray(Wres[l], np.float32)[rp]
        WlA_f.append(Wl_r[:, perm])
        blA_f.append(bl_[perm])
        WrA_f.append(Wr_r[:, perm])
        shared[f'WlA{l}'] = WlA_f[l].astype(np.float16)
        shared[f'blA{l}'] = blA_f[l].reshape(1, D).astype(np.float16)
        shared[f'WrA{l}'] = WrA_f[l].astype(np.float16)
        wex = np.concatenate([We_, br_.reshape(1, D)], 0)
        shared[f'WeX{l}'] = wex[:, perm].astype(np.float16)
        shared[f'Wres{l}'] = Wres_r[:, perm].astype(np.float16)
        shared[f'combo{l}'] = np.asarray(bias[l], np.float32)[perm].astype(np.float16).reshape(1, D)
        # full att vector, c-major: [P, D]
        aC = np.asarray(att[l], np.float32).reshape(H * C)[perm].astype(np.float16)
        shared[f'attC{l}'] = np.tile(aC.reshape(1, D), (P, 1))
        shared[f'W1_{l}'] = np.asarray(W1[l], np.float32)[perm]
        w2 = np.asarray(W2[l], np.float32)[:, perm]
        shared[f'W2_{l}'] = np.concatenate(
            [w2[k * P:(k + 1) * P, :] for k in range(HID // P)], axis=1)
        shared[f'b2_{l}'] = np.asarray(b2[l], np.float32)[perm].reshape(1, D)
        shared[f'bng{l}'] = np.asarray(bn_gamma[l], np.float32).reshape(1, HID)
        shared[f'bnb{l}'] = np.asarray(bn_beta[l], np.float32).reshape(1, HID)
        shared[f'lng{l}'] = np.asarray(ln_gamma[l], np.float32)[perm].reshape(D, 1)
        shared[f'lnb{l}'] = np.asarray(ln_beta[l], np.float32)[perm].reshape(D, 1)

    # layer-0 host precomputes: gather table, xr0, resid0
    xl0 = np.zeros((NPT, EL), np.float16)
    xr0 = np.zeros((NCORE, P, W * D), np.float16)
    rs0 = np.zeros((NCORE, P, W * D), np.float16)
    for c in range(NCORE):
        xs = xf[n0s[c]:n0s[c + 1]]
        xl0[c * N_pad:c * N_pad + Nl[c], :D] = (xs @ WlA_f[0] + blA_f[0]).astype(np.float16)
        xrv = (xs @ WrA_f[0]).astype(np.float16)          # [Nl, D]
        rsv = (xs @ np.asarray(Wres[0], np.float32)[:, perm]
               + np.asarray(bias[0], np.float32)[perm]).astype(np.float16)
        pad_s = np.zeros((N_pad - Nl[c], D), np.float16)
        xr0[c] = np.concatenate([xrv, pad_s]).reshape(W, P, D).transpose(1, 0, 2).reshape(P, W * D)
        rs0[c] = np.concatenate([rsv, pad_s]).reshape(W, P, D).transpose(1, 0, 2).reshape(P, W * D)

    in_maps = []
    arangeP = np.arange(P)
    for c in range(NCORE):
        sel = co == c
        oc = order[sel]
        pc = pos[sel]
        wc = wo[sel]
        bc = bo[sel]
        rc = rank[sel]
        es = gsrc[oc]

        dflat = np.full(E_pad, -1.0, np.float32)
        dflat[pc] = eslot_all[oc]
        ST_h = (dflat[None, :] == arangeP[:, None]).astype(np.float16)
        dc = dflat.reshape(nT, P).T
        S_h = (dc[:, :, None] == arangeP[None, None, :]).astype(
            np.float16).reshape(P, nT * P)

        eaf = np.zeros((ED + 1, E_pad), np.float16)
        eaf[:ED, pc] = ea[oc].T
        eaf[ED, pc] = 1.0

        mA = bc == 0
        idxA_flat = np.zeros(LA, np.int64)
        idxA_flat[tsA[wc[mA]] * P + rc[mA]] = es[mA]
        idxA = np.concatenate(
            [idxA_flat[tsA[w] * P:tsA[w + 1] * P].reshape(-1, 16).T
             for w in range(W)], axis=1).astype(np.int16)
        idxA = np.tile(idxA, (8, 1))
        if LB:
            mB = ~mA
            idxB_flat = np.zeros(LB, np.int64)
            idxB_flat[tsB[wc[mB]] * P + rc[mB]] = es[mB] - SPLIT
            idxB = np.concatenate(
                [idxB_flat[tsB[w] * P:tsB[w + 1] * P].reshape(-1, 16).T
                 for w in range(W) if tB_w[w]], axis=1).astype(np.int16)
            idxB = np.tile(idxB, (8, 1))
        else:
            idxB = np.zeros((P, 16), np.int16)

        lg = nb[n0s[c]:n0s[c + 1]] - c * gpc
        batch = np.full(N_pad, -1.0, np.float32)
        batch[:Nl[c]] = lg.astype(np.float32)
        valid = np.zeros(N_pad, np.float32)
        valid[:Nl[c]] = 1.0
        cnt = np.maximum(gb[np.minimum(c * gpc + np.arange(1, gpc + 1), G)]
                         - gb[np.minimum(c * gpc + np.arange(gpc), G)], 1)
        im = dict(shared)
        im.update({
            'idxA': idxA,
            'idxB': idxB,
            'ST_h': ST_h,
            'S_h': S_h,
            'eaT': eaf,
            'xl0': xl0,
            'xr0': xr0[c],
            'rs0': rs0[c],
            'batch_row': batch.reshape(1, N_pad).astype(np.float16),
            'batch_col': batch.reshape(W, P).T.copy(),
            'valid_col': valid.reshape(W, P).T.copy(),
            'invcntD': (1.0 / (cnt * D)).astype(np.float32).reshape(gpc, 1),
        })
        in_maps.append(im)

    dims = dict(N=N, D=D, E=E, ED=ED, L=L, HID=HID, G=G, H=H, C=C, gpc=gpc,
                N_pad=N_pad, W=W, NCH=NCH, NPT=NPT, nT=nT, E_pad=E_pad,
                T_w=[int(t) for t in T_w], tA_w=[int(t) for t in tA_w],
                tB_w=[int(t) for t in tB_w], tstart=[int(t) for t in tstart],
                colA=[int(t) for t in colA], colB=[int(t) for t in colB],
                nA_cols=int(colA[-1]), nB_cols=max(int(colB[-1]), 16),
                n0s=n0s, Nl=Nl, inv_perm=inv_perm)
    return in_maps, dims


# --------------------------------------------------------------- bass kernel
def build_nc(dims):
    D = dims['D']
    ED = dims['ED']
    L = dims['L']
    HID = dims['HID']
    H = dims['H']
    C = dims['C']
    DA = D + H
    gpc = dims['gpc']
    N_pad = dims['N_pad']
    W = dims['W']
    NCH = dims['NCH']
    NPT = dims['NPT']
    nT = dims['nT']
    E_pad = dims['E_pad']
    T_w = dims['T_w']
    tA_w = dims['tA_w']
    tB_w = dims['tB_w']
    tstart = dims['tstart']
    colA = dims['colA']
    colB = dims['colB']
    N = dims['N']
    HB = HID // P
    tpo = GMAX // P
    NB = min(REPS, 2)

    nc = bacc.Bacc("TRN2", target_bir_lowering=False, debug=False, num_devices=NCORE)

    def inp(name, shape, dt=F32):
        return nc.dram_tensor(name, list(shape), dt, kind="ExternalInput").ap()

    t_idxA = inp('idxA', (P, dims['nA_cols']), I16)
    t_idxB = inp('idxB', (P, dims['nB_cols']), I16)
    t_ST = inp('ST_h', (P, nT * P), F16)
    t_S = inp('S_h', (P, nT * P), F16)
    t_eaT = inp('eaT', (ED + 1, E_pad), F16)
    t_xl0 = inp('xl0', (NPT, EL), F16)
    t_xr0 = inp('xr0', (P, W * D), F16)
    t_rs0 = inp('rs0', (P, W * D), F16)
    t_batch_row = inp('batch_row', (1, N_pad), F16)
    t_batch_col = inp('batch_col', (P, W))
    t_valid_col = inp('valid_col', (P, W))
    t_invcntD = inp('invcntD', (gpc, 1))
    t_iota_row = inp('iota_row', (P, P), F16)
    t_giota_rep = inp('giota_rep', (P, gpc))
    t_giota_col = inp('giota_col', (gpc, 1))
    t_ident = inp('ident', (P, P), F16)
    t_ident32 = inp('ident32', (P, P), F32)
    t_ones_col = inp('ones_col', (P, 1))
    t_ones_col16 = inp('ones_col16', (P, 1), F16)
    t_ones_row = inp('ones_row', (1, 512))
    t_ones16 = inp('ones16', (1, P), F16)
    t_ashift = inp('ashift_col', (P, 1))
    tw = {}
    wspec = []
    for l in range(L):
        wspec += [(f'WlA{l}', (P, D), F16), (f'blA{l}', (1, D), F16),
                  (f'WrA{l}', (P, D), F16), (f'WeX{l}', (ED + 1, D), F16),
                  (f'Wres{l}', (P, D), F16), (f'combo{l}', (1, D), F16),
                  (f'attC{l}', (P, D), F16),
                  (f'W1_{l}', (P, HID), F32), (f'W2_{l}', (P, HID), F32),
                  (f'b2_{l}', (1, D), F32), (f'bng{l}', (1, HID), F32),
                  (f'bnb{l}', (1, HID), F32),
                  (f'lng{l}', (D, 1), F32), (f'lnb{l}', (D, 1), F32)]
    for key, shape, dt in wspec:
        tw[key] = inp(key, shape, dt)

    t_out = nc.dram_tensor('out_rows', [N_pad, D], F32, kind="ExternalOutput").ap()

    # layer-1 gather tables: AllGather output in Shared address space (fast
    # HBM-HBM collective path); input staged in Local scratch.
    t_xl1full = [nc.dram_tensor(f'xl1full{r}', [NPT, EL], F16,
                                kind="Internal", addr_space="Shared").ap()
                 for r in range(NB)]

    with tile.TileContext(nc) as tc:
        with tc.tile_pool(name="const", bufs=1) as cpool, \
             tc.tile_pool(name="dram", bufs=1, space="DRAM") as dpool, \
             tc.tile_pool(name="big", bufs=1) as bigpool:

            def ld(ap, shape, dt=F32, pool=cpool, name=None):
                if name is None:
                    name = 'c_' + ap.tensor.name
                t = pool.tile(list(shape), dt, name=name, tag=name)
                nc.sync.dma_start(t[:], ap[:])
                return t

            s_idxA = ld(t_idxA, (P, dims['nA_cols']), I16, bigpool)
            s_idxB = ld(t_idxB, (P, dims['nB_cols']), I16, bigpool)
            s_batch_col = ld(t_batch_col, (P, W))
            s_valid_col = ld(t_valid_col, (P, W))
            s_invcntD = ld(t_invcntD, (gpc, 1))
            s_iota_row = ld(t_iota_row, (P, P), F16)
            s_giota_rep = ld(t_giota_rep, (P, gpc))
            s_giota_col = ld(t_giota_col, (gpc, 1))
            s_ident = ld(t_ident, (P, P), F16)
            s_ident32 = ld(t_ident32, (P, P), F32)
            s_ones_col = ld(t_ones_col, (P, 1))
            s_ones_col16 = ld(t_ones_col16, (P, 1), F16)
            s_ones_row = ld(t_ones_row, (1, 512))
            s_ones16 = ld(t_ones16, (1, P), F16)
            s_ashift = ld(t_ashift, (P, 1))
            sw = {}
            for key, shape, dt in wspec:
                sw[key] = ld(tw[key], shape, dt)

            d_xl1loc_r = [dpool.tile([N_pad, EL], F16, tag=f'xl1loc{r}',
                                     name=f'd_xl1loc{r}') for r in range(NB)]
            d_rs1 = [dpool.tile([P, (N_pad // P) * D], F16, tag=f'rs1_{r}',
                                name=f'd_rs1_{r}') for r in range(NB)]
            d_arin = [dpool.tile([P, D + 1], F32, tag=f'arin{l}', name=f'd_arin{l}')
                      for l in range(L)]
            d_arout = [dpool.tile([P, D + 1], F32, tag=f'arout{l}', name=f'd_arout{l}')
                       for l in range(L)]

            x1Tb = [bigpool.tile([P, N_pad], F16, tag=f'x1T{i}', name=f'x1T{i}')
                    for i in range(NB)]
            x3Tb = [bigpool.tile([P, N_pad], F16, tag=f'x3T{i}', name=f'x3T{i}')
                    for i in range(NB)]

            from contextlib import ExitStack

            def emit_B(rep, l):
                x1T = x1Tb[rep % NB]
                x3T = x3Tb[rep % NB]
                tab = t_xl0 if l == 0 else t_xl1full[rep % NB]
                ctx = ExitStack()
                pC = ctx.enter_context(tc.tile_pool(name="pC", bufs=1))
                pCsp = ctx.enter_context(tc.tile_pool(name="pCs", bufs=1, space="PSUM"))
                pCs = pCsp.tile([P, D + 1], F32, tag='cs')
                with tc.tile_pool(name="pB", bufs=3) as pB, \
                     tc.tile_pool(name="pB1", bufs=2) as pB1, \
                     tc.tile_pool(name="pBz", bufs=3, space="PSUM") as pBz, \
                     tc.tile_pool(name="pBa", bufs=2, space="PSUM") as pBa, \
                     tc.tile_pool(name="pBr", bufs=1, space="PSUM") as pBr:
                    if l == 1:
                        # all windows' dst transforms + residuals up front:
                        # independent of the gather table, so PE computes them
                        # during the AllGather instead of stalling behind
                        # window 0. Residuals stage through DRAM (no SBUF
                        # budget for a second resident [P, W*D] tile).
                        xr1T = pC.tile([P, W * D], F16, tag='xr1T')
                        for w2 in range(W):
                            pxr = pBr.tile([P, D], F32, tag='pxr')
                            nc.tensor.matmul(pxr[:], lhsT=x3T[:, w2 * P:(w2 + 1) * P],
                                             rhs=sw['WrA1'][:], start=True, stop=True)
                            nc.scalar.activation(out=xr1T[:, w2 * D:(w2 + 1) * D],
                                                 in_=pxr[:], func=AF.Copy)
                            prs = pBr.tile([P, D], F32, tag='pxr')
                            nc.tensor.matmul(prs[:], lhsT=x3T[:, w2 * P:(w2 + 1) * P],
                                             rhs=sw['Wres1'][:], start=True, stop=False)
                            nc.tensor.matmul(prs[:], lhsT=s_ones16[:, 0:1].to_broadcast([1, P]),
                                             rhs=sw['combo1'][:], start=False, stop=True)
                            rstg = pB.tile([P, D], F16, tag='rstg')
                            nc.scalar.activation(out=rstg[:], in_=prs[:], func=AF.Copy)
                            nc.sync.dma_start(
                                d_rs1[rep % NB][:, w2 * D:(w2 + 1) * D], rstg[:])
                    for w in range(W):
                        T = T_w[w]
                        tA = tA_w[w]
                        tB = tB_w[w]
                        tb = tstart[w]
                        EW = T * P
                        eat = pB.tile([ED + 1, EW], F16, tag='eat')
                        nc.sync.dma_start(eat[:], t_eaT[:, tb * P:tb * P + EW])
                        xsv = pB.tile([P, T * EL], F16, tag='xsv')
                        xsr = xsv[:].rearrange("p (t q) -> p t q", q=EL)
                        for o in range(0, tA, tpo):
                            t0, t1 = o, min(o + tpo, tA)
                            ni = (t1 - t0) * P
                            nc.gpsimd.dma_gather(
                                xsr[:, t0:t1, :], tab,
                                s_idxA[:, (colA[w] + t0 * 8):(colA[w] + t1 * 8)],
                                ni, ni, EL)
                        for o in range(0, tB, tpo):
                            t0, t1 = o, min(o + tpo, tB)
                            ni = (t1 - t0) * P
                            nc.gpsimd.dma_gather(
                                xsr[:, tA + t0:tA + t1, :], tab[SPLIT:, :],
                                s_idxB[:, (colB[w] + t0 * 8):(colB[w] + t1 * 8)],
                                ni, ni, EL)
                        rsw = pB.tile([P, D], F16, tag='rsw')
                        if l == 0:
                            xrw = pB.tile([P, D], F16, tag='xrw')
                            nc.sync.dma_start(xrw[:], t_xr0[:, w * D:(w + 1) * D])
                            nc.sync.dma_start(rsw[:], t_rs0[:, w * D:(w + 1) * D])
                            xrw_ap = xrw[:]
                        else:
                            nc.sync.dma_start(rsw[:],
                                              d_rs1[rep % NB][:, w * D:(w + 1) * D])
                            xrw_ap = xr1T[:, w * D:(w + 1) * D]
                        # S / ST one-hots: host-built, DMA-loaded
                        S = pB1.tile([P, EW], F16, tag='S')
                        nc.sync.dma_start(S[:], t_S[:, tb * P:tb * P + EW])
                        ST = pB1.tile([P, EW], F16, tag='ST')
                        nc.sync.dma_start(ST[:], t_ST[:, tb * P:tb * P + EW])
                        m16 = pB1.tile([P, T * D], F16, tag='m16')
                        KP = 4
                        for g0 in range(0, T, KP):
                            gn = min(KP, T - g0)
                            pz3 = pBz.tile([P, KP * D], F32, tag='pz')
                            for u in range(gn):
                                t = g0 + u
                                sl = slice(u * D, (u + 1) * D)
                                nc.tensor.matmul(pz3[:, sl], lhsT=eat[:, t * P:(t + 1) * P],
                                                 rhs=sw[f'WeX{l}'][:], start=True, stop=False,
                                                 skip_group_check=True)
                                nc.tensor.matmul(pz3[:, sl], lhsT=ST[:, t * P:(t + 1) * P],
                                                 rhs=xrw_ap,
                                                 start=False, stop=True,
                                                 skip_group_check=True)
                            pzv = pz3[:].rearrange("p (u q) -> p u q", q=D)
                            mv = m16[:, g0 * D:(g0 + gn) * D].rearrange("p (u n) -> p u n", n=D)
                            nc.vector.tensor_tensor(out=mv, in0=pzv[:, 0:gn, :],
                                                    in1=xsr[:, g0:g0 + gn, 0:D],
                                                    op=ALU.add)
                            nc.scalar.activation(out=mv, in_=mv, func=AF.Prelu, alpha=NEG)
                        eng_tt = nc.vector if (w % 3 != 2) else nc.gpsimd
                        eng_tt.tensor_tensor(
                            out=m16[:].rearrange("p (t n) -> p t n", t=T),
                            in0=m16[:].rearrange("p (t n) -> p t n", t=T),
                            in1=sw[f'attC{l}'][:].rearrange("p (o n) -> p o n", o=1).to_broadcast([P, T, P]),
                            op=ALU.mult)
                        alpha2 = pB.tile([P, T * H], F32, tag='alpha2')
                        with nc.allow_low_precision(reason="16 fp16 terms, |alpha|<~30"):
                            nc.vector.tensor_reduce(
                                out=alpha2[:],
                                in_=m16[:].rearrange("p (t c h) -> p t h c", c=C, h=H),
                                axis=mybir.AxisListType.X, op=ALU.add)
                        ybuf = pB.tile([P, T * (D + 8)], F16, tag='ybuf')
                        yv = ybuf[:].rearrange("p (t q) -> p t q", q=D + 8)
                        nc.scalar.activation(
                            out=yv[:, :, D:D + 8],
                            in_=alpha2[:].rearrange("p (t h) -> p t h", t=T),
                            func=AF.Exp, bias=s_ashift[:])
                        nc.vector.tensor_tensor(
                            out=yv[:, :, 0:D].rearrange("p t (c h) -> p t c h", c=C),
                            in0=xsr[:, :, 0:D].rearrange("p t (c h) -> p t c h", c=C),
                            in1=yv[:, :, D:D + 8].rearrange("p t (o h) -> p t o h", o=1).to_broadcast([P, T, C, H]),
                            op=ALU.mult)
                        pagg = pBa.tile([P, D + 8], F32, tag='pagg')
                        for t in range(T):
                            nc.tensor.matmul(pagg[:], lhsT=S[:, t * P:(t + 1) * P],
                                             rhs=yv[:, t, :], start=(t == 0),
                                             stop=(t == T - 1))
                        den = pB.tile([P, H], F32, tag='den')
                        nc.vector.tensor_scalar(out=den[:], in0=pagg[:, D:D + 8],
                                                scalar1=1e-16, scalar2=None, op0=ALU.add)
                        rec = pB.tile([P, H], F32, tag='rec')
                        nc.vector.reciprocal(rec[:], den[:])
                        x1w = pB.tile([P, D + 1], F16, tag='x1w')
                        nc.vector.tensor_tensor(
                            out=x1w[:, 0:D].rearrange("p (c h) -> p c h", c=C),
                            in0=pagg[:, 0:D].rearrange("p (c h) -> p c h", c=C),
                            in1=rec[:].rearrange("p (o h) -> p o h", o=1).to_broadcast([P, C, H]),
                            op=ALU.mult)
                        nc.vector.tensor_add(x1w[:, 0:D], x1w[:, 0:D],
                                             rsw[:])
                        nc.vector.tensor_scalar(out=x1w[:, 0:D], in0=x1w[:, 0:D],
                                                scalar1=s_valid_col[:, w:w + 1],
                                                scalar2=None, op0=ALU.mult)
                        nc.vector.tensor_copy(x1w[:, D:D + 1], s_valid_col[:, w:w + 1])
                        nc.tensor.matmul(pCs[:], lhsT=x1w[:, 0:D], rhs=x1w[:, 0:D + 1],
                                         start=(w == 0), stop=(w == W - 1),
                                         skip_group_check=True)
                        ptr = pBr.tile([P, P], F16, tag='ptr')
                        nc.tensor.transpose(out=ptr[:], in_=x1w[:, 0:D], identity=s_ident[:])
                        nc.scalar.activation(out=x1T[:, w * P:(w + 1) * P],
                                             in_=ptr[:], func=AF.Copy)
                return ctx, pC, pCs

            def emit_CDE(rep, l, ctx, pC, pCs):
                x1T = x1Tb[rep % NB]
                x3T = x3Tb[rep % NB]
                # ======== Phase C: BN stats (AllReduce) ==================
                pCp_cm = tc.tile_pool(name="pCp", bufs=1, space="PSUM")
                pCp = pCp_cm.__enter__()
                cs_sb = pC.tile([P, D + 1], F32, tag='cs')
                nc.vector.tensor_copy(cs_sb[:], pCs[:])
                nc.sync.dma_start(d_arin[l][:], cs_sb[:])
                nc.gpsimd.collective_compute(
                    "AllReduce", ALU.add,
                    replica_groups=[list(range(NCORE))],
                    ins=[d_arin[l][:].opt()], outs=[d_arout[l][:].opt()])
                csr = pC.tile([P, D + 1], F32, tag='csr')
                nc.sync.dma_start(csr[:], d_arout[l][:])
                mu = pC.tile([P, 1], F32, tag='mu')
                nc.vector.tensor_scalar(out=mu[:], in0=csr[:, D:D + 1],
                                        scalar1=1.0 / N, scalar2=None, op0=ALU.mult)
                pmu = pCp.tile([1, HID], F32, tag='pmu')
                nc.tensor.matmul(pmu[:], lhsT=mu[:], rhs=sw[f'W1_{l}'][:],
                                 start=True, stop=True)
                pP1 = pCp.tile([P, HID], F32, tag='pP1')
                nc.tensor.matmul(pP1[:], lhsT=csr[:, 0:D], rhs=sw[f'W1_{l}'][:],
                                 start=True, stop=True)
                w1p1 = pC.tile([P, HID], F32, tag='w1p1')
                nc.vector.tensor_tensor(out=w1p1[:], in0=sw[f'W1_{l}'][:],
                                        in1=pP1[:], op=ALU.mult)
                pt2 = pCp.tile([1, HID], F32, tag='pt2')
                nc.tensor.matmul(pt2[:], lhsT=s_ones_col[:], rhs=w1p1[:],
                                 start=True, stop=True)
                mh = pC.tile([1, HID], F32, tag='mh')
                nc.vector.tensor_copy(mh[:], pmu[:])
                var = pC.tile([1, HID], F32, tag='var')
                nc.vector.tensor_scalar(out=var[:], in0=pt2[:], scalar1=1.0 / N,
                                        scalar2=None, op0=ALU.mult)
                m2 = pC.tile([1, HID], F32, tag='m2')
                nc.vector.tensor_tensor(out=m2[:], in0=mh[:], in1=mh[:], op=ALU.mult)
                nc.vector.tensor_tensor(out=var[:], in0=var[:], in1=m2[:], op=ALU.subtract)
                sd = pC.tile([1, HID], F32, tag='sd')
                nc.vector.tensor_scalar(out=var[:], in0=var[:], scalar1=EPS,
                                        scalar2=None, op0=ALU.add)
                nc.scalar.activation(out=sd[:], in_=var[:], func=AF.Sqrt)
                rsd = pC.tile([1, HID], F32, tag='rsd')
                nc.vector.reciprocal(rsd[:], sd[:])
                geff = pC.tile([1, HID], F32, tag='geff')
                nc.vector.tensor_tensor(out=geff[:], in0=sw[f'bng{l}'][:],
                                        in1=rsd[:], op=ALU.mult)
                beff = pC.tile([1, HID], F32, tag='beff')
                nc.vector.tensor_tensor(out=beff[:], in0=mh[:], in1=geff[:], op=ALU.mult)
                nc.vector.tensor_tensor(out=beff[:], in0=sw[f'bnb{l}'][:],
                                        in1=beff[:], op=ALU.subtract)
                pgrep = pCp.tile([P, HID], F32, tag='pgrep')
                nc.tensor.matmul(pgrep[:], lhsT=s_ones_col[:1, :].rearrange("o p -> p o").to_broadcast([1, P]),
                                 rhs=geff[:], start=True, stop=True)
                w1eff = pC.tile([P, HID], F16, tag='w1eff')
                nc.vector.tensor_tensor(out=w1eff[:], in0=sw[f'W1_{l}'][:],
                                        in1=pgrep[:], op=ALU.mult)
                becol = pC.tile([P, HB], F32, tag='becol')
                for k in range(HB):
                    ptb = pCp.tile([P, 1], F32, tag='ptb')
                    nc.tensor.transpose(out=ptb[:], in_=beff[:, k * P:(k + 1) * P],
                                        identity=s_ident32[:1, :1])
                    nc.vector.tensor_copy(becol[:, k:k + 1], ptb[:])
                pCp_cm.__exit__(None, None, None)

                # ======== Phase D: MLP (x2 overwrites x1T in place) ======
                with tc.tile_pool(name="pD", bufs=2) as pD, \
                     tc.tile_pool(name="pDp", bufs=2, space="PSUM") as pDp, \
                     tc.tile_pool(name="pDx", bufs=2, space="PSUM") as pDx:
                    for i in range(NCH):
                        c0 = i * 512
                        px2 = pDx.tile([P, 512], F32, tag='px2')
                        for k in range(HB):
                            ph = pDp.tile([P, 512], F32, tag='ph')
                            nc.tensor.matmul(ph[:], lhsT=w1eff[:, k * P:(k + 1) * P],
                                             rhs=x1T[:, c0:c0 + 512],
                                             start=True, stop=True)
                            hs = pD.tile([P, 512], F32, tag='hs')
                            nc.scalar.activation(out=hs[:], in_=ph[:], func=AF.Relu,
                                                 bias=becol[:, k:k + 1], scale=1.0)
                            nc.tensor.matmul(px2[:], lhsT=sw[f'W2_{l}'][:, k * P:(k + 1) * P],
                                             rhs=hs[:], start=(k == 0), stop=False,
                                             skip_group_check=True)
                        nc.tensor.matmul(px2[:], lhsT=sw[f'b2_{l}'][:],
                                         rhs=s_ones_row[:], start=False, stop=True,
                                         skip_group_check=True)
                        nc.vector.tensor_add(x1T[:, c0:c0 + 512], px2[:],
                                             x1T[:, c0:c0 + 512])

                # ======== Phase E: graph LayerNorm (+ fused layer-1 table
                # production and AllGather when l == 0) ====================
                with tc.tile_pool(name="pE", bufs=2) as pE, \
                     tc.tile_pool(name="pEg", bufs=1, space="PSUM") as pEgp, \
                     tc.tile_pool(name="pEp", bufs=1, space="PSUM") as pEp:
                    pgs = pEgp.tile([gpc, 2], F32, tag='pgs')
                    for w in range(W):
                        sl = slice(w * P, (w + 1) * P)
                        sq = pE.tile([P, P], F16, tag='sq')
                        nc.scalar.activation(out=sq[:], in_=x1T[:, sl],
                                             func=AF.Square)
                        pcs = pEp.tile([1, 2 * P], F32, tag='pcs')
                        nc.tensor.matmul(pcs[:, 0:P], lhsT=s_ones_col16[:], rhs=x1T[:, sl],
                                         start=True, stop=True, skip_group_check=True)
                        nc.tensor.matmul(pcs[:, P:2 * P], lhsT=s_ones_col16[:], rhs=sq[:],
                                         start=True, stop=True, skip_group_check=True)
                        rows = pE.tile([1, 2 * P], F32, tag='rows')
                        nc.vector.tensor_copy(rows[:], pcs[:])
                        csc = pE.tile([P, 2], F32, tag='csc')
                        for q in range(2):
                            ptb = pEp.tile([P, 1], F32, tag='ptb2')
                            nc.tensor.transpose(out=ptb[:], in_=rows[:, q * P:(q + 1) * P],
                                                identity=s_ident32[:1, :1])
                            nc.vector.tensor_copy(csc[:, q:q + 1], ptb[:])
                        bg = pE.tile([P, gpc], F32, tag='bg')
                        nc.vector.tensor_scalar(out=bg[:], in0=s_giota_rep[:],
                                                scalar1=s_batch_col[:, w:w + 1],
                                                scalar2=None, op0=ALU.is_equal)
                        nc.tensor.matmul(pgs[:], lhsT=bg[:], rhs=csc[:],
                                         start=(w == 0), stop=(w == W - 1),
                                         skip_group_check=True)
                    gm = pE.tile([gpc, 1], F32, tag='gm')
                    nc.vector.tensor_tensor(out=gm[:], in0=pgs[:, 0:1],
                                            in1=s_invcntD[:], op=ALU.mult)
                    e2 = pE.tile([gpc, 1], F32, tag='e2')
                    nc.vector.tensor_tensor(out=e2[:], in0=pgs[:, 1:2],
                                            in1=s_invcntD[:], op=ALU.mult)
                    gv = pE.tile([gpc, 1], F32, tag='gv')
                    nc.vector.tensor_tensor(out=gv[:], in0=gm[:], in1=gm[:], op=ALU.mult)
                    nc.vector.tensor_tensor(out=gv[:], in0=e2[:], in1=gv[:], op=ALU.subtract)
                    sdg = pE.tile([gpc, 1], F32, tag='sdg')
                    nc.vector.tensor_scalar(out=gv[:], in0=gv[:], scalar1=EPS,
                                            scalar2=None, op0=ALU.add)
                    nc.scalar.activation(out=sdg[:], in_=gv[:], func=AF.Sqrt)
                    ivg = pE.tile([gpc, 1], F32, tag='ivg')
                    nc.vector.reciprocal(ivg[:], sdg[:])
                    gmr = pE.tile([gpc, P], F32, tag='gmr')
                    nc.vector.tensor_copy(gmr[:], gm[:].to_broadcast([gpc, P]))
                    ivr = pE.tile([gpc, P], F32, tag='ivr')
                    nc.vector.tensor_copy(ivr[:], ivg[:].to_broadcast([gpc, P]))
                    for i in range(NCH):
                        c0 = i * 512
                        brc = pE.tile([1, 512], F16, tag='brc')
                        nc.sync.dma_start(brc[:], t_batch_row[:, c0:c0 + 512])
                        pbr = pEp.tile([gpc, 512], F32, tag='pbr')
                        nc.tensor.matmul(pbr[:],
                                         lhsT=s_ones16[:, 0:1].to_broadcast([1, gpc]),
                                         rhs=brc[:],
                                         start=True, stop=True)
                        bgT = pE.tile([gpc, 512], F32, tag='bgT')
                        nc.vector.tensor_scalar(out=bgT[:], in0=pbr[:],
                                                scalar1=s_giota_col[:],
                                                scalar2=None, op0=ALU.is_equal)
                        pgm = pEp.tile([P, 512], F32, tag='pgm')
                        nc.tensor.matmul(pgm[:], lhsT=gmr[:], rhs=bgT[:],
                                         start=True, stop=True)
                        piv = pEp.tile([P, 512], F32, tag='piv')
                        nc.tensor.matmul(piv[:], lhsT=ivr[:], rhs=bgT[:],
                                         start=True, stop=True)
                        tmp = pE.tile([P, 512], F32, tag='tmp')
                        nc.vector.tensor_tensor(out=tmp[:], in0=x1T[:, c0:c0 + 512],
                                                in1=pgm[:], op=ALU.subtract)
                        nc.vector.tensor_tensor(out=tmp[:], in0=tmp[:],
                                                in1=piv[:], op=ALU.mult)
                        if l == 0:
                            nc.vector.tensor_scalar(out=x3T[:, c0:c0 + 512], in0=tmp[:],
                                                    scalar1=sw[f'lng{l}'][:],
                                                    scalar2=sw[f'lnb{l}'][:],
                                                    op0=ALU.mult, op1=ALU.add)
                            # fused layer-1 gather-table production
                            xa = pE.tile([P, 4 * D], F16, tag='xa')
                            for q in range(4):
                                pxa = pEp.tile([P, D], F32, tag='pxa')
                                nc.tensor.matmul(
                                    pxa[:], lhsT=x3T[:, c0 + q * P:c0 + (q + 1) * P],
                                    rhs=sw['WlA1'][:], start=True, stop=False)
                                nc.tensor.matmul(
                                    pxa[:], lhsT=s_ones16[:, 0:1].to_broadcast([1, P]),
                                    rhs=sw['blA1'][:], start=False, stop=True)
                                nc.vector.tensor_copy(xa[:, q * D:(q + 1) * D], pxa[:])
                            nc.sync.dma_start(
                                d_xl1loc_r[rep % NB][c0:c0 + 512, 0:D].rearrange(
                                    "(q p) d -> p q d", p=P),
                                xa[:].rearrange("p (q d) -> p q d", d=D))
                        else:
                            x3c = pE.tile([P, 512], F16, tag='x3c')
                            nc.vector.tensor_scalar(out=x3c[:], in0=tmp[:],
                                                    scalar1=sw[f'lng{l}'][:],
                                                    scalar2=sw[f'lnb{l}'][:],
                                                    op0=ALU.mult, op1=ALU.add)
                            for q in range(4):
                                ptb2 = pEp.tile([P, P], F16, tag='ptb2')
                                nc.tensor.transpose(out=ptb2[:],
                                                    in_=x3c[:, q * P:(q + 1) * P],
                                                    identity=s_ident[:])
                                orow = pE.tile([P, P], F32, tag='orow')
                                nc.vector.tensor_copy(orow[:], ptb2[:])
                                r0 = c0 + q * P
                                nc.sync.dma_start(t_out[r0:r0 + P, :], orow[:])
                if l == 0:
                    nc.gpsimd.collective_compute(
                        "AllGather", ALU.bypass,
                        replica_groups=[list(range(NCORE))],
                        ins=[d_xl1loc_r[rep % NB][:].opt()],
                        outs=[t_xl1full[rep % NB][:].opt()])
                ctx.close()

            for r in range(REPS):
                for l in range(L):
                    ctx, pC, pCs = emit_B(r, l)
                    emit_CDE(r, l, ctx, pC, pCs)

    nc.compile()
    return nc


# ---------------------------------------------------------------- entry point
_CACHE = {}


def kernel(**inputs):
    in_maps, dims = host_prep(**inputs)
    key = (REPS, dims['N'], dims['E'], dims['N_pad'], dims['nT'],
           tuple(dims['T_w']), tuple(dims['tA_w']))
    if key not in _CACHE:
        _CACHE[key] = build_nc(dims)
    nc = _CACHE[key]
    res = run_bass_kernel_spmd(nc, in_maps, core_ids=list(range(NCORE)), trace=False)
    global _last_res, _last_dims
    _last_res, _last_dims = res, dims
    N, D = dims['N'], dims['D']
    out = np.zeros((N, D), dtype=np.float32)
    n0s, Nl = dims['n0s'], dims['Nl']
    inv = dims['inv_perm']
    for c in range(NCORE):
        out[n0s[c]:n0s[c + 1]] = res.results[c]['out_rows'][:Nl[c]][:, inv]
    return out
